# revision 1
# baseline (speedup 1.0000x reference)
"""Trainium2 Bass kernel for nn_EssentialMatrixEstimator.

Distribution: data-parallel over the N=3072 rows of Pc across 8 cores
(384 rows each).

Math: the (N*M, 9) epipolar design-matrix Gram collapses to small monomial
Grams. Two phases to match reference f32 conditioning:
  A) C_raw = M1^T W M2 (6x6, raw-coordinate monomials); its row/col 5 hold
     the weighted moments that define the Hartley normalizations T1/T2.
  B) rebuild monomials from *centered* coordinates x^ = s(x - c) (no
     cancellation) and redo the 6x6 Gram -> C2; Mmat (9x9) is then a pure
     index expansion Mmat[3p+q,3r+s] = C2[pair(p,r), pair(q,s)].
W is the bidirectional-top3 (+ >0.01) masked score matrix; exact top-3 with
multiplicity via the hardware Max8 instruction. Column thresholds need all
rows: per-core partials are AllGathered and combined with another Max8.
The 50-step power iterations run as rescaled repeated squaring
(M <- 2*(M@M)); scale/sign drop out of the final normalized eigvectors.
Sign-fix dets are provably +1 and omitted. Validated against reference.
"""

import os

os.environ.setdefault("JAX_PLATFORMS", "axon")

import numpy as np

import concourse.bass as bass
import concourse.bass_isa as bass_isa
import concourse.mybir as mybir
import concourse.bacc as bacc
import concourse.tile as tile

NCORES = 8
N = 3072
SH = N // NCORES          # 384 rows per core
RT = SH // 128            # 3 row tiles per core
CB = N // 128             # 24 column blocks
F32 = mybir.dt.float32
AF = mybir.ActivationFunctionType
OP = mybir.AluOpType
AX = mybir.AxisListType

EPS = 1e-8
SQRT2 = 1.4142135623730951
INV_SQRT3 = 1.0 / 1.7320508075688772
T0 = float(np.nextafter(np.float32(0.01), np.float32(1)))  # x > 0.01 == x >= T0
H, W = 64, 64

# how many of the 24 mask-blocks the DVE handles (rest go to gpsimd)
DVE_BLOCKS = 9

# cpack const layout (tensor [9, 36]): column ranges
C_I9H = 0      # I9 * 0.5          [9, 9]
C_ET69 = 9     # E^T selector      [6, 9]
C_I3 = 18      # I3                [3, 3]
C_V09 = 21     # full(1/3)         [9, 1]
C_V06 = 22     # full(1/sqrt3)     [6, 1]
C_SEL1 = 23    # [I3 | 0]          [3, 6]
C_SEL2 = 29    # [0 | I3]          [3, 6]
C_E5 = 35      # e5 selector       [6, 1]

PAIRS = [(0, 0), (0, 1), (0, 2), (1, 1), (1, 2), (2, 2)]


def _pidx():
    d = {}
    for i, (a, b) in enumerate(PAIRS):
        d[(a, b)] = i
        d[(b, a)] = i
    return d


def host_constants(K):
    """Monomial matrices + packed tail constants (all f32, mirrors reference)."""
    idx = np.arange(H * W, dtype=np.float32)
    pix = np.stack([idx % np.float32(W), np.floor(idx / np.float32(W))], -1)
    K_inv = np.linalg.inv(np.asarray(K, np.float32)).astype(np.float32)
    p1h = np.concatenate([pix[:N], np.ones((N, 1), np.float32)], -1)
    pts = (p1h @ K_inv.T)[:, :2].astype(np.float32)  # same grid both sides
    x, y = pts[:, 0], pts[:, 1]
    M = np.stack([x * x, x * y, x, y * y, y, np.ones_like(x)], -1).astype(np.float32)

    cpack = np.zeros((9, 36), np.float32)
    cpack[:9, C_I9H:C_I9H + 9] = 0.5 * np.eye(9, dtype=np.float32)
    pid = _pidx()
    for a in range(3):
        for b in range(3):
            cpack[pid[(a, b)], C_ET69 + 3 * a + b] = 1.0  # ET69[m, 3a+b]
    cpack[:3, C_I3:C_I3 + 3] = np.eye(3, dtype=np.float32)
    cpack[:9, C_V09] = 1.0 / 3.0
    cpack[:6, C_V06] = INV_SQRT3
    cpack[:3, C_SEL1:C_SEL1 + 3] = np.eye(3, dtype=np.float32)
    cpack[:3, C_SEL2 + 3:C_SEL2 + 6] = np.eye(3, dtype=np.float32)
    cpack[5, C_E5] = 1.0
    return M, cpack


def _tile128(a, ntiles):
    """[ntiles*128, F] -> [128, ntiles*F] with [p, t*F+f] = a[t*128+p, f]."""
    F = a.shape[1]
    return np.ascontiguousarray(
        a.reshape(ntiles, 128, F).transpose(1, 0, 2).reshape(128, ntiles * F)
    )


def _act_copy(nc, out, in_, scale=1.0):
    nc.scalar.activation(out, in_, AF.Copy, scale=scale)


def build_nc(repeats=1, no_coll=False, no_tail=False):
    """Build the SPMD 8-core Bass program; returns compiled nc.

    repeats > 1 runs the whole body that many times (timing only).
    no_coll/no_tail disable pieces (timing experiments; wrong outputs)."""
    nc = bacc.Bacc("TRN2", target_bir_lowering=False, debug=False,
                   num_devices=NCORES)

    xin = nc.dram_tensor("xin", [128, RT * N], F32, kind="ExternalInput")
    m1s = nc.dram_tensor("m1s", [128, RT * 6], F32, kind="ExternalInput")
    m2t = nc.dram_tensor("m2t", [128, CB * 6], F32, kind="ExternalInput")
    ident = nc.dram_tensor("ident", [128, 128], F32, kind="ExternalInput")
    cpk = nc.dram_tensor("cpack", [9, 36], F32, kind="ExternalInput")
    out_d = nc.dram_tensor("out", [3, 3], F32, kind="ExternalOutput")

    cp_in = nc.dram_tensor("cp_in", [128, CB * 3], F32)
    cp_out = nc.dram_tensor("cp_out", [NCORES * 128, CB * 3], F32,
                            addr_space="Shared")
    cr_in = nc.dram_tensor("cr_in", [6, 6], F32)
    cr_out = nc.dram_tensor("cr_out", [6, 6], F32, addr_space="Shared")
    c2_in = nc.dram_tensor("c2_in", [6, 6], F32)
    c2_out = nc.dram_tensor("c2_out", [6, 6], F32, addr_space="Shared")
    stage = nc.dram_tensor("stage", [64], F32)
    mshuf = nc.dram_tensor("mshuf", [81], F32)

    groups = [list(range(NCORES))]

    with tile.TileContext(nc) as tc:
        with (
            tc.tile_pool(name="persist", bufs=1) as pp,
            tc.tile_pool(name="scratch", bufs=2) as sp,
            tc.tile_pool(name="ps_pt", bufs=2, space="PSUM") as ps,
            tc.tile_pool(name="ps_acc", bufs=2, space="PSUM") as psa,
            tc.tile_pool(name="ps_c", bufs=1, space="PSUM") as psc,
        ):
            for _rep in range(repeats):
                # ---------- P0: loads ----------
                X = pp.tile([128, RT * N], F32, tag="X")
                for t in range(RT):
                    nc.sync.dma_start(X[:, t * N:(t + 1) * N],
                                      xin[:, t * N:(t + 1) * N])
                m1t_s = pp.tile([128, RT * 6], F32, tag="m1")
                nc.sync.dma_start(m1t_s[:], m1s[:])
                m2t_s = pp.tile([128, CB * 6], F32, tag="m2")
                nc.sync.dma_start(m2t_s[:], m2t[:])
                idn = pp.tile([128, 128], F32, tag="idn")
                nc.sync.dma_start(idn[:], ident[:])
                cps = pp.tile([9, 36], F32, tag="cpk")
                nc.sync.dma_start(cps[:], cpk[:])

                def Xt(t):
                    return X[:, t * N:(t + 1) * N]

                # ---------- P1: row thresholds ----------
                r8 = pp.tile([128, RT * 8], F32, tag="r8")
                for t in range(RT):
                    nc.vector.max(out=r8[:, t * 8:t * 8 + 8], in_=Xt(t))
                trRow = pp.tile([1, SH], F32, tag="trRow")
                for t in range(RT):
                    ptr = ps.tile([1, 128], F32, tag="pt")
                    nc.tensor.transpose(ptr[:], r8[:, t * 8 + 2:t * 8 + 3], idn[:])
                    nc.scalar.activation(trRow[:, t * 128:(t + 1) * 128], ptr[:],
                                         AF.Copy)
                trRow2 = pp.tile([1, SH], F32, tag="trRow2")
                nc.vector.tensor_scalar_max(trRow2[:], trRow[:], T0)
                trB = pp.tile([128, SH], F32, tag="trB")
                nc.gpsimd.partition_broadcast(trB[:], trRow2[:], channels=128)

                # ---------- P2: transposes + column-top3 partials ----------
                XT = pp.tile([128, CB * SH], F32, tag="XT")  # [p=col, (j, r)]
                c8all = pp.tile([128, CB * 8], F32, tag="c8all")
                for j in range(CB):
                    for t in range(RT):
                        pt = ps.tile([128, 128], F32, tag="pt")
                        nc.tensor.transpose(
                            pt[:], Xt(t)[:, j * 128:(j + 1) * 128], idn[:])
                        nc.scalar.activation(
                            XT[:, j * SH + t * 128: j * SH + (t + 1) * 128],
                            pt[:], AF.Copy)
                    nc.vector.max(out=c8all[:, j * 8:j * 8 + 8],
                                  in_=XT[:, j * SH:(j + 1) * SH])
                c3all = pp.tile([128, CB * 3], F32, tag="c3all")
                nc.vector.tensor_copy(
                    c3all[:].rearrange("p (j s) -> p j s", s=3),
                    c8all[:].rearrange("p (j s) -> p j s", s=8)[:, :, 0:3])
                nc.sync.dma_start(cp_in[:], c3all[:])

                # ---------- collective 1: AllGather column partials ----------
                if no_coll:
                    nc.sync.dma_start(cp_out[0:128, :], cp_in[:])
                else:
                    nc.gpsimd.collective_compute(
                        "AllGather", OP.bypass, replica_groups=groups,
                        ins=[cp_in[:]], outs=[cp_out[:]])

                gath = pp.tile([128, NCORES * CB * 3], F32, tag="gath")
                nc.sync.dma_start(
                    gath[:].rearrange("p (k f) -> p k f", k=NCORES),
                    cp_out[:].rearrange("(k p) f -> p k f", p=128))

                # ---------- P3: combine -> exact column thresholds ----------
                cm8 = pp.tile([128, CB * 8], F32, tag="cm8")
                gv = gath[:].rearrange("p (k j s) -> p j k s", k=NCORES, s=3)
                for j in range(CB):
                    nc.vector.max(out=cm8[:, j * 8:j * 8 + 8], in_=gv[:, j])

                # ---------- P4: masking (transposed space) ----------
                # thr = max(trB, tc, T0); in place: thr <- [XT >= thr],
                # XT <- XT * thr  (XT becomes the masked W^T)
                thr = pp.tile([128, CB * SH], F32, tag="thr")
                for j in range(CB):
                    nc.vector.tensor_scalar_max(
                        thr[:, j * SH:(j + 1) * SH], trB[:],
                        cm8[:, j * 8 + 2:j * 8 + 3])
                CHUNK = 6 * SH
                for c0 in range(0, CB * SH, CHUNK):
                    c1 = c0 + CHUNK
                    nc.vector.tensor_tensor(thr[:, c0:c1], XT[:, c0:c1],
                                            thr[:, c0:c1], OP.is_ge)
                    nc.vector.tensor_tensor(XT[:, c0:c1], XT[:, c0:c1],
                                            thr[:, c0:c1], OP.mult)

                # ---------- phase-A Gram: C_raw = M1^T W M2 ----------
                def gram(m1_t, m2_t, pc_tile, tagb):
                    Bsb = sp.tile([128, RT * 6], F32, tag=tagb)
                    for t in range(RT):
                        pb = psa.tile([128, 6], F32, tag="pb")
                        for j in range(CB):
                            nc.tensor.matmul(
                                pb[:],
                                XT[:, j * SH + t * 128: j * SH + (t + 1) * 128],
                                m2_t[:, j * 6:(j + 1) * 6],
                                start=(j == 0), stop=(j == CB - 1))
                        nc.scalar.activation(Bsb[:, t * 6:(t + 1) * 6], pb[:],
                                             AF.Copy)
                    for t in range(RT):
                        nc.tensor.matmul(pc_tile[:], m1_t[:, t * 6:(t + 1) * 6],
                                         Bsb[:, t * 6:(t + 1) * 6],
                                         start=(t == 0), stop=(t == RT - 1))

                pc1 = psc.tile([6, 6], F32, tag="pc1")
                gram(m1t_s, m2t_s, pc1, "Bsb1")
                Cp = sp.tile([6, 6], F32, tag="Cp")
                _act_copy(nc, Cp[:], pc1[:])
                nc.sync.dma_start(cr_in[:], Cp[:])

                # ---------- collective 2: AllReduce raw 6x6 Gram ----------
                if no_coll:
                    nc.sync.dma_start(cr_out[:], cr_in[:])
                else:
                    nc.gpsimd.collective_compute(
                        "AllReduce", OP.add, replica_groups=groups,
                        ins=[cr_in[:]], outs=[cr_out[:]])

                if no_tail:
                    dummy = sp.tile([3, 3], F32, tag="dummy")
                    nc.sync.dma_start(dummy[:], cr_out[0:3, 0:3])
                    nc.sync.dma_start(out_d[:], dummy[:])
                    continue
                # ---------- tail part A: Hartley scalars from moments ----------
                sc, nrmB = _hartley(nc, pp, sp, ps, cps, idn, cr_out, stage)

                # ---------- phase-B Gram on centered monomials ----------
                M1n = pp.tile([128, RT * 6], F32, tag="M1n")
                M2n = pp.tile([128, CB * 6], F32, tag="M2n")

                def build_norm(src, dst, nt, sB, cxB, cyB, tagn):
                    sv = src[:].rearrange("p (t d) -> p d t", d=6)
                    dv = dst[:].rearrange("p (t d) -> p d t", d=6)
                    xh = sp.tile([128, nt], F32, tag=f"xh{tagn}")
                    nc.vector.tensor_scalar(xh[:], sv[:, 2], cxB, sB,
                                            OP.subtract, OP.mult)
                    yh = sp.tile([128, nt], F32, tag=f"yh{tagn}")
                    nc.vector.tensor_scalar(yh[:], sv[:, 4], cyB, sB,
                                            OP.subtract, OP.mult)
                    nc.vector.tensor_tensor(dv[:, 0], xh[:], xh[:], OP.mult)
                    nc.vector.tensor_tensor(dv[:, 1], xh[:], yh[:], OP.mult)
                    nc.vector.tensor_copy(dv[:, 2], xh[:])
                    nc.vector.tensor_tensor(dv[:, 3], yh[:], yh[:], OP.mult)
                    nc.vector.tensor_copy(dv[:, 4], yh[:])
                    nc.vector.memset(dv[:, 5], 1.0)

                # nrmB cols: [s1, c1x, c1y, s2, c2x, c2y]
                build_norm(m1t_s, M1n, RT, nrmB[:, 0:1], nrmB[:, 1:2],
                           nrmB[:, 2:3], "1")
                build_norm(m2t_s, M2n, CB, nrmB[:, 3:4], nrmB[:, 4:5],
                           nrmB[:, 5:6], "2")

                pc2 = psc.tile([6, 6], F32, tag="pc2")
                gram(M1n, M2n, pc2, "Bsb2")
                C2p = sp.tile([6, 6], F32, tag="C2p")
                _act_copy(nc, C2p[:], pc2[:])
                nc.sync.dma_start(c2_in[:], C2p[:])

                # ---------- collective 3: AllReduce normalized 6x6 Gram ------
                if no_coll:
                    nc.sync.dma_start(c2_out[:], c2_in[:])
                else:
                    nc.gpsimd.collective_compute(
                        "AllReduce", OP.add, replica_groups=groups,
                        ins=[c2_in[:]], outs=[c2_out[:]])

                # ---------- tail part B ----------
                _solve(nc, pp, sp, ps, cps, idn, sc, c2_out, stage, mshuf, out_d)

    nc.compile()
    return nc


def _transpose(nc, ps, sp, in_sb, n, idn, tag):
    """PE-transpose square [n, n] SBUF -> new SBUF tile."""
    pt = ps.tile([n, n], F32, tag="tps")
    nc.tensor.transpose(pt[:], in_sb, idn[:n, :n])
    ot = sp.tile([n, n], F32, tag=f"ot_{tag}")
    _act_copy(nc, ot[:], pt[:])
    return ot


def _pow50(nc, ps, sp, m_sb, n, tag):
    """Direction of M^50 v via rescaled squarings M <- 2*(M@M);
    M50 = 2*((2*(M32@M16)) @ M2). All operands symmetric."""
    powers = {}
    cur = m_sb
    for i in range(1, 6):  # M2, M4, M8, M16, M32
        pm = ps.tile([n, n], F32, tag="tps")
        nc.tensor.matmul(pm[:], cur, cur, start=True, stop=True)
        nxt = sp.tile([n, n], F32, tag=f"pws_{tag}_{i}")
        _act_copy(nc, nxt[:], pm[:], scale=2.0)
        powers[2 ** i] = nxt
        cur = nxt[:]
    pm = ps.tile([n, n], F32, tag="tps")
    nc.tensor.matmul(pm[:], powers[32][:], powers[16][:], start=True, stop=True)
    m48 = sp.tile([n, n], F32, tag=f"pws_{tag}_48")
    _act_copy(nc, m48[:], pm[:], scale=2.0)
    pm = ps.tile([n, n], F32, tag="tps")
    nc.tensor.matmul(pm[:], m48[:], powers[2][:], start=True, stop=True)
    m50 = sp.tile([n, n], F32, tag=f"pws_{tag}_50")
    _act_copy(nc, m50[:], pm[:], scale=2.0)
    return m50


def _hartley(nc, pp, sp, ps, cps, idn, cr_out, stage):
    """Moments -> Hartley scalars on partition 0; stage T1/T2 row-major;
    return (sc scratch tile, nrmB [128, 6] = bcast [s1,c1x,c1y,s2,c2x,c2y])."""
    e5 = cps[0:6, C_E5:C_E5 + 1]

    Cr = sp.tile([6, 6], F32, tag="Cr")
    nc.sync.dma_start(Cr[:], cr_out[:])
    CrT = _transpose(nc, ps, sp, Cr[:], 6, idn, "crt")

    sc = pp.tile([128, 96], F32, tag="tailsc")

    def scv(a, b):
        return sc[0:1, a:b]

    mo_ps = ps.tile([1, 6], F32, tag="tps")
    nc.tensor.matmul(mo_ps[:], e5, CrT[:], start=True, stop=True)
    _act_copy(nc, scv(0, 6), mo_ps[:])              # side1 moments
    mo_ps2 = ps.tile([1, 6], F32, tag="tps")
    nc.tensor.matmul(mo_ps2[:], e5, Cr[:], start=True, stop=True)
    _act_copy(nc, scv(6, 12), mo_ps2[:])            # side2 moments

    def pair(k):  # element k of each side: free idxs (k, k+6)
        return sc[0:1, 0:12].rearrange("p (g d) -> p d g", g=2)[:, k, :]

    Sxx, Sx, Syy, Sy, Sw = pair(0), pair(2), pair(3), pair(4), pair(5)
    ws = scv(12, 14); nc.vector.tensor_scalar_add(ws, Sw, EPS)
    rws = scv(14, 16); nc.vector.reciprocal(rws, ws)
    cx = scv(16, 18); nc.vector.tensor_tensor(cx, Sx, rws, OP.mult)
    cy = scv(18, 20); nc.vector.tensor_tensor(cy, Sy, rws, OP.mult)
    t_a = scv(20, 22); nc.vector.tensor_tensor(t_a, cx, Sx, OP.mult)
    t_b = scv(22, 24); nc.vector.tensor_tensor(t_b, cy, Sy, OP.mult)
    cdS = scv(24, 26); nc.vector.tensor_tensor(cdS, t_a, t_b, OP.add)
    u_a = scv(26, 28); nc.vector.tensor_tensor(u_a, cx, cx, OP.mult)
    u_b = scv(28, 30); nc.vector.tensor_tensor(u_b, cy, cy, OP.mult)
    c2_ = scv(30, 32); nc.vector.tensor_tensor(c2_, u_a, u_b, OP.add)
    sq_ = scv(32, 34); nc.vector.tensor_tensor(sq_, Sxx, Syy, OP.add)
    n2c = scv(34, 36); nc.vector.tensor_scalar_mul(n2c, cdS, -2.0)
    c2w = scv(36, 38); nc.vector.tensor_tensor(c2w, c2_, Sw, OP.mult)
    m_ = scv(38, 40); nc.vector.tensor_tensor(m_, sq_, n2c, OP.add)
    m2_ = scv(40, 42); nc.vector.tensor_tensor(m2_, m_, c2w, OP.add)
    md2 = scv(42, 44); nc.vector.tensor_tensor(md2, m2_, rws, OP.mult)
    md2e = scv(44, 46); nc.vector.tensor_scalar_add(md2e, md2, EPS)
    md = scv(46, 48); nc.scalar.activation(md, md2e, AF.Sqrt)
    mde = scv(48, 50); nc.vector.tensor_scalar_add(mde, md, EPS)
    rmd = scv(50, 52); nc.vector.reciprocal(rmd, mde)
    s_ = scv(52, 54); nc.vector.tensor_scalar_mul(s_, rmd, SQRT2)
    scx = scv(54, 56); nc.vector.tensor_tensor(scx, s_, cx, OP.mult)
    scy = scv(56, 58); nc.vector.tensor_tensor(scy, s_, cy, OP.mult)
    nscx = scv(58, 60); nc.vector.tensor_scalar_mul(nscx, scx, -1.0)
    nscy = scv(60, 62); nc.vector.tensor_scalar_mul(nscy, scy, -1.0)

    # T row-major 9-vectors: t1v at 64:73, t2v at 73:82
    nc.vector.memset(scv(64, 82), 0.0)
    tv = sc[0:1, 64:82]
    tv9 = tv.rearrange("p (v f) -> p v f", v=2)  # [1, 2(side), 9]
    nc.vector.tensor_copy(tv9[:, :, 0:1], s_.unsqueeze(2))
    nc.vector.tensor_copy(tv9[:, :, 4:5], s_.unsqueeze(2))
    nc.vector.tensor_copy(
        tv9[:, :, 2:8].rearrange("p v (c d) -> p v c d", c=2)[:, :, :, 0:1],
        sc[0:1, 58:62].rearrange("p (c v) -> p v c", c=2).unsqueeze(3))
    nc.vector.memset(tv9[:, :, 8:9], 1.0)
    nc.sync.dma_start(stage[0:18], tv)

    # normalization scalar vector [s1, c1x, c1y, s2, c2x, c2y] -> bcast
    nv = scv(84, 90)
    nc.vector.tensor_copy(sc[0:1, 84:85], sc[0:1, 52:53])   # s1
    nc.vector.tensor_copy(
        sc[0:1, 85:87],
        sc[0:1, 16:20].rearrange("p (d g) -> p d g", d=2)[:, :, 0])  # c1x c1y
    nc.vector.tensor_copy(sc[0:1, 87:88], sc[0:1, 53:54])   # s2
    nc.vector.tensor_copy(
        sc[0:1, 88:90],
        sc[0:1, 16:20].rearrange("p (d g) -> p d g", d=2)[:, :, 1])  # c2x c2y
    nrmB = pp.tile([128, 6], F32, tag="nrmB")
    nc.gpsimd.partition_broadcast(nrmB[:], nv, channels=128)
    return sc, nrmB


def _solve(nc, pp, sp, ps, cps, idn, sc, c2_out, stage, mshuf, out_d):
    """Mmat expansion, power chains, projection, output."""
    i9h = cps[0:9, C_I9H:C_I9H + 9]
    et69 = cps[0:6, C_ET69:C_ET69 + 9]
    i3c = cps[0:3, C_I3:C_I3 + 3]
    v09 = cps[0:9, C_V09:C_V09 + 1]
    v06 = cps[0:6, C_V06:C_V06 + 1]
    sel1 = cps[0:3, C_SEL1:C_SEL1 + 6]
    sel2 = cps[0:3, C_SEL2:C_SEL2 + 6]

    C2r = sp.tile([6, 6], F32, tag="C2r")
    nc.sync.dma_start(C2r[:], c2_out[:])
    C2rT = _transpose(nc, ps, sp, C2r[:], 6, idn, "c2rt")

    # G2 = E C2 E^T : G2[3a+b, 3c+d] = C2[pair(a,b), pair(c,d)]
    z_ps = ps.tile([6, 9], F32, tag="tps")
    nc.tensor.matmul(z_ps[:], C2rT[:], et69, start=True, stop=True)  # C2 E^T
    Zs = sp.tile([6, 9], F32, tag="Zs")
    _act_copy(nc, Zs[:], z_ps[:])
    g_ps = ps.tile([9, 9], F32, tag="tps")
    nc.tensor.matmul(g_ps[:], et69, Zs[:], start=True, stop=True)    # E @ Z
    G2 = sp.tile([9, 9], F32, tag="G2")
    _act_copy(nc, G2[:], g_ps[:])

    # Mmat[3p+q, 3r+s] = G2[3p+r, 3q+s]: bounce via DRAM, 9 row reads
    nc.sync.dma_start(mshuf[:], G2[:])
    Mmat = sp.tile([9, 9], F32, tag="Mmat")
    for p in range(3):
        # Mmat[3p+q, 3r+s] <- mshuf[27p + 9r + 3q + s]; dims (q, r, s), s contig
        nc.sync.dma_start(
            Mmat[3 * p:3 * p + 3, :].rearrange("q (r s) -> q r s", s=3),
            mshuf[:].rearrange("(p q1 r s) -> p q1 r s", p=3, q1=3, r=3)
            .transpose([0, 2, 1, 3])[p])

    # shifted scaled 9x9: Msp = Mmat/(2 lam) - I/2 (sign irrelevant, even pow)
    dg = sp.tile([9, 9], F32, tag="dg")
    nc.vector.tensor_tensor(dg[:], Mmat[:], i9h, OP.mult)  # diag/2
    lam2 = sp.tile([9, 1], F32, tag="lam2")
    nc.vector.tensor_reduce(lam2[:], dg[:], AX.X, OP.add)
    lam2r = sp.tile([9, 1], F32, tag="lam2r")
    nc.gpsimd.partition_all_reduce(lam2r[:], lam2[:], channels=9,
                                   reduce_op=bass_isa.ReduceOp.add)
    lam4 = sp.tile([9, 1], F32, tag="lam4")
    nc.vector.tensor_scalar_mul(lam4[:], lam2r[:], 4.0)  # = 2*lam
    inv2l = sp.tile([9, 1], F32, tag="inv2l")
    nc.vector.reciprocal(inv2l[:], lam4[:])
    Msp = sp.tile([9, 9], F32, tag="Msp")
    nc.vector.scalar_tensor_tensor(Msp[:], Mmat[:], inv2l[:], i9h,
                                   OP.mult, OP.subtract)
    M50 = _pow50(nc, ps, sp, Msp[:], 9, "m9")

    w9ps = ps.tile([1, 9], F32, tag="tps")
    nc.tensor.matmul(w9ps[:], v09, M50[:], start=True, stop=True)
    w9 = sp.tile([1, 9], F32, tag="w9")
    _act_copy(nc, w9[:], w9ps[:])
    w9sq = sp.tile([1, 9], F32, tag="w9sq")
    nc.vector.tensor_tensor(w9sq[:], w9[:], w9[:], OP.mult)
    nn9 = sp.tile([1, 1], F32, tag="nn9")
    nc.vector.tensor_reduce(nn9[:], w9sq[:], AX.X, OP.add)
    sr9 = sp.tile([1, 1], F32, tag="sr9")
    nc.scalar.activation(sr9[:], nn9[:], AF.Sqrt)
    rs9 = sp.tile([1, 1], F32, tag="rs9")
    nc.vector.reciprocal(rs9[:], sr9[:])
    v9 = sp.tile([1, 9], F32, tag="v9")
    nc.vector.tensor_tensor(v9[:], w9[:], rs9[:].to_broadcast([1, 9]), OP.mult)
    nc.sync.dma_start(stage[24:33], v9[:])

    # E = T2^T E_raw T1 (and E^T)
    T1m = sp.tile([3, 3], F32, tag="T1m")
    nc.sync.dma_start(T1m[:], stage[0:9].rearrange("(i j) -> i j", j=3))
    T2m = sp.tile([3, 3], F32, tag="T2m")
    nc.sync.dma_start(T2m[:], stage[9:18].rearrange("(i j) -> i j", j=3))
    Eraw = sp.tile([3, 3], F32, tag="Eraw")
    nc.sync.dma_start(Eraw[:], stage[24:33].rearrange("(i j) -> i j", j=3))

    a1ps = ps.tile([3, 3], F32, tag="tps")
    nc.tensor.matmul(a1ps[:], T2m[:], Eraw[:], start=True, stop=True)
    A1 = sp.tile([3, 3], F32, tag="A1")
    _act_copy(nc, A1[:], a1ps[:])
    A1T = _transpose(nc, ps, sp, A1[:], 3, idn, "a1t")
    etps = ps.tile([3, 3], F32, tag="tps")
    nc.tensor.matmul(etps[:], T1m[:], A1T[:], start=True, stop=True)
    ETs = sp.tile([3, 3], F32, tag="ETs")
    _act_copy(nc, ETs[:], etps[:])
    Es = _transpose(nc, ps, sp, ETs[:], 3, idn, "es")

    # B = E^T E ; blockdiag 6x6 chain for v1 (max) and v3 (min)
    bps = ps.tile([3, 3], F32, tag="tps")
    nc.tensor.matmul(bps[:], Es[:], Es[:], start=True, stop=True)
    Bm = sp.tile([3, 3], F32, tag="Bm")
    _act_copy(nc, Bm[:], bps[:])
    dg3 = sp.tile([3, 3], F32, tag="dg3")
    nc.vector.tensor_tensor(dg3[:], Bm[:], i3c, OP.mult)
    lb = sp.tile([3, 1], F32, tag="lb")
    nc.vector.tensor_reduce(lb[:], dg3[:], AX.X, OP.add)
    lbr = sp.tile([3, 1], F32, tag="lbr")
    nc.gpsimd.partition_all_reduce(lbr[:], lb[:], channels=3,
                                   reduce_op=bass_isa.ReduceOp.add)
    invlb = sp.tile([3, 1], F32, tag="invlb")
    nc.vector.reciprocal(invlb[:], lbr[:])
    Bs3 = sp.tile([3, 3], F32, tag="Bs3")
    nc.vector.tensor_scalar_mul(Bs3[:], Bm[:], invlb[:])
    IB = sp.tile([3, 3], F32, tag="IB")
    nc.vector.tensor_tensor(IB[:], i3c, Bs3[:], OP.subtract)
    bdps = ps.tile([6, 6], F32, tag="tps")
    nc.tensor.matmul(bdps[:, 0:3], sel1, Bs3[:], start=True, stop=True)
    nc.tensor.matmul(bdps[:, 3:6], sel2, IB[:], start=True, stop=True)
    BD = sp.tile([6, 6], F32, tag="BD")
    _act_copy(nc, BD[:], bdps[:])
    BD50 = _pow50(nc, ps, sp, BD[:], 6, "m6")

    w6ps = ps.tile([1, 6], F32, tag="tps")
    nc.tensor.matmul(w6ps[:], v06, BD50[:], start=True, stop=True)
    w6 = sp.tile([1, 6], F32, tag="w6")
    _act_copy(nc, w6[:], w6ps[:])
    w6sq = sp.tile([1, 6], F32, tag="w6sq")
    nc.vector.tensor_tensor(w6sq[:], w6[:], w6[:], OP.mult)
    nn6 = sp.tile([1, 2], F32, tag="nn6")
    nc.vector.tensor_reduce(nn6[:].unsqueeze(2),
                            w6sq[:].rearrange("p (g d) -> p g d", g=2), AX.X,
                            OP.add)
    sr6 = sp.tile([1, 2], F32, tag="sr6")
    nc.scalar.activation(sr6[:], nn6[:], AF.Sqrt)
    rs6 = sp.tile([1, 2], F32, tag="rs6")
    nc.vector.reciprocal(rs6[:], sr6[:])
    vv = sp.tile([1, 6], F32, tag="vv")
    nc.vector.tensor_tensor(
        vv[:].rearrange("p (g d) -> p g d", g=2),
        w6[:].rearrange("p (g d) -> p g d", g=2),
        rs6[:].unsqueeze(2).to_broadcast([1, 2, 3]), OP.mult)

    # v2 = cross(v3, v1), normalized with EPS (as reference)
    aa = sp.tile([1, 6], F32, tag="aa")
    nc.vector.tensor_copy(
        aa[:].rearrange("p (r d) -> p r d", r=2),
        vv[:, 3:6].unsqueeze(1).to_broadcast([1, 2, 3]))
    bb = sp.tile([1, 6], F32, tag="bb")
    nc.vector.tensor_copy(
        bb[:].rearrange("p (r d) -> p r d", r=2),
        vv[:, 0:3].unsqueeze(1).to_broadcast([1, 2, 3]))
    cr1 = sp.tile([1, 3], F32, tag="cr1")
    nc.vector.tensor_tensor(cr1[:], aa[:, 1:4], bb[:, 2:5], OP.mult)
    cr2 = sp.tile([1, 3], F32, tag="cr2")
    nc.vector.tensor_tensor(cr2[:], aa[:, 2:5], bb[:, 1:4], OP.mult)
    v2r = sp.tile([1, 3], F32, tag="v2r")
    nc.vector.tensor_tensor(v2r[:], cr1[:], cr2[:], OP.subtract)
    v2sq = sp.tile([1, 3], F32, tag="v2sq")
    nc.vector.tensor_tensor(v2sq[:], v2r[:], v2r[:], OP.mult)
    nn2 = sp.tile([1, 1], F32, tag="nn2")
    nc.vector.tensor_reduce(nn2[:], v2sq[:], AX.X, OP.add)
    sr2 = sp.tile([1, 1], F32, tag="sr2")
    nc.scalar.activation(sr2[:], nn2[:], AF.Sqrt)
    sr2e = sp.tile([1, 1], F32, tag="sr2e")
    nc.vector.tensor_scalar_add(sr2e[:], sr2[:], EPS)
    rs2 = sp.tile([1, 1], F32, tag="rs2")
    nc.vector.reciprocal(rs2[:], sr2e[:])
    v2 = sp.tile([1, 3], F32, tag="v2")
    nc.vector.tensor_tensor(v2[:], v2r[:], rs2[:].to_broadcast([1, 3]), OP.mult)

    # stage v1, v2; Ev rows; final assembly
    nc.sync.dma_start(stage[33:36], vv[:, 0:3])
    nc.sync.dma_start(stage[36:39], v2[:])
    Vc = sp.tile([3, 2], F32, tag="Vc")
    nc.sync.dma_start(Vc[:], stage[33:39].rearrange("(i k) -> k i", k=3))
    Vr = sp.tile([2, 3], F32, tag="Vr")
    nc.sync.dma_start(Vr[:], stage[33:39].rearrange("(i k) -> i k", k=3))
    evps = ps.tile([2, 3], F32, tag="tps")
    nc.tensor.matmul(evps[:], Vc[:], ETs[:], start=True, stop=True)
    Evr = sp.tile([2, 3], F32, tag="Evr")
    _act_copy(nc, Evr[:], evps[:])
    evsq = sp.tile([2, 3], F32, tag="evsq")
    nc.vector.tensor_tensor(evsq[:], Evr[:], Evr[:], OP.mult)
    ss2 = sp.tile([2, 1], F32, tag="ss2")
    nc.vector.tensor_reduce(ss2[:], evsq[:], AX.X, OP.add)
    sv = sp.tile([2, 1], F32, tag="sv")
    nc.scalar.activation(sv[:], ss2[:], AF.Sqrt)
    ssum = sp.tile([2, 1], F32, tag="ssum")
    nc.gpsimd.partition_all_reduce(ssum[:], sv[:], channels=2,
                                   reduce_op=bass_isa.ReduceOp.add)
    savg = sp.tile([2, 1], F32, tag="savg")
    nc.vector.tensor_scalar_mul(savg[:], ssum[:], 0.5)
    sve = sp.tile([2, 1], F32, tag="sve")
    nc.vector.tensor_scalar_add(sve[:], sv[:], EPS)
    rsv = sp.tile([2, 1], F32, tag="rsv")
    nc.vector.reciprocal(rsv[:], sve[:])
    f2 = sp.tile([2, 1], F32, tag="f2")
    nc.vector.tensor_tensor(f2[:], rsv[:], savg[:], OP.mult)
    U2 = sp.tile([2, 3], F32, tag="U2")
    nc.vector.tensor_scalar_mul(U2[:], Evr[:], f2[:])
    ops_ = ps.tile([3, 3], F32, tag="tps")
    nc.tensor.matmul(ops_[:], U2[:], Vr[:], start=True, stop=True)
    outs = sp.tile([3, 3], F32, tag="outs")
    _act_copy(nc, outs[:], ops_[:])
    nc.sync.dma_start(out_d[:], outs[:])


def make_in_maps(P, K):
    """Host-side shard + constant prep: list of 8 input dicts."""
    P = np.asarray(P, np.float32)
    K = np.asarray(K, np.float32)
    Pc = np.ascontiguousarray(P[:N, :N])
    M, cpack = host_constants(K)
    m2t = _tile128(M, CB)
    ident = np.eye(128, dtype=np.float32)
    in_maps = []
    for k in range(NCORES):
        sh = Pc[k * SH:(k + 1) * SH]
        in_maps.append({
            "xin": _tile128(sh, RT),
            "m1s": _tile128(M[k * SH:(k + 1) * SH], RT),
            "m2t": m2t,
            "ident": ident,
            "cpack": cpack,
        })
    return in_maps


_NC_CACHE = {}


def kernel(P, K):
    from concourse.bass_utils import run_bass_kernel_spmd
    if "nc" not in _NC_CACHE:
        _NC_CACHE["nc"] = build_nc()
    nc = _NC_CACHE["nc"]
    in_maps = make_in_maps(P, K)
    res = run_bass_kernel_spmd(nc, in_maps, core_ids=list(range(NCORES)))
    return np.asarray(res.results[0]["out"], np.float32)



# revision 12
# speedup vs baseline: 1.1484x; 1.1484x over previous
"""Trainium2 Bass kernel for nn_EssentialMatrixEstimator (v2).

Distribution (8 cores):
  - XN: natural row-shard  (384 rows x 3072 cols) -> exact row top-3 thresholds.
  - XC: transposed col-shard (384 cols x 3072 rows as [col, row]) -> exact col
    top-3 thresholds + dense masking + col-sharded gram.
  - coll1: AllGather of per-core row thresholds (384 f32 -> 3072).
  - coll2: AllReduce of the 6x6 gram C' on PRE-CENTERED monomials.

Math: the (N*M,9) epipolar Gram collapses to the 6x6 monomial Gram C'.
Monomials are pre-centered about the host constant c0 (grid centroid), so C'
is well-conditioned; the Hartley normalization is recovered from C' moments
(row/col 5) and applied as a 6x6 L-transform C2 = L1 C' L2^T instead of a
second gram pass.  Mmat (9x9) is an index expansion of C2; min-eigvector via
50-step shifted power iteration (rescaled repeated squaring), projection via
a 32-step 6x6 blockdiag chain (insensitive; validated 2.9e-4).

The big T = M2'^T W^T contraction streams in float32r (1 cy/row); validated
tolerant to tf32/bf16-level rounding (5e-4 / 3.9e-3 final rel err).
"""

import os

os.environ.setdefault("JAX_PLATFORMS", "axon")

import numpy as np

import concourse.bass as bass
import concourse.bass_isa as bass_isa
import concourse.mybir as mybir
import concourse.bacc as bacc
import concourse.tile as tile

NCORES = 8
N = 3072
SH = N // NCORES          # 384 rows/cols per core
RT = SH // 128            # 3 tiles per core shard
CB = N // 128             # 24 tiles across the full dim
F32 = mybir.dt.float32
F32R = mybir.dt.float32r
AF = mybir.ActivationFunctionType
OP = mybir.AluOpType
AX = mybir.AxisListType

EPS = 1e-8
SQRT2 = 1.4142135623730951
INV_SQRT3 = 1.0 / 1.7320508075688772
T0 = float(np.nextafter(np.float32(0.01), np.float32(1)))  # x > 0.01 == x >= T0
H, W = 64, 64

# cpack const layout (tensor [9, C_TOT]): column ranges
C_I9H = 0      # I9 * 0.5            [9, 9]
C_ET69 = 9     # E^T selector        [6, 9]
C_I3 = 18      # I3                  [3, 3]
C_V09 = 21     # full(1/3)           [9, 1]
C_V06 = 22     # full(1/sqrt3)       [6, 1]
C_SEL1 = 23    # [I3 | 0]            [3, 6]
C_SEL2 = 29    # [0 | I3]            [3, 6]
C_SHT = 35     # Sh component mats^T: I6, E1^T..E5^T   [6, 6*6]
C_MSK = 71     # svec masks [c2m c1m c0m]  [6, 3]
C_IDN = 74     # identity 9x9        [9, 9]
C_TOT = 83

PAIRS = [(0, 0), (0, 1), (0, 2), (1, 1), (1, 2), (2, 2)]


def _pidx():
    d = {}
    for i, (a, b) in enumerate(PAIRS):
        d[(a, b)] = i
        d[(b, a)] = i
    return d


def grid_pts(K):
    idx = np.arange(H * W, dtype=np.float32)
    pix = np.stack([idx % np.float32(W), np.floor(idx / np.float32(W))], -1)
    K_inv = np.linalg.inv(np.asarray(K, np.float32)).astype(np.float32)
    p1h = np.concatenate([pix[:N], np.ones((N, 1), np.float32)], -1)
    pts = (p1h @ K_inv.T)[:, :2].astype(np.float32)
    return pts


def host_constants(K):
    """Pre-centered monomials + packed tail constants (f32)."""
    pts = grid_pts(K)
    x, y = pts[:, 0], pts[:, 1]
    c0x = np.float32(x.mean())
    c0y = np.float32(y.mean())
    xs = (x - c0x).astype(np.float32)
    ys = (y - c0y).astype(np.float32)
    Mp = np.stack([xs * xs, xs * ys, xs, ys * ys, ys, np.ones_like(xs)],
                  -1).astype(np.float32)

    cpack = np.zeros((9, C_TOT), np.float32)
    cpack[:9, C_I9H:C_I9H + 9] = 0.5 * np.eye(9, dtype=np.float32)
    pid = _pidx()
    for a in range(3):
        for b in range(3):
            cpack[pid[(a, b)], C_ET69 + 3 * a + b] = 1.0
    cpack[:3, C_I3:C_I3 + 3] = np.eye(3, dtype=np.float32)
    cpack[:9, C_V09] = 1.0 / 3.0
    cpack[:6, C_V06] = INV_SQRT3
    cpack[:3, C_SEL1:C_SEL1 + 3] = np.eye(3, dtype=np.float32)
    cpack[:3, C_SEL2 + 3:C_SEL2 + 6] = np.eye(3, dtype=np.float32)

    # Sh(dx,dy) = I + dx*E1 + dy*E2 + dx^2*E3 + dx*dy*E4 + dy^2*E5
    # (rows of L before the diag scale; see proto.Lmat)
    E1 = np.zeros((6, 6), np.float32)  # dx terms
    E1[0, 2] = -2.0
    E1[1, 4] = -1.0
    E1[2, 5] = -1.0
    E2 = np.zeros((6, 6), np.float32)  # dy terms
    E2[1, 2] = -1.0
    E2[3, 4] = -2.0
    E2[4, 5] = -1.0
    E3 = np.zeros((6, 6), np.float32)  # dx^2
    E3[0, 5] = 1.0
    E4 = np.zeros((6, 6), np.float32)  # dx*dy
    E4[1, 5] = 1.0
    E5 = np.zeros((6, 6), np.float32)  # dy^2
    E5[3, 5] = 1.0
    mats = [np.eye(6, dtype=np.float32), E1, E2, E3, E4, E5]
    for i, Em in enumerate(mats):
        cpack[:6, C_SHT + 6 * i:C_SHT + 6 * i + 6] = Em.T
    # svec masks: svec = [s2,s2,s,s2,s,1] = c2m*s2 + c1m*s + c0m
    cpack[:6, C_MSK + 0] = [1, 1, 0, 1, 0, 0]
    cpack[:6, C_MSK + 1] = [0, 0, 1, 0, 1, 0]
    cpack[:6, C_MSK + 2] = [0, 0, 0, 0, 0, 1]
    cpack[:9, C_IDN:C_IDN + 9] = np.eye(9, dtype=np.float32)
    return Mp, cpack, float(c0x), float(c0y)


def _tile128(a, ntiles):
    """[ntiles*128, F] -> [128, ntiles*F] with [p, t*F+f] = a[t*128+p, f]."""
    F = a.shape[1]
    return np.ascontiguousarray(
        a.reshape(ntiles, 128, F).transpose(1, 0, 2).reshape(128, ntiles * F)
    )


def build_nc(repeats=1, no_coll=False, no_tail=False, use_f32r=True, dbg_c=False):
    nc = bacc.Bacc("TRN2", target_bir_lowering=False, debug=False,
                   num_devices=NCORES)

    xn = nc.dram_tensor("xn", [128, RT * N], F32, kind="ExternalInput")
    xc = nc.dram_tensor("xc", [128, RT * N], F32, kind="ExternalInput")
    m1f = nc.dram_tensor("m1f", [128, CB * 6], F32, kind="ExternalInput")
    m2s = nc.dram_tensor("m2s", [128, RT * 6], F32, kind="ExternalInput")
    cpk = nc.dram_tensor("cpack", [9, C_TOT], F32, kind="ExternalInput")
    c0t = nc.dram_tensor("c0t", [1, 4], F32, kind="ExternalInput")
    out_d = nc.dram_tensor("out", [6, 6] if dbg_c else [3, 3], F32, kind="ExternalOutput")

    tr_in = nc.dram_tensor("tr_in", [1, SH], F32)
    tr_out = nc.dram_tensor("tr_out", [NCORES, SH], F32, addr_space="Shared")
    cr_in = nc.dram_tensor("cr_in", [6, 6], F32)
    cr_out = nc.dram_tensor("cr_out", [6, 6], F32, addr_space="Shared")
    tb = nc.dram_tensor("tb", [6, N], F32)        # T bounce
    stage = nc.dram_tensor("stage", [64], F32)
    mshuf = nc.dram_tensor("mshuf", [81], F32)

    groups = [list(range(NCORES))]

    with tile.TileContext(nc) as tc:
        with (
            tc.tile_pool(name="persist", bufs=1) as pp,
            tc.tile_pool(name="scratch", bufs=2) as sp,
            tc.tile_pool(name="ps_t", bufs=2, space="PSUM") as ps,
            tc.tile_pool(name="ps_T", bufs=2, space="PSUM") as psT,
            tc.tile_pool(name="ps_c", bufs=1, space="PSUM") as psc,
        ):
            for _rep in range(repeats):
                # ---------- P0: loads (XN on qSP, XC on qACT) ----------
                XN = pp.tile([128, RT * N], F32, tag="XN")
                XC = pp.tile([128, RT * N], F32, tag="XC")
                for t in range(RT):
                    nc.sync.dma_start(XN[:, t * N:(t + 1) * N],
                                      xn[:, t * N:(t + 1) * N])
                    nc.scalar.dma_start(XC[:, t * N:(t + 1) * N],
                                        xc[:, t * N:(t + 1) * N])
                m1s_s = pp.tile([128, CB * 6], F32, tag="m1f")
                nc.scalar.dma_start(m1s_s[:], m1f[:])
                m2s_s = pp.tile([128, RT * 6], F32, tag="m2s")
                nc.scalar.dma_start(m2s_s[:], m2s[:])
                cps = pp.tile([9, C_TOT], F32, tag="cpk")
                nc.scalar.dma_start(cps[:], cpk[:])
                c0s = pp.tile([1, 4], F32, tag="c0")
                nc.scalar.dma_start(c0s[:], c0t[:])

                def XNt(t):
                    return XN[:, t * N:(t + 1) * N]

                def XCt(t):
                    return XC[:, t * N:(t + 1) * N]

                # ---------- P1: row thresholds -> coll1 ----------
                r8 = pp.tile([128, RT * 8], F32, tag="r8")
                for t in range(RT):
                    nc.vector.max(out=r8[:, t * 8:t * 8 + 8], in_=XNt(t))
                trT0 = pp.tile([128, RT], F32, tag="trT0")
                nc.vector.tensor_scalar_max(
                    trT0[:],
                    r8[:].rearrange("p (t e) -> p t e", e=8)[:, :, 2], T0)
                for t in range(RT):
                    nc.sync.dma_start(tr_in[0:1, t * 128:(t + 1) * 128],
                                      trT0[:, t:t + 1])

                if no_coll:
                    nc.sync.dma_start(tr_out[0:1, :], tr_in[:])
                else:
                    nc.gpsimd.collective_compute(
                        "AllGather", OP.bypass, replica_groups=groups,
                        ins=[tr_in[:]], outs=[tr_out[:]])

                # ---------- P2: col thresholds (local, exact) ----------
                c8 = pp.tile([128, RT * 8], F32, tag="c8")
                for t in range(RT):
                    nc.vector.max(out=c8[:, t * 8:t * 8 + 8], in_=XCt(t))

                # ---------- P3: broadcast row-threshold table ----------
                trow = pp.tile([1, N], F32, tag="trow")
                nc.sync.dma_start(trow[:], tr_out[:].rearrange("k i -> (k i)"))
                trB = pp.tile([128, N], F32, tag="trB")
                MCH = 1536
                for c0_ in range(0, N, MCH):
                    nc.gpsimd.partition_broadcast(
                        trB[:, c0_:c0_ + MCH], trow[:, c0_:c0_ + MCH],
                        channels=128)

                # ---------- P4: dense mask + fp32r T-gram ----------
                # W (f32r): W = XC * [XC >= max(trB, tc_t)]
                # T[b, r] = sum_c m2'[c, b] * W^T[c, r]   (PSUM chunks [6,512])
                WDT = F32R if use_f32r else F32
                m2r = pp.tile([128, RT * 6], WDT, tag="m2r")
                nc.vector.tensor_copy(m2r[:], m2s_s[:])
                Wr = pp.tile([128, RT * N], WDT, tag="Wr")
                Tsb = sp.tile([6, N], F32, tag="Tsb")
                for h in range(2):
                    for t in range(RT):
                        tcl = c8[:, t * 8 + 2:t * 8 + 3]
                        sl = slice(t * N + h * MCH, t * N + (h + 1) * MCH)
                        msk = sp.tile([128, MCH], F32, tag="msk")
                        nc.vector.scalar_tensor_tensor(
                            msk[:], trB[:, h * MCH:(h + 1) * MCH], tcl,
                            XC[:, sl], OP.max, OP.is_le)
                        nc.vector.tensor_tensor(Wr[:, sl], XC[:, sl], msk[:],
                                                OP.mult)
                    for q in range(3):
                        ch = h * 3 + q
                        Tp = psT.tile([6, 512], F32, tag="Tp")
                        for t in range(RT):
                            c0_ = t * N + h * MCH + q * 512
                            nc.tensor.matmul(
                                Tp[:],
                                m2r[:, t * 6:(t + 1) * 6],
                                Wr[:, c0_:c0_ + 512],
                                start=(t == 0), stop=(t == RT - 1))
                        nc.scalar.activation(Tsb[:, ch * 512:(ch + 1) * 512],
                                             Tp[:], AF.Copy)
                nc.sync.dma_start(tb[:], Tsb[:])

                # bounce-transpose: TT[p, (j b)] = T[b, 128j+p]
                TT = sp.tile([128, CB * 6], F32, tag="TT")
                TTv = TT[:].rearrange("p (j b) -> p j b", b=6)
                for b in range(6):
                    nc.sync.dma_start(
                        TTv[:, :, b],
                        tb[b:b + 1, :].rearrange("one (j p) -> p (one j)",
                                                 p=128))

                # C[a, b] = sum_j m1'_j^T TT_j
                pc = psc.tile([6, 6], F32, tag="pc")
                for j in range(CB):
                    nc.tensor.matmul(pc[:], m1s_s[:, j * 6:(j + 1) * 6],
                                     TT[:, j * 6:(j + 1) * 6],
                                     start=(j == 0), stop=(j == CB - 1))
                Cp = sp.tile([6, 6], F32, tag="Cp")
                nc.vector.tensor_copy(Cp[:], pc[:])
                nc.sync.dma_start(cr_in[:], Cp[:])

                # ---------- coll2: AllReduce 6x6 gram ----------
                if no_coll:
                    nc.sync.dma_start(cr_out[:], cr_in[:])
                else:
                    nc.gpsimd.collective_compute(
                        "AllReduce", OP.add, replica_groups=groups,
                        ins=[cr_in[:]], outs=[cr_out[:]])

                if no_tail:
                    nn = 6 if dbg_c else 3
                    dummy = sp.tile([nn, nn], F32, tag="dummy")
                    nc.sync.dma_start(dummy[:], cr_out[0:nn, 0:nn])
                    nc.sync.dma_start(out_d[:], dummy[:])
                    continue

                # ---------- tail ----------
                _tail(nc, pp, sp, ps, cps, c0s, cr_out, stage, mshuf, out_d)

    nc.compile()
    return nc


def _transpose(nc, ps, sp, in_sb, n, idn, tag):
    pt = ps.tile([n, n], F32, tag="tps")
    nc.tensor.transpose(pt[:], in_sb, idn[:n, :n])
    ot = sp.tile([n, n], F32, tag=f"ot_{tag}")
    nc.vector.tensor_copy(ot[:], pt[:])
    return ot


def _powchain(nc, ps, sp, m_sb, n, tag, n_squarings=5, extra=True):
    """M^50 (extra=True: 5 squarings + M48=M32@M16 + M50=M48@M2) or M^32."""
    powers = {}
    cur = m_sb
    for i in range(1, n_squarings + 1):
        pm = ps.tile([n, n], F32, tag="tps")
        nc.tensor.matmul(pm[:], cur, cur, start=True, stop=True)
        nxt = sp.tile([n, n], F32, tag=f"pw_{tag}_{i}")
        nc.vector.tensor_scalar_mul(nxt[:], pm[:], 2.0)
        powers[2 ** i] = nxt
        cur = nxt[:]
    if not extra:
        return powers[2 ** n_squarings]
    pm = ps.tile([n, n], F32, tag="tps")
    nc.tensor.matmul(pm[:], powers[32][:], powers[16][:], start=True, stop=True)
    m48 = sp.tile([n, n], F32, tag=f"pw_{tag}_48")
    nc.vector.tensor_scalar_mul(m48[:], pm[:], 2.0)
    pm = ps.tile([n, n], F32, tag="tps")
    nc.tensor.matmul(pm[:], m48[:], powers[2][:], start=True, stop=True)
    m50 = sp.tile([n, n], F32, tag=f"pw_{tag}_50")
    nc.vector.tensor_scalar_mul(m50[:], pm[:], 2.0)
    return m50


def _tail(nc, pp, sp, ps, cps, c0s, cr_out, stage, mshuf, out_d):
    """C' -> Hartley -> L-transform -> Mmat -> chains -> projection."""
    idn = cps[0:9, C_IDN:C_IDN + 9]

    Cp = sp.tile([6, 6], F32, tag="Cpr")
    nc.sync.dma_start(Cp[:], cr_out[:])
    CpT = sp.tile([6, 6], F32, tag="CprT")
    nc.sync.dma_start(CpT[:], cr_out[:].rearrange("a b -> b a"))

    # moments [1,12]: side1 = C'[:,5], side2 = C'[5,:]
    sc = pp.tile([128, 112], F32, tag="tailsc")
    nc.sync.dma_start(sc[0:1, 0:6],
                      cr_out[:].rearrange("a b -> b a")[5:6, :])
    nc.sync.dma_start(sc[0:1, 6:12], cr_out[5:6, :])

    def scv(a, b):
        return sc[0:1, a:b]

    def pair(k):
        return sc[0:1, 0:12].rearrange("p (g d) -> p d g", g=2)[:, k, :]

    Sxx, Sx, Syy, Sy, Sw = pair(0), pair(2), pair(3), pair(4), pair(5)
    ws = scv(12, 14); nc.vector.tensor_scalar_add(ws, Sw, EPS)
    rws = scv(14, 16); nc.vector.reciprocal(rws, ws)
    cx = scv(16, 18); nc.vector.tensor_tensor(cx, Sx, rws, OP.mult)  # = dx
    cy = scv(18, 20); nc.vector.tensor_tensor(cy, Sy, rws, OP.mult)  # = dy
    t_a = scv(20, 22); nc.vector.tensor_tensor(t_a, cx, Sx, OP.mult)
    t_b = scv(22, 24); nc.vector.tensor_tensor(t_b, cy, Sy, OP.mult)
    cdS = scv(24, 26); nc.vector.tensor_tensor(cdS, t_a, t_b, OP.add)
    u_a = scv(26, 28); nc.vector.tensor_tensor(u_a, cx, cx, OP.mult)
    u_b = scv(28, 30); nc.vector.tensor_tensor(u_b, cy, cy, OP.mult)
    c2_ = scv(30, 32); nc.vector.tensor_tensor(c2_, u_a, u_b, OP.add)
    sq_ = scv(32, 34); nc.vector.tensor_tensor(sq_, Sxx, Syy, OP.add)
    n2c = scv(34, 36); nc.vector.tensor_scalar_mul(n2c, cdS, -2.0)
    c2w = scv(36, 38); nc.vector.tensor_tensor(c2w, c2_, Sw, OP.mult)
    m_ = scv(38, 40); nc.vector.tensor_tensor(m_, sq_, n2c, OP.add)
    m2_ = scv(40, 42); nc.vector.tensor_tensor(m2_, m_, c2w, OP.add)
    md2 = scv(42, 44); nc.vector.tensor_tensor(md2, m2_, rws, OP.mult)
    md2e = scv(44, 46); nc.vector.tensor_scalar_add(md2e, md2, EPS)
    md = scv(46, 48); nc.scalar.activation(md, md2e, AF.Sqrt)
    mde = scv(48, 50); nc.vector.tensor_scalar_add(mde, md, EPS)
    rmd = scv(50, 52); nc.vector.reciprocal(rmd, mde)
    s_ = scv(52, 54); nc.vector.tensor_scalar_mul(s_, rmd, SQRT2)
    # real centroids: cr = dx + c0 ; c0s = [c0x c0x c0y c0y] paired
    cxr = scv(54, 56); nc.vector.tensor_tensor(cxr, cx, c0s[0:1, 0:2], OP.add)
    cyr = scv(56, 58); nc.vector.tensor_tensor(cyr, cy, c0s[0:1, 2:4], OP.add)
    scx = scv(58, 60); nc.vector.tensor_tensor(scx, s_, cxr, OP.mult)
    scy = scv(60, 62); nc.vector.tensor_tensor(scy, s_, cyr, OP.mult)
    nscx = scv(62, 64); nc.vector.tensor_scalar_mul(nscx, scx, -1.0)
    nscy = scv(64, 66); nc.vector.tensor_scalar_mul(nscy, scy, -1.0)
    # L scalars: s2, dx2, dxy, dy2 (paired)
    s2p = scv(66, 68); nc.vector.tensor_tensor(s2p, s_, s_, OP.mult)
    dx2 = scv(68, 70); nc.vector.tensor_tensor(dx2, cx, cx, OP.mult)
    dxy = scv(70, 72); nc.vector.tensor_tensor(dxy, cx, cy, OP.mult)
    dy2 = scv(72, 74); nc.vector.tensor_tensor(dy2, cy, cy, OP.mult)

    # T row-major 9-vectors: t1v at 76:85, t2v at 85:94
    nc.vector.memset(scv(76, 94), 0.0)
    tv = sc[0:1, 76:94]
    tv9 = tv.rearrange("p (v f) -> p v f", v=2)
    nc.vector.tensor_copy(tv9[:, :, 0:1], s_.unsqueeze(2))
    nc.vector.tensor_copy(tv9[:, :, 4:5], s_.unsqueeze(2))
    nc.vector.tensor_copy(
        tv9[:, :, 2:8].rearrange("p v (c d) -> p v c d", c=2)[:, :, :, 0:1],
        sc[0:1, 62:66].rearrange("p (c v) -> p v c", c=2).unsqueeze(3))
    nc.vector.memset(tv9[:, :, 8:9], 1.0)
    nc.sync.dma_start(stage[0:18], tv)

    # broadcast scalar strip to 6 partitions for the L build
    scB = sp.tile([6, 80], F32, tag="scB")
    nc.gpsimd.partition_broadcast(scB[:], sc[0:1, 0:80], channels=6)

    def shT(side, tag):
        """Sh^T for side (0/1): I^T + dx E1^T + dy E2^T + dx2 E3^T + ..."""
        dx = scB[:, 16 + side:17 + side]
        dy = scB[:, 18 + side:19 + side]
        dx2_ = scB[:, 68 + side:69 + side]
        dxy_ = scB[:, 70 + side:71 + side]
        dy2_ = scB[:, 72 + side:73 + side]
        def M(i):
            return cps[0:6, C_SHT + 6 * i:C_SHT + 6 * i + 6]
        acc = sp.tile([6, 6], F32, tag=f"sh_{tag}")
        nc.vector.scalar_tensor_tensor(acc[:], M(1), dx, M(0), OP.mult, OP.add)
        for i, sval in [(2, dy), (3, dx2_), (4, dxy_), (5, dy2_)]:
            nc.vector.scalar_tensor_tensor(acc[:], M(i), sval, acc[:],
                                           OP.mult, OP.add)
        return acc

    Sh1T = shT(0, "1")
    Sh2T = shT(1, "2")
    # svec side1 as a [6,1] column (per-partition): c2m*s2 + c1m*s + c0m
    sv1c = sp.tile([6, 1], F32, tag="sv1c")
    tmp1 = sp.tile([6, 1], F32, tag="svt1")
    nc.vector.scalar_tensor_tensor(
        tmp1[:], cps[0:6, C_MSK:C_MSK + 1], scB[:, 66:67],
        cps[0:6, C_MSK + 2:C_MSK + 3], OP.mult, OP.add)
    nc.vector.scalar_tensor_tensor(
        sv1c[:], cps[0:6, C_MSK + 1:C_MSK + 2], scB[:, 52:53],
        tmp1[:], OP.mult, OP.add)
    # svec side2 as a [1,6] row on partition 0: [s2 s2 s s2 s 1]
    svr2 = sc[0:1, 96:102]
    s2v2 = sc[0:1, 67:68]
    sv2 = sc[0:1, 53:54]
    nc.vector.tensor_copy(
        svr2.rearrange("p (a b) -> p a b", a=3)[:, 0:2, 0:1],
        s2v2.unsqueeze(2).to_broadcast([1, 2, 1]))   # slots 0,2 = s2 (a-major)
    nc.vector.tensor_copy(svr2[:, 1:2], s2v2)        # slot 1 = s2
    nc.vector.tensor_copy(svr2[:, 3:4], s2v2)        # slot 3 = s2
    nc.vector.tensor_copy(svr2[:, 2:3], sv2)         # slot 2 = s
    nc.vector.tensor_copy(svr2[:, 4:5], sv2)         # slot 4 = s
    nc.vector.memset(svr2[:, 5:6], 1.0)
    sv2B = sp.tile([6, 6], F32, tag="sv2B")
    nc.gpsimd.partition_broadcast(sv2B[:], svr2, channels=6)

    # C2 = D1 Sh1 C' Sh2^T D2
    vps = ps.tile([6, 6], F32, tag="tps")
    nc.tensor.matmul(vps[:], Sh1T[:], Cp[:], start=True, stop=True)  # Sh1 C'
    vS = sp.tile([6, 6], F32, tag="vS")
    nc.vector.tensor_copy(vS[:], vps[:])
    vT = _transpose(nc, ps, sp, vS[:], 6, idn, "vT")
    ups = ps.tile([6, 6], F32, tag="tps")
    nc.tensor.matmul(ups[:], vT[:], Sh2T[:], start=True, stop=True)  # v Sh2^T
    # C2[r, c] = svec1[r] * u[r, c] * svec2[c]
    u1 = sp.tile([6, 6], F32, tag="u1")
    nc.vector.tensor_scalar_mul(u1[:], ups[:], sv1c[:])
    C2 = sp.tile([6, 6], F32, tag="C2")
    nc.vector.tensor_tensor(C2[:], u1[:], sv2B[:], OP.mult)
    C2T = _transpose(nc, ps, sp, C2[:], 6, idn, "c2t")

    _solve(nc, pp, sp, ps, cps, idn, sc, C2[:], C2T[:], stage, mshuf, out_d)


def _solve(nc, pp, sp, ps, cps, idn, sc, C2, C2T, stage, mshuf, out_d):
    i9h = cps[0:9, C_I9H:C_I9H + 9]
    et69 = cps[0:6, C_ET69:C_ET69 + 9]
    i3c = cps[0:3, C_I3:C_I3 + 3]
    v09 = cps[0:9, C_V09:C_V09 + 1]
    v06 = cps[0:6, C_V06:C_V06 + 1]
    sel1 = cps[0:3, C_SEL1:C_SEL1 + 6]
    sel2 = cps[0:3, C_SEL2:C_SEL2 + 6]

    # G2 = E C2 E^T : G2[3a+b, 3c+d] = C2[pair(a,b), pair(c,d)]
    z_ps = ps.tile([6, 9], F32, tag="tps")
    nc.tensor.matmul(z_ps[:], C2T, et69, start=True, stop=True)  # C2 E^T
    Zs = sp.tile([6, 9], F32, tag="Zs")
    nc.vector.tensor_copy(Zs[:], z_ps[:])
    g_ps = ps.tile([9, 9], F32, tag="tps")
    nc.tensor.matmul(g_ps[:], et69, Zs[:], start=True, stop=True)    # E @ Z
    G2 = sp.tile([9, 9], F32, tag="G2")
    nc.vector.tensor_copy(G2[:], g_ps[:])

    # Mmat[3p+q, 3r+s] = G2[3p+r, 3q+s]: bounce via DRAM
    nc.sync.dma_start(mshuf[:], G2[:])
    Mmat = sp.tile([9, 9], F32, tag="Mmat")
    for p in range(3):
        nc.sync.dma_start(
            Mmat[3 * p:3 * p + 3, :].rearrange("q (r s) -> q r s", s=3),
            mshuf[:].rearrange("(p q1 r s) -> p q1 r s", p=3, q1=3, r=3)
            .transpose([0, 2, 1, 3])[p])

    # Msp = Mmat/(2 lam) - I/2
    dg = sp.tile([9, 9], F32, tag="dg")
    nc.vector.tensor_tensor(dg[:], Mmat[:], i9h, OP.mult)
    lam2 = sp.tile([9, 1], F32, tag="lam2")
    nc.vector.tensor_reduce(lam2[:], dg[:], AX.X, OP.add)
    lam2r = sp.tile([9, 1], F32, tag="lam2r")
    nc.gpsimd.partition_all_reduce(lam2r[:], lam2[:], channels=9,
                                   reduce_op=bass_isa.ReduceOp.add)
    lam4 = sp.tile([9, 1], F32, tag="lam4")
    nc.vector.tensor_scalar_mul(lam4[:], lam2r[:], 4.0)
    inv2l = sp.tile([9, 1], F32, tag="inv2l")
    nc.vector.reciprocal(inv2l[:], lam4[:])
    Msp = sp.tile([9, 9], F32, tag="Msp")
    nc.vector.scalar_tensor_tensor(Msp[:], Mmat[:], inv2l[:], i9h,
                                   OP.mult, OP.subtract)
    M50 = _powchain(nc, ps, sp, Msp[:], 9, "m9", 5, extra=True)

    w9ps = ps.tile([1, 9], F32, tag="tps")
    nc.tensor.matmul(w9ps[:], v09, M50[:], start=True, stop=True)
    w9 = sp.tile([1, 9], F32, tag="w9")
    nc.vector.tensor_copy(w9[:], w9ps[:])
    w9sq = sp.tile([1, 9], F32, tag="w9sq")
    nc.vector.tensor_tensor(w9sq[:], w9[:], w9[:], OP.mult)
    nn9 = sp.tile([1, 1], F32, tag="nn9")
    nc.vector.tensor_reduce(nn9[:], w9sq[:], AX.X, OP.add)
    sr9 = sp.tile([1, 1], F32, tag="sr9")
    nc.scalar.activation(sr9[:], nn9[:], AF.Sqrt)
    rs9 = sp.tile([1, 1], F32, tag="rs9")
    nc.vector.reciprocal(rs9[:], sr9[:])
    v9 = sp.tile([1, 9], F32, tag="v9")
    nc.vector.tensor_tensor(v9[:], w9[:], rs9[:].to_broadcast([1, 9]), OP.mult)
    nc.sync.dma_start(stage[24:33], v9[:])

    # E = T2^T E_raw T1 (and E^T)
    T1m = sp.tile([3, 3], F32, tag="T1m")
    nc.sync.dma_start(T1m[:], stage[0:9].rearrange("(i j) -> i j", j=3))
    T2m = sp.tile([3, 3], F32, tag="T2m")
    nc.sync.dma_start(T2m[:], stage[9:18].rearrange("(i j) -> i j", j=3))
    Eraw = sp.tile([3, 3], F32, tag="Eraw")
    nc.sync.dma_start(Eraw[:], stage[24:33].rearrange("(i j) -> i j", j=3))

    a1ps = ps.tile([3, 3], F32, tag="tps")
    nc.tensor.matmul(a1ps[:], T2m[:], Eraw[:], start=True, stop=True)
    A1 = sp.tile([3, 3], F32, tag="A1")
    nc.vector.tensor_copy(A1[:], a1ps[:])
    A1T = _transpose(nc, ps, sp, A1[:], 3, idn, "a1t")
    etps = ps.tile([3, 3], F32, tag="tps")
    nc.tensor.matmul(etps[:], T1m[:], A1T[:], start=True, stop=True)
    ETs = sp.tile([3, 3], F32, tag="ETs")
    nc.vector.tensor_copy(ETs[:], etps[:])
    Es = _transpose(nc, ps, sp, ETs[:], 3, idn, "es")

    # B = E^T E ; blockdiag 6x6 chain (32 iters) for v1 (max) and v3 (min)
    bps = ps.tile([3, 3], F32, tag="tps")
    nc.tensor.matmul(bps[:], Es[:], Es[:], start=True, stop=True)
    Bm = sp.tile([3, 3], F32, tag="Bm")
    nc.vector.tensor_copy(Bm[:], bps[:])
    dg3 = sp.tile([3, 3], F32, tag="dg3")
    nc.vector.tensor_tensor(dg3[:], Bm[:], i3c, OP.mult)
    lb = sp.tile([3, 1], F32, tag="lb")
    nc.vector.tensor_reduce(lb[:], dg3[:], AX.X, OP.add)
    lbr = sp.tile([3, 1], F32, tag="lbr")
    nc.gpsimd.partition_all_reduce(lbr[:], lb[:], channels=3,
                                   reduce_op=bass_isa.ReduceOp.add)
    invlb = sp.tile([3, 1], F32, tag="invlb")
    nc.vector.reciprocal(invlb[:], lbr[:])
    Bs3 = sp.tile([3, 3], F32, tag="Bs3")
    nc.vector.tensor_scalar_mul(Bs3[:], Bm[:], invlb[:])
    IB = sp.tile([3, 3], F32, tag="IB")
    nc.vector.tensor_tensor(IB[:], i3c, Bs3[:], OP.subtract)
    bdps = ps.tile([6, 6], F32, tag="tps")
    nc.tensor.matmul(bdps[:, 0:3], sel1, Bs3[:], start=True, stop=True)
    nc.tensor.matmul(bdps[:, 3:6], sel2, IB[:], start=True, stop=True)
    BD = sp.tile([6, 6], F32, tag="BD")
    nc.vector.tensor_copy(BD[:], bdps[:])
    BD32 = _powchain(nc, ps, sp, BD[:], 6, "m6", 5, extra=False)

    w6ps = ps.tile([1, 6], F32, tag="tps")
    nc.tensor.matmul(w6ps[:], v06, BD32[:], start=True, stop=True)
    w6 = sp.tile([1, 6], F32, tag="w6")
    nc.vector.tensor_copy(w6[:], w6ps[:])
    w6sq = sp.tile([1, 6], F32, tag="w6sq")
    nc.vector.tensor_tensor(w6sq[:], w6[:], w6[:], OP.mult)
    nn6 = sp.tile([1, 2], F32, tag="nn6")
    nc.vector.tensor_reduce(nn6[:].unsqueeze(2),
                            w6sq[:].rearrange("p (g d) -> p g d", g=2), AX.X,
                            OP.add)
    sr6 = sp.tile([1, 2], F32, tag="sr6")
    nc.scalar.activation(sr6[:], nn6[:], AF.Sqrt)
    rs6 = sp.tile([1, 2], F32, tag="rs6")
    nc.vector.reciprocal(rs6[:], sr6[:])
    vv = sp.tile([1, 6], F32, tag="vv")
    nc.vector.tensor_tensor(
        vv[:].rearrange("p (g d) -> p g d", g=2),
        w6[:].rearrange("p (g d) -> p g d", g=2),
        rs6[:].unsqueeze(2).to_broadcast([1, 2, 3]), OP.mult)

    # v2 = cross(v3, v1), normalized with EPS
    aa = sp.tile([1, 6], F32, tag="aa")
    nc.vector.tensor_copy(
        aa[:].rearrange("p (r d) -> p r d", r=2),
        vv[:, 3:6].unsqueeze(1).to_broadcast([1, 2, 3]))
    bb = sp.tile([1, 6], F32, tag="bb")
    nc.vector.tensor_copy(
        bb[:].rearrange("p (r d) -> p r d", r=2),
        vv[:, 0:3].unsqueeze(1).to_broadcast([1, 2, 3]))
    cr1 = sp.tile([1, 3], F32, tag="cr1")
    nc.vector.tensor_tensor(cr1[:], aa[:, 1:4], bb[:, 2:5], OP.mult)
    cr2 = sp.tile([1, 3], F32, tag="cr2")
    nc.vector.tensor_tensor(cr2[:], aa[:, 2:5], bb[:, 1:4], OP.mult)
    v2r = sp.tile([1, 3], F32, tag="v2r")
    nc.vector.tensor_tensor(v2r[:], cr1[:], cr2[:], OP.subtract)
    v2sq = sp.tile([1, 3], F32, tag="v2sq")
    nc.vector.tensor_tensor(v2sq[:], v2r[:], v2r[:], OP.mult)
    nn2 = sp.tile([1, 1], F32, tag="nn2")
    nc.vector.tensor_reduce(nn2[:], v2sq[:], AX.X, OP.add)
    sr2 = sp.tile([1, 1], F32, tag="sr2")
    nc.scalar.activation(sr2[:], nn2[:], AF.Sqrt)
    sr2e = sp.tile([1, 1], F32, tag="sr2e")
    nc.vector.tensor_scalar_add(sr2e[:], sr2[:], EPS)
    rs2 = sp.tile([1, 1], F32, tag="rs2")
    nc.vector.reciprocal(rs2[:], sr2e[:])
    v2 = sp.tile([1, 3], F32, tag="v2")
    nc.vector.tensor_tensor(v2[:], v2r[:], rs2[:].to_broadcast([1, 3]), OP.mult)

    nc.sync.dma_start(stage[33:36], vv[:, 0:3])
    nc.sync.dma_start(stage[36:39], v2[:])
    Vc = sp.tile([3, 2], F32, tag="Vc")
    nc.sync.dma_start(Vc[:], stage[33:39].rearrange("(i k) -> k i", k=3))
    Vr = sp.tile([2, 3], F32, tag="Vr")
    nc.sync.dma_start(Vr[:], stage[33:39].rearrange("(i k) -> i k", k=3))
    evps = ps.tile([2, 3], F32, tag="tps")
    nc.tensor.matmul(evps[:], Vc[:], ETs[:], start=True, stop=True)
    Evr = sp.tile([2, 3], F32, tag="Evr")
    nc.vector.tensor_copy(Evr[:], evps[:])
    evsq = sp.tile([2, 3], F32, tag="evsq")
    nc.vector.tensor_tensor(evsq[:], Evr[:], Evr[:], OP.mult)
    ss2 = sp.tile([2, 1], F32, tag="ss2")
    nc.vector.tensor_reduce(ss2[:], evsq[:], AX.X, OP.add)
    sv = sp.tile([2, 1], F32, tag="sv")
    nc.scalar.activation(sv[:], ss2[:], AF.Sqrt)
    ssum = sp.tile([2, 1], F32, tag="ssum")
    nc.gpsimd.partition_all_reduce(ssum[:], sv[:], channels=2,
                                   reduce_op=bass_isa.ReduceOp.add)
    savg = sp.tile([2, 1], F32, tag="savg")
    nc.vector.tensor_scalar_mul(savg[:], ssum[:], 0.5)
    sve = sp.tile([2, 1], F32, tag="sve")
    nc.vector.tensor_scalar_add(sve[:], sv[:], EPS)
    rsv = sp.tile([2, 1], F32, tag="rsv")
    nc.vector.reciprocal(rsv[:], sve[:])
    f2 = sp.tile([2, 1], F32, tag="f2")
    nc.vector.tensor_tensor(f2[:], rsv[:], savg[:], OP.mult)
    U2 = sp.tile([2, 3], F32, tag="U2")
    nc.vector.tensor_scalar_mul(U2[:], Evr[:], f2[:])
    ops_ = ps.tile([3, 3], F32, tag="tps")
    nc.tensor.matmul(ops_[:], U2[:], Vr[:], start=True, stop=True)
    outs = sp.tile([3, 3], F32, tag="outs")
    nc.vector.tensor_copy(outs[:], ops_[:])
    nc.sync.dma_start(out_d[:], outs[:])


def make_in_maps(P, K):
    P = np.asarray(P, np.float32)
    K = np.asarray(K, np.float32)
    Pc = np.ascontiguousarray(P[:N, :N])
    PcT = np.ascontiguousarray(Pc.T)
    Mp, cpack, c0x, c0y = host_constants(K)
    m1full = _tile128(Mp, CB)
    c0t = np.array([[c0x, c0x, c0y, c0y]], np.float32)
    in_maps = []
    for k in range(NCORES):
        in_maps.append({
            "xn": _tile128(Pc[k * SH:(k + 1) * SH], RT),
            "xc": _tile128(PcT[k * SH:(k + 1) * SH], RT),
            "m1f": m1full,
            "m2s": _tile128(Mp[k * SH:(k + 1) * SH], RT),
            "cpack": cpack,
            "c0t": c0t,
        })
    return in_maps


_NC_CACHE = {}


def kernel(P, K):
    from concourse.bass_utils import run_bass_kernel_spmd
    if "nc" not in _NC_CACHE:
        _NC_CACHE["nc"] = build_nc()
    nc = _NC_CACHE["nc"]
    in_maps = make_in_maps(P, K)
    res = run_bass_kernel_spmd(nc, in_maps, core_ids=list(range(NCORES)))
    return np.asarray(res.results[0]["out"], np.float32)


# revision 16
# speedup vs baseline: 1.3360x; 1.1633x over previous
"""Trainium2 Bass kernel for nn_EssentialMatrixEstimator (v2).

Distribution (8 cores):
  - XN: natural row-shard  (384 rows x 3072 cols) -> exact row top-3 thresholds.
  - XC: transposed col-shard (384 cols x 3072 rows as [col, row]) -> exact col
    top-3 thresholds + dense masking + col-sharded gram.
  - coll1: AllGather of per-core row thresholds (384 f32 -> 3072).
  - coll2: AllReduce of the 6x6 gram C' on PRE-CENTERED monomials.

Math: the (N*M,9) epipolar Gram collapses to the 6x6 monomial Gram C'.
Monomials are pre-centered about the host constant c0 (grid centroid), so C'
is well-conditioned; the Hartley normalization is recovered from C' moments
(row/col 5) and applied as a 6x6 L-transform C2 = L1 C' L2^T instead of a
second gram pass.  Mmat (9x9) is an index expansion of C2; min-eigvector via
50-step shifted power iteration (rescaled repeated squaring), projection via
a 32-step 6x6 blockdiag chain (insensitive; validated 2.9e-4).

The big T = M2'^T W^T contraction streams in float32r (1 cy/row); validated
tolerant to tf32/bf16-level rounding (5e-4 / 3.9e-3 final rel err).
"""

import os

os.environ.setdefault("JAX_PLATFORMS", "axon")

import numpy as np

import concourse.bass as bass
import concourse.bass_isa as bass_isa
import concourse.mybir as mybir
import concourse.bacc as bacc
import concourse.tile as tile

NCORES = 8
N = 3072
SH = N // NCORES          # 384 rows/cols per core
RT = SH // 128            # 3 tiles per core shard
CB = N // 128             # 24 tiles across the full dim
F32 = mybir.dt.float32
F32R = mybir.dt.float32r
AF = mybir.ActivationFunctionType
OP = mybir.AluOpType
AX = mybir.AxisListType

EPS = 1e-8
SQRT2 = 1.4142135623730951
INV_SQRT3 = 1.0 / 1.7320508075688772
T0 = float(np.nextafter(np.float32(0.01), np.float32(1)))  # x > 0.01 == x >= T0
H, W = 64, 64

# cpack const layout (tensor [9, C_TOT]): column ranges
C_I9H = 0      # I9 * 0.5            [9, 9]
C_ET69 = 9     # E^T selector        [6, 9]
C_I3 = 18      # I3                  [3, 3]
C_V09 = 21     # full(1/3)           [9, 1]
C_V06 = 22     # full(1/sqrt3)       [6, 1]
C_SEL1 = 23    # [I3 | 0]            [3, 6]
C_SEL2 = 29    # [0 | I3]            [3, 6]
C_SHT = 35     # Sh component mats^T: I6, E1^T..E5^T   [6, 6*6]
C_MSK = 71     # svec masks [c2m c1m c0m]  [6, 3]
C_IDN = 74     # identity 9x9        [9, 9]
C_TOT = 83

PAIRS = [(0, 0), (0, 1), (0, 2), (1, 1), (1, 2), (2, 2)]


def _pidx():
    d = {}
    for i, (a, b) in enumerate(PAIRS):
        d[(a, b)] = i
        d[(b, a)] = i
    return d


def grid_pts(K):
    idx = np.arange(H * W, dtype=np.float32)
    pix = np.stack([idx % np.float32(W), np.floor(idx / np.float32(W))], -1)
    K_inv = np.linalg.inv(np.asarray(K, np.float32)).astype(np.float32)
    p1h = np.concatenate([pix[:N], np.ones((N, 1), np.float32)], -1)
    pts = (p1h @ K_inv.T)[:, :2].astype(np.float32)
    return pts


def host_constants(K):
    """Pre-centered monomials + packed tail constants (f32)."""
    pts = grid_pts(K)
    x, y = pts[:, 0], pts[:, 1]
    c0x = np.float32(x.mean())
    c0y = np.float32(y.mean())
    xs = (x - c0x).astype(np.float32)
    ys = (y - c0y).astype(np.float32)
    Mp = np.stack([xs * xs, xs * ys, xs, ys * ys, ys, np.ones_like(xs)],
                  -1).astype(np.float32)

    cpack = np.zeros((9, C_TOT), np.float32)
    cpack[:9, C_I9H:C_I9H + 9] = 0.5 * np.eye(9, dtype=np.float32)
    pid = _pidx()
    for a in range(3):
        for b in range(3):
            cpack[pid[(a, b)], C_ET69 + 3 * a + b] = 1.0
    cpack[:3, C_I3:C_I3 + 3] = np.eye(3, dtype=np.float32)
    cpack[:9, C_V09] = 1.0 / 3.0
    cpack[:6, C_V06] = INV_SQRT3
    cpack[:3, C_SEL1:C_SEL1 + 3] = np.eye(3, dtype=np.float32)
    cpack[:3, C_SEL2 + 3:C_SEL2 + 6] = np.eye(3, dtype=np.float32)

    # Sh(dx,dy) = I + dx*E1 + dy*E2 + dx^2*E3 + dx*dy*E4 + dy^2*E5
    # (rows of L before the diag scale; see proto.Lmat)
    E1 = np.zeros((6, 6), np.float32)  # dx terms
    E1[0, 2] = -2.0
    E1[1, 4] = -1.0
    E1[2, 5] = -1.0
    E2 = np.zeros((6, 6), np.float32)  # dy terms
    E2[1, 2] = -1.0
    E2[3, 4] = -2.0
    E2[4, 5] = -1.0
    E3 = np.zeros((6, 6), np.float32)  # dx^2
    E3[0, 5] = 1.0
    E4 = np.zeros((6, 6), np.float32)  # dx*dy
    E4[1, 5] = 1.0
    E5 = np.zeros((6, 6), np.float32)  # dy^2
    E5[3, 5] = 1.0
    mats = [np.eye(6, dtype=np.float32), E1, E2, E3, E4, E5]
    for i, Em in enumerate(mats):
        cpack[:6, C_SHT + 6 * i:C_SHT + 6 * i + 6] = Em.T
    # svec masks: svec = [s2,s2,s,s2,s,1] = c2m*s2 + c1m*s + c0m
    cpack[:6, C_MSK + 0] = [1, 1, 0, 1, 0, 0]
    cpack[:6, C_MSK + 1] = [0, 0, 1, 0, 1, 0]
    cpack[:6, C_MSK + 2] = [0, 0, 0, 0, 0, 1]
    cpack[:9, C_IDN:C_IDN + 9] = np.eye(9, dtype=np.float32)
    return Mp, cpack, float(c0x), float(c0y)


def _tile128(a, ntiles):
    """[ntiles*128, F] -> [128, ntiles*F] with [p, t*F+f] = a[t*128+p, f]."""
    F = a.shape[1]
    return np.ascontiguousarray(
        a.reshape(ntiles, 128, F).transpose(1, 0, 2).reshape(128, ntiles * F)
    )


def build_nc(repeats=1, no_coll=False, no_tail=False, use_f32r=True, dbg_c=False):
    nc = bacc.Bacc("TRN2", target_bir_lowering=False, debug=False,
                   num_devices=NCORES)

    xn = nc.dram_tensor("xn", [128, RT * N], F32, kind="ExternalInput")
    xc = nc.dram_tensor("xc", [128, RT * N], F32, kind="ExternalInput")
    m1f = nc.dram_tensor("m1f", [128, CB * 6], F32, kind="ExternalInput")
    m2s = nc.dram_tensor("m2s", [128, RT * 6], F32, kind="ExternalInput")
    cpk = nc.dram_tensor("cpack", [9, C_TOT], F32, kind="ExternalInput")
    c0t = nc.dram_tensor("c0t", [1, 4], F32, kind="ExternalInput")
    out_d = nc.dram_tensor("out", [6, 6] if dbg_c else [3, 3], F32, kind="ExternalOutput")

    tr_in = nc.dram_tensor("tr_in", [1, SH], F32)
    tr_out = nc.dram_tensor("tr_out", [NCORES, SH], F32, addr_space="Shared")
    cr_in = nc.dram_tensor("cr_in", [6, 6], F32)
    cr_out = nc.dram_tensor("cr_out", [6, 6], F32, addr_space="Shared")
    tb = nc.dram_tensor("tb", [6, N], F32)        # T bounce
    stage = nc.dram_tensor("stage", [64], F32)
    mshuf = nc.dram_tensor("mshuf", [81], F32)

    groups = [list(range(NCORES))]

    with tile.TileContext(nc) as tc:
        with (
            tc.tile_pool(name="persist", bufs=1) as pp,
            tc.tile_pool(name="scratch", bufs=2) as sp,
            tc.tile_pool(name="ps_t", bufs=2, space="PSUM") as ps,
            tc.tile_pool(name="ps_T", bufs=2, space="PSUM") as psT,
            tc.tile_pool(name="ps_c", bufs=1, space="PSUM") as psc,
        ):
            for _rep in range(repeats):
                # ---------- P0: loads (XN on qSP, XC on qACT) ----------
                XN = pp.tile([128, RT * N], F32, tag="XN")
                XC = pp.tile([128, RT * N], F32, tag="XC")
                for t in range(RT):
                    nc.sync.dma_start(XN[:, t * N:(t + 1) * N],
                                      xn[:, t * N:(t + 1) * N])
                    nc.scalar.dma_start(XC[:, t * N:(t + 1) * N],
                                        xc[:, t * N:(t + 1) * N])
                m1s_s = pp.tile([128, CB * 6], F32, tag="m1f")
                nc.scalar.dma_start(m1s_s[:], m1f[:])
                m2s_s = pp.tile([128, RT * 6], F32, tag="m2s")
                nc.scalar.dma_start(m2s_s[:], m2s[:])
                cps = pp.tile([9, C_TOT], F32, tag="cpk")
                nc.scalar.dma_start(cps[:], cpk[:])
                c0s = pp.tile([1, 4], F32, tag="c0")
                nc.scalar.dma_start(c0s[:], c0t[:])

                def XNt(t):
                    return XN[:, t * N:(t + 1) * N]

                def XCt(t):
                    return XC[:, t * N:(t + 1) * N]

                # ---------- P1: row thresholds -> coll1 ----------
                r8 = pp.tile([128, RT * 8], F32, tag="r8")
                for t in range(RT):
                    nc.vector.max(out=r8[:, t * 8:t * 8 + 8], in_=XNt(t))
                trT0 = pp.tile([128, RT], F32, tag="trT0")
                nc.vector.tensor_scalar_max(
                    trT0[:],
                    r8[:].rearrange("p (t e) -> p t e", e=8)[:, :, 2], T0)
                for t in range(RT):
                    nc.sync.dma_start(tr_in[0:1, t * 128:(t + 1) * 128],
                                      trT0[:, t:t + 1])

                if no_coll:
                    nc.sync.dma_start(tr_out[0:1, :], tr_in[:])
                else:
                    nc.gpsimd.collective_compute(
                        "AllGather", OP.bypass, replica_groups=groups,
                        ins=[tr_in[:]], outs=[tr_out[:]])

                # ---------- P2: col thresholds (local, exact) ----------
                c8 = pp.tile([128, RT * 8], F32, tag="c8")
                for t in range(RT):
                    nc.vector.max(out=c8[:, t * 8:t * 8 + 8], in_=XCt(t))

                # ---------- P3: broadcast row-threshold table ----------
                trow = pp.tile([1, N], F32, tag="trow")
                nc.sync.dma_start(trow[:], tr_out[:].rearrange("k i -> (k i)"))
                trB = pp.tile([128, N], F32, tag="trB")
                MCH = 1536
                for c0_ in range(0, N, MCH):
                    nc.gpsimd.partition_broadcast(
                        trB[:, c0_:c0_ + MCH], trow[:, c0_:c0_ + MCH],
                        channels=128)

                # ---------- P4: dense mask + fp32r T-gram ----------
                # W (f32r): W = XC * [XC >= max(trB, tc_t)]
                # T[b, r] = sum_c m2'[c, b] * W^T[c, r]   (PSUM chunks [6,512])
                WDT = F32R if use_f32r else F32
                m2r = pp.tile([128, RT * 6], WDT, tag="m2r")
                nc.vector.tensor_copy(m2r[:], m2s_s[:])
                Wr = pp.tile([128, RT * N], WDT, tag="Wr")
                Wf = pp.tile([128, N], F32, tag="Wf")  # t=2 chunk via gpsimd
                Tsb = sp.tile([6, N], F32, tag="Tsb")
                TT = sp.tile([128, CB * 6], F32, tag="TT")
                i6 = cps[0:6, C_IDN:C_IDN + 6]
                for h in range(2):
                    for t in range(RT):
                        tcl = c8[:, t * 8 + 2:t * 8 + 3]
                        sl = slice(t * N + h * MCH, t * N + (h + 1) * MCH)
                        msk = sp.tile([128, MCH], F32, tag="msk")
                        nc.vector.scalar_tensor_tensor(
                            msk[:], trB[:, h * MCH:(h + 1) * MCH], tcl,
                            XC[:, sl], OP.max, OP.is_le)
                        if t == RT - 1:
                            nc.gpsimd.tensor_tensor(
                                Wf[:, h * MCH:(h + 1) * MCH], XC[:, sl],
                                msk[:], OP.mult)
                        else:
                            nc.vector.tensor_tensor(Wr[:, sl], XC[:, sl],
                                                    msk[:], OP.mult)
                    for q in range(3):
                        ch = h * 3 + q
                        Tp = psT.tile([6, 512], F32, tag="Tp")
                        for t in range(RT):
                            c0_ = t * N + h * MCH + q * 512
                            if t == RT - 1:
                                nc.tensor.matmul(
                                    Tp[:], m2s_s[:, t * 6:(t + 1) * 6],
                                    Wf[:, h * MCH + q * 512:
                                        h * MCH + q * 512 + 512],
                                    start=False, stop=True)
                            else:
                                nc.tensor.matmul(
                                    Tp[:],
                                    m2r[:, t * 6:(t + 1) * 6],
                                    Wr[:, c0_:c0_ + 512],
                                    start=(t == 0), stop=False)
                        nc.scalar.activation(Tsb[:, ch * 512:(ch + 1) * 512],
                                             Tp[:], AF.Copy)
                        # PE-transpose T chunk into TT[p, (j b)] blocks
                        for jj in range(4):
                            j = ch * 4 + jj
                            pt = ps.tile([128, 6], F32, tag="ptT")
                            nc.tensor.transpose(
                                pt[:], Tsb[:, j * 128:(j + 1) * 128], i6)
                            nc.scalar.activation(TT[:, j * 6:(j + 1) * 6],
                                                 pt[:], AF.Copy)

                # C[a, b] = sum_j m1'_j^T TT_j
                pc = psc.tile([6, 6], F32, tag="pc")
                for j in range(CB):
                    nc.tensor.matmul(pc[:], m1s_s[:, j * 6:(j + 1) * 6],
                                     TT[:, j * 6:(j + 1) * 6],
                                     start=(j == 0), stop=(j == CB - 1))
                Cp = sp.tile([6, 6], F32, tag="Cp")
                nc.vector.tensor_copy(Cp[:], pc[:])
                nc.sync.dma_start(cr_in[:], Cp[:])

                # ---------- coll2: AllReduce 6x6 gram ----------
                if no_coll:
                    nc.sync.dma_start(cr_out[:], cr_in[:])
                else:
                    nc.gpsimd.collective_compute(
                        "AllReduce", OP.add, replica_groups=groups,
                        ins=[cr_in[:]], outs=[cr_out[:]])

                if no_tail:
                    nn = 6 if dbg_c else 3
                    dummy = sp.tile([nn, nn], F32, tag="dummy")
                    nc.sync.dma_start(dummy[:], cr_out[0:nn, 0:nn])
                    nc.sync.dma_start(out_d[:], dummy[:])
                    continue

                # ---------- tail ----------
                _tail(nc, pp, sp, ps, cps, c0s, cr_out, stage, mshuf, out_d)

    nc.compile()
    return nc


def _transpose(nc, ps, sp, in_sb, n, idn, tag):
    pt = ps.tile([n, n], F32, tag="tps")
    nc.tensor.transpose(pt[:], in_sb, idn[:n, :n])
    ot = sp.tile([n, n], F32, tag=f"ot_{tag}")
    nc.vector.tensor_copy(ot[:], pt[:])
    return ot


def _powchain(nc, ps, sp, m_sb, n, tag, n_squarings=5, extra=True):
    """M^50 (extra=True: 5 squarings + M48=M32@M16 + M50=M48@M2) or M^32."""
    powers = {}
    cur = m_sb
    for i in range(1, n_squarings + 1):
        pm = ps.tile([n, n], F32, tag="tps")
        nc.tensor.matmul(pm[:], cur, cur, start=True, stop=True)
        nxt = sp.tile([n, n], F32, tag=f"pw_{tag}_{i}")
        nc.vector.tensor_scalar_mul(nxt[:], pm[:], 2.0)
        powers[2 ** i] = nxt
        cur = nxt[:]
    if not extra:
        return powers[2 ** n_squarings]
    pm = ps.tile([n, n], F32, tag="tps")
    nc.tensor.matmul(pm[:], powers[32][:], powers[16][:], start=True, stop=True)
    m48 = sp.tile([n, n], F32, tag=f"pw_{tag}_48")
    nc.vector.tensor_scalar_mul(m48[:], pm[:], 2.0)
    pm = ps.tile([n, n], F32, tag="tps")
    nc.tensor.matmul(pm[:], m48[:], powers[2][:], start=True, stop=True)
    m50 = sp.tile([n, n], F32, tag=f"pw_{tag}_50")
    nc.vector.tensor_scalar_mul(m50[:], pm[:], 2.0)
    return m50


def _tail(nc, pp, sp, ps, cps, c0s, cr_out, stage, mshuf, out_d):
    """C' -> Hartley -> L-transform -> Mmat -> chains -> projection."""
    idn = cps[0:9, C_IDN:C_IDN + 9]

    Cp = sp.tile([6, 6], F32, tag="Cpr")
    nc.sync.dma_start(Cp[:], cr_out[:])
    CpT = sp.tile([6, 6], F32, tag="CprT")
    nc.scalar.dma_start(CpT[:], cr_out[:].rearrange("a b -> b a"))

    # moments [1,12]: side1 = C'[:,5], side2 = C'[5,:]
    sc = pp.tile([128, 112], F32, tag="tailsc")
    nc.scalar.dma_start(sc[0:1, 0:6],
                        cr_out[:].rearrange("a b -> b a")[5:6, :])
    nc.sync.dma_start(sc[0:1, 6:12], cr_out[5:6, :])

    def scv(a, b):
        return sc[0:1, a:b]

    def pair(k):
        return sc[0:1, 0:12].rearrange("p (g d) -> p d g", g=2)[:, k, :]

    Sxx, Sx, Syy, Sy, Sw = pair(0), pair(2), pair(3), pair(4), pair(5)
    ws = scv(12, 14); nc.vector.tensor_scalar_add(ws, Sw, EPS)
    rws = scv(14, 16); nc.vector.reciprocal(rws, ws)
    cx = scv(16, 18); nc.vector.tensor_tensor(cx, Sx, rws, OP.mult)  # = dx
    cy = scv(18, 20); nc.vector.tensor_tensor(cy, Sy, rws, OP.mult)  # = dy
    t_a = scv(20, 22); nc.vector.tensor_tensor(t_a, cx, Sx, OP.mult)
    t_b = scv(22, 24); nc.vector.tensor_tensor(t_b, cy, Sy, OP.mult)
    cdS = scv(24, 26); nc.vector.tensor_tensor(cdS, t_a, t_b, OP.add)
    u_a = scv(26, 28); nc.vector.tensor_tensor(u_a, cx, cx, OP.mult)
    u_b = scv(28, 30); nc.vector.tensor_tensor(u_b, cy, cy, OP.mult)
    c2_ = scv(30, 32); nc.vector.tensor_tensor(c2_, u_a, u_b, OP.add)
    sq_ = scv(32, 34); nc.vector.tensor_tensor(sq_, Sxx, Syy, OP.add)
    n2c = scv(34, 36); nc.vector.tensor_scalar_mul(n2c, cdS, -2.0)
    c2w = scv(36, 38); nc.vector.tensor_tensor(c2w, c2_, Sw, OP.mult)
    m_ = scv(38, 40); nc.vector.tensor_tensor(m_, sq_, n2c, OP.add)
    m2_ = scv(40, 42); nc.vector.tensor_tensor(m2_, m_, c2w, OP.add)
    md2 = scv(42, 44); nc.vector.tensor_tensor(md2, m2_, rws, OP.mult)
    md2e = scv(44, 46); nc.vector.tensor_scalar_add(md2e, md2, EPS)
    md = scv(46, 48); nc.scalar.activation(md, md2e, AF.Sqrt)
    mde = scv(48, 50); nc.vector.tensor_scalar_add(mde, md, EPS)
    rmd = scv(50, 52); nc.vector.reciprocal(rmd, mde)
    s_ = scv(52, 54); nc.vector.tensor_scalar_mul(s_, rmd, SQRT2)
    # real centroids: cr = dx + c0 ; c0s = [c0x c0x c0y c0y] paired
    cxr = scv(54, 56); nc.vector.tensor_tensor(cxr, cx, c0s[0:1, 0:2], OP.add)
    cyr = scv(56, 58); nc.vector.tensor_tensor(cyr, cy, c0s[0:1, 2:4], OP.add)
    scx = scv(58, 60); nc.vector.tensor_tensor(scx, s_, cxr, OP.mult)
    scy = scv(60, 62); nc.vector.tensor_tensor(scy, s_, cyr, OP.mult)
    nscx = scv(62, 64); nc.vector.tensor_scalar_mul(nscx, scx, -1.0)
    nscy = scv(64, 66); nc.vector.tensor_scalar_mul(nscy, scy, -1.0)
    # L scalars: s2, dx2, dxy, dy2 (paired)
    s2p = scv(66, 68); nc.vector.tensor_tensor(s2p, s_, s_, OP.mult)
    dx2 = scv(68, 70); nc.vector.tensor_tensor(dx2, cx, cx, OP.mult)
    dxy = scv(70, 72); nc.vector.tensor_tensor(dxy, cx, cy, OP.mult)
    dy2 = scv(72, 74); nc.vector.tensor_tensor(dy2, cy, cy, OP.mult)

    # T row-major 9-vectors: t1v at 76:85, t2v at 85:94
    nc.vector.memset(scv(76, 94), 0.0)
    tv = sc[0:1, 76:94]
    tv9 = tv.rearrange("p (v f) -> p v f", v=2)
    nc.vector.tensor_copy(tv9[:, :, 0:1], s_.unsqueeze(2))
    nc.vector.tensor_copy(tv9[:, :, 4:5], s_.unsqueeze(2))
    nc.vector.tensor_copy(
        tv9[:, :, 2:8].rearrange("p v (c d) -> p v c d", c=2)[:, :, :, 0:1],
        sc[0:1, 62:66].rearrange("p (c v) -> p v c", c=2).unsqueeze(3))
    nc.vector.memset(tv9[:, :, 8:9], 1.0)
    nc.sync.dma_start(stage[0:18], tv)
    T12 = sp.tile([3, 6], F32, tag="T12")
    nc.sync.dma_start(
        T12[:].rearrange("i (v j) -> i v j", v=2),
        stage[0:18].rearrange("(v i j) -> i v j", i=3, j=3))

    # broadcast scalar strip to 6 partitions for the L build
    scB = sp.tile([6, 80], F32, tag="scB")
    nc.gpsimd.partition_broadcast(scB[:], sc[0:1, 0:80], channels=6)

    def shT(side, tag):
        """Sh^T for side (0/1): I^T + dx E1^T + dy E2^T + dx2 E3^T + ..."""
        dx = scB[:, 16 + side:17 + side]
        dy = scB[:, 18 + side:19 + side]
        dx2_ = scB[:, 68 + side:69 + side]
        dxy_ = scB[:, 70 + side:71 + side]
        dy2_ = scB[:, 72 + side:73 + side]
        def M(i):
            return cps[0:6, C_SHT + 6 * i:C_SHT + 6 * i + 6]
        acc = sp.tile([6, 6], F32, tag=f"sh_{tag}")
        nc.vector.scalar_tensor_tensor(acc[:], M(1), dx, M(0), OP.mult, OP.add)
        for i, sval in [(2, dy), (3, dx2_), (4, dxy_), (5, dy2_)]:
            nc.vector.scalar_tensor_tensor(acc[:], M(i), sval, acc[:],
                                           OP.mult, OP.add)
        return acc

    Sh1T = shT(0, "1")
    Sh2T = shT(1, "2")
    # svec side1 as a [6,1] column (per-partition): c2m*s2 + c1m*s + c0m
    sv1c = sp.tile([6, 1], F32, tag="sv1c")
    tmp1 = sp.tile([6, 1], F32, tag="svt1")
    nc.vector.scalar_tensor_tensor(
        tmp1[:], cps[0:6, C_MSK:C_MSK + 1], scB[:, 66:67],
        cps[0:6, C_MSK + 2:C_MSK + 3], OP.mult, OP.add)
    nc.vector.scalar_tensor_tensor(
        sv1c[:], cps[0:6, C_MSK + 1:C_MSK + 2], scB[:, 52:53],
        tmp1[:], OP.mult, OP.add)
    # svec side2 as a [1,6] row on partition 0: [s2 s2 s s2 s 1]
    svr2 = sc[0:1, 96:102]
    s2v2 = sc[0:1, 67:68]
    sv2 = sc[0:1, 53:54]
    nc.vector.tensor_copy(
        svr2.rearrange("p (a b) -> p a b", a=3)[:, 0:2, 0:1],
        s2v2.unsqueeze(2).to_broadcast([1, 2, 1]))   # slots 0,2 = s2 (a-major)
    nc.vector.tensor_copy(svr2[:, 1:2], s2v2)        # slot 1 = s2
    nc.vector.tensor_copy(svr2[:, 3:4], s2v2)        # slot 3 = s2
    nc.vector.tensor_copy(svr2[:, 2:3], sv2)         # slot 2 = s
    nc.vector.tensor_copy(svr2[:, 4:5], sv2)         # slot 4 = s
    nc.vector.memset(svr2[:, 5:6], 1.0)
    sv2B = sp.tile([6, 6], F32, tag="sv2B")
    nc.gpsimd.partition_broadcast(sv2B[:], svr2, channels=6)

    # C2 = D1 Sh1 C' Sh2^T D2
    vps = ps.tile([6, 6], F32, tag="tps")
    nc.tensor.matmul(vps[:], Sh1T[:], Cp[:], start=True, stop=True)  # Sh1 C'
    vS = sp.tile([6, 6], F32, tag="vS")
    nc.vector.tensor_copy(vS[:], vps[:])
    vT = _transpose(nc, ps, sp, vS[:], 6, idn, "vT")
    ups = ps.tile([6, 6], F32, tag="tps")
    nc.tensor.matmul(ups[:], vT[:], Sh2T[:], start=True, stop=True)  # v Sh2^T
    # C2[r, c] = svec1[r] * u[r, c] * svec2[c]
    u1 = sp.tile([6, 6], F32, tag="u1")
    nc.vector.tensor_scalar_mul(u1[:], ups[:], sv1c[:])
    C2 = sp.tile([6, 6], F32, tag="C2")
    nc.vector.tensor_tensor(C2[:], u1[:], sv2B[:], OP.mult)
    C2T = _transpose(nc, ps, sp, C2[:], 6, idn, "c2t")

    _solve(nc, pp, sp, ps, cps, idn, sc, C2[:], C2T[:], stage, mshuf,
           out_d, T12)


def _solve(nc, pp, sp, ps, cps, idn, sc, C2, C2T, stage, mshuf, out_d, T12):
    i9h = cps[0:9, C_I9H:C_I9H + 9]
    et69 = cps[0:6, C_ET69:C_ET69 + 9]
    i3c = cps[0:3, C_I3:C_I3 + 3]
    v09 = cps[0:9, C_V09:C_V09 + 1]
    v06 = cps[0:6, C_V06:C_V06 + 1]
    sel1 = cps[0:3, C_SEL1:C_SEL1 + 6]
    sel2 = cps[0:3, C_SEL2:C_SEL2 + 6]

    # G2 = E C2 E^T : G2[3a+b, 3c+d] = C2[pair(a,b), pair(c,d)]
    z_ps = ps.tile([6, 9], F32, tag="tps")
    nc.tensor.matmul(z_ps[:], C2T, et69, start=True, stop=True)  # C2 E^T
    Zs = sp.tile([6, 9], F32, tag="Zs")
    nc.vector.tensor_copy(Zs[:], z_ps[:])
    g_ps = ps.tile([9, 9], F32, tag="tps")
    nc.tensor.matmul(g_ps[:], et69, Zs[:], start=True, stop=True)    # E @ Z
    G2 = sp.tile([9, 9], F32, tag="G2")
    nc.vector.tensor_copy(G2[:], g_ps[:])

    # Mmat[3p+q, 3r+s] = G2[3p+r, 3q+s]: bounce via DRAM
    nc.sync.dma_start(mshuf[:], G2[:])
    Mmat = sp.tile([9, 9], F32, tag="Mmat")
    for p in range(3):
        eng = nc.scalar if p == 1 else nc.sync
        eng.dma_start(
            Mmat[3 * p:3 * p + 3, :].rearrange("q (r s) -> q r s", s=3),
            mshuf[:].rearrange("(p q1 r s) -> p q1 r s", p=3, q1=3, r=3)
            .transpose([0, 2, 1, 3])[p])

    # Msp = Mmat/(2 lam) - I/2
    dg = sp.tile([9, 9], F32, tag="dg")
    nc.vector.tensor_tensor(dg[:], Mmat[:], i9h, OP.mult)
    lam2 = sp.tile([9, 1], F32, tag="lam2")
    nc.vector.tensor_reduce(lam2[:], dg[:], AX.X, OP.add)
    lam2r = sp.tile([9, 1], F32, tag="lam2r")
    nc.gpsimd.partition_all_reduce(lam2r[:], lam2[:], channels=9,
                                   reduce_op=bass_isa.ReduceOp.add)
    lam4 = sp.tile([9, 1], F32, tag="lam4")
    nc.vector.tensor_scalar_mul(lam4[:], lam2r[:], 4.0)
    inv2l = sp.tile([9, 1], F32, tag="inv2l")
    nc.vector.reciprocal(inv2l[:], lam4[:])
    Msp = sp.tile([9, 9], F32, tag="Msp")
    nc.vector.scalar_tensor_tensor(Msp[:], Mmat[:], inv2l[:], i9h,
                                   OP.mult, OP.subtract)
    M50 = _powchain(nc, ps, sp, Msp[:], 9, "m9", 5, extra=True)

    w9ps = ps.tile([1, 9], F32, tag="tps")
    nc.tensor.matmul(w9ps[:], v09, M50[:], start=True, stop=True)
    w9 = sp.tile([1, 9], F32, tag="w9")
    nc.vector.tensor_copy(w9[:], w9ps[:])
    w9sq = sp.tile([1, 9], F32, tag="w9sq")
    nc.vector.tensor_tensor(w9sq[:], w9[:], w9[:], OP.mult)
    nn9 = sp.tile([1, 1], F32, tag="nn9")
    nc.vector.tensor_reduce(nn9[:], w9sq[:], AX.X, OP.add)
    sr9 = sp.tile([1, 1], F32, tag="sr9")
    nc.scalar.activation(sr9[:], nn9[:], AF.Sqrt)
    rs9 = sp.tile([1, 1], F32, tag="rs9")
    nc.vector.reciprocal(rs9[:], sr9[:])
    v9 = sp.tile([1, 9], F32, tag="v9")
    nc.vector.tensor_tensor(v9[:], w9[:], rs9[:].to_broadcast([1, 9]), OP.mult)
    nc.sync.dma_start(stage[24:33], v9[:])

    # E = T2^T E_raw T1 (and E^T);  T1m/T2m preloaded in T12
    T1m = T12[:, 0:3]
    T2m = T12[:, 3:6]
    Eraw = sp.tile([3, 3], F32, tag="Eraw")
    nc.sync.dma_start(Eraw[:], stage[24:33].rearrange("(i j) -> i j", j=3))

    a1ps = ps.tile([3, 3], F32, tag="tps")
    nc.tensor.matmul(a1ps[:], T2m, Eraw[:], start=True, stop=True)
    A1 = sp.tile([3, 3], F32, tag="A1")
    nc.vector.tensor_copy(A1[:], a1ps[:])
    A1T = _transpose(nc, ps, sp, A1[:], 3, idn, "a1t")
    etps = ps.tile([3, 3], F32, tag="tps")
    nc.tensor.matmul(etps[:], T1m, A1T[:], start=True, stop=True)
    ETs = sp.tile([3, 3], F32, tag="ETs")
    nc.vector.tensor_copy(ETs[:], etps[:])
    Es = _transpose(nc, ps, sp, ETs[:], 3, idn, "es")

    # B = E^T E ; blockdiag 6x6 chain (32 iters) for v1 (max) and v3 (min)
    bps = ps.tile([3, 3], F32, tag="tps")
    nc.tensor.matmul(bps[:], Es[:], Es[:], start=True, stop=True)
    Bm = sp.tile([3, 3], F32, tag="Bm")
    nc.vector.tensor_copy(Bm[:], bps[:])
    dg3 = sp.tile([3, 3], F32, tag="dg3")
    nc.vector.tensor_tensor(dg3[:], Bm[:], i3c, OP.mult)
    lb = sp.tile([3, 1], F32, tag="lb")
    nc.vector.tensor_reduce(lb[:], dg3[:], AX.X, OP.add)
    lbr = sp.tile([3, 1], F32, tag="lbr")
    nc.gpsimd.partition_all_reduce(lbr[:], lb[:], channels=3,
                                   reduce_op=bass_isa.ReduceOp.add)
    invlb = sp.tile([3, 1], F32, tag="invlb")
    nc.vector.reciprocal(invlb[:], lbr[:])
    Bs3 = sp.tile([3, 3], F32, tag="Bs3")
    nc.vector.tensor_scalar_mul(Bs3[:], Bm[:], invlb[:])
    IB = sp.tile([3, 3], F32, tag="IB")
    nc.vector.tensor_tensor(IB[:], i3c, Bs3[:], OP.subtract)
    bdps = ps.tile([6, 6], F32, tag="tps")
    nc.tensor.matmul(bdps[:, 0:3], sel1, Bs3[:], start=True, stop=True)
    nc.tensor.matmul(bdps[:, 3:6], sel2, IB[:], start=True, stop=True)
    BD = sp.tile([6, 6], F32, tag="BD")
    nc.vector.tensor_copy(BD[:], bdps[:])
    BD32 = _powchain(nc, ps, sp, BD[:], 6, "m6", 5, extra=False)

    w6ps = ps.tile([1, 6], F32, tag="tps")
    nc.tensor.matmul(w6ps[:], v06, BD32[:], start=True, stop=True)
    w6 = sp.tile([1, 6], F32, tag="w6")
    nc.vector.tensor_copy(w6[:], w6ps[:])
    w6sq = sp.tile([1, 6], F32, tag="w6sq")
    nc.vector.tensor_tensor(w6sq[:], w6[:], w6[:], OP.mult)
    nn6 = sp.tile([1, 2], F32, tag="nn6")
    nc.vector.tensor_reduce(nn6[:].unsqueeze(2),
                            w6sq[:].rearrange("p (g d) -> p g d", g=2), AX.X,
                            OP.add)
    sr6 = sp.tile([1, 2], F32, tag="sr6")
    nc.scalar.activation(sr6[:], nn6[:], AF.Sqrt)
    rs6 = sp.tile([1, 2], F32, tag="rs6")
    nc.vector.reciprocal(rs6[:], sr6[:])
    vv = sp.tile([1, 6], F32, tag="vv")
    nc.vector.tensor_tensor(
        vv[:].rearrange("p (g d) -> p g d", g=2),
        w6[:].rearrange("p (g d) -> p g d", g=2),
        rs6[:].unsqueeze(2).to_broadcast([1, 2, 3]), OP.mult)

    # v2 = cross(v3, v1), normalized with EPS
    aa = sp.tile([1, 6], F32, tag="aa")
    nc.vector.tensor_copy(
        aa[:].rearrange("p (r d) -> p r d", r=2),
        vv[:, 3:6].unsqueeze(1).to_broadcast([1, 2, 3]))
    bb = sp.tile([1, 6], F32, tag="bb")
    nc.vector.tensor_copy(
        bb[:].rearrange("p (r d) -> p r d", r=2),
        vv[:, 0:3].unsqueeze(1).to_broadcast([1, 2, 3]))
    cr1 = sp.tile([1, 3], F32, tag="cr1")
    nc.vector.tensor_tensor(cr1[:], aa[:, 1:4], bb[:, 2:5], OP.mult)
    cr2 = sp.tile([1, 3], F32, tag="cr2")
    nc.vector.tensor_tensor(cr2[:], aa[:, 2:5], bb[:, 1:4], OP.mult)
    v2r = sp.tile([1, 3], F32, tag="v2r")
    nc.vector.tensor_tensor(v2r[:], cr1[:], cr2[:], OP.subtract)
    v2sq = sp.tile([1, 3], F32, tag="v2sq")
    nc.vector.tensor_tensor(v2sq[:], v2r[:], v2r[:], OP.mult)
    nn2 = sp.tile([1, 1], F32, tag="nn2")
    nc.vector.tensor_reduce(nn2[:], v2sq[:], AX.X, OP.add)
    sr2 = sp.tile([1, 1], F32, tag="sr2")
    nc.scalar.activation(sr2[:], nn2[:], AF.Sqrt)
    sr2e = sp.tile([1, 1], F32, tag="sr2e")
    nc.vector.tensor_scalar_add(sr2e[:], sr2[:], EPS)
    rs2 = sp.tile([1, 1], F32, tag="rs2")
    nc.vector.reciprocal(rs2[:], sr2e[:])
    v2 = sp.tile([1, 3], F32, tag="v2")
    nc.vector.tensor_tensor(v2[:], v2r[:], rs2[:].to_broadcast([1, 3]), OP.mult)

    vvv = sp.tile([1, 6], F32, tag="vvv")
    nc.vector.tensor_copy(vvv[:, 0:3], vv[:, 0:3])
    nc.vector.tensor_copy(vvv[:, 3:6], v2[:])
    nc.sync.dma_start(stage[33:39], vvv[:])
    Vr = sp.tile([2, 3], F32, tag="Vr")
    nc.sync.dma_start(Vr[:], stage[33:39].rearrange("(i k) -> i k", k=3))
    vcps = ps.tile([3, 2], F32, tag="tps")
    nc.tensor.transpose(vcps[:], Vr[:], cps[0:2, C_IDN:C_IDN + 2])
    Vc = sp.tile([3, 2], F32, tag="Vc")
    nc.vector.tensor_copy(Vc[:], vcps[:])
    evps = ps.tile([2, 3], F32, tag="tps")
    nc.tensor.matmul(evps[:], Vc[:], ETs[:], start=True, stop=True)
    Evr = sp.tile([2, 3], F32, tag="Evr")
    nc.vector.tensor_copy(Evr[:], evps[:])
    evsq = sp.tile([2, 3], F32, tag="evsq")
    nc.vector.tensor_tensor(evsq[:], Evr[:], Evr[:], OP.mult)
    ss2 = sp.tile([2, 1], F32, tag="ss2")
    nc.vector.tensor_reduce(ss2[:], evsq[:], AX.X, OP.add)
    sv = sp.tile([2, 1], F32, tag="sv")
    nc.scalar.activation(sv[:], ss2[:], AF.Sqrt)
    ssum = sp.tile([2, 1], F32, tag="ssum")
    nc.gpsimd.partition_all_reduce(ssum[:], sv[:], channels=2,
                                   reduce_op=bass_isa.ReduceOp.add)
    savg = sp.tile([2, 1], F32, tag="savg")
    nc.vector.tensor_scalar_mul(savg[:], ssum[:], 0.5)
    sve = sp.tile([2, 1], F32, tag="sve")
    nc.vector.tensor_scalar_add(sve[:], sv[:], EPS)
    rsv = sp.tile([2, 1], F32, tag="rsv")
    nc.vector.reciprocal(rsv[:], sve[:])
    f2 = sp.tile([2, 1], F32, tag="f2")
    nc.vector.tensor_tensor(f2[:], rsv[:], savg[:], OP.mult)
    U2 = sp.tile([2, 3], F32, tag="U2")
    nc.vector.tensor_scalar_mul(U2[:], Evr[:], f2[:])
    ops_ = ps.tile([3, 3], F32, tag="tps")
    nc.tensor.matmul(ops_[:], U2[:], Vr[:], start=True, stop=True)
    outs = sp.tile([3, 3], F32, tag="outs")
    nc.vector.tensor_copy(outs[:], ops_[:])
    nc.sync.dma_start(out_d[:], outs[:])


def make_in_maps(P, K):
    P = np.asarray(P, np.float32)
    K = np.asarray(K, np.float32)
    Pc = np.ascontiguousarray(P[:N, :N])
    PcT = np.ascontiguousarray(Pc.T)
    Mp, cpack, c0x, c0y = host_constants(K)
    m1full = _tile128(Mp, CB)
    c0t = np.array([[c0x, c0x, c0y, c0y]], np.float32)
    in_maps = []
    for k in range(NCORES):
        in_maps.append({
            "xn": _tile128(Pc[k * SH:(k + 1) * SH], RT),
            "xc": _tile128(PcT[k * SH:(k + 1) * SH], RT),
            "m1f": m1full,
            "m2s": _tile128(Mp[k * SH:(k + 1) * SH], RT),
            "cpack": cpack,
            "c0t": c0t,
        })
    return in_maps


_NC_CACHE = {}


def kernel(P, K):
    from concourse.bass_utils import run_bass_kernel_spmd
    if "nc" not in _NC_CACHE:
        _NC_CACHE["nc"] = build_nc()
    nc = _NC_CACHE["nc"]
    in_maps = make_in_maps(P, K)
    res = run_bass_kernel_spmd(nc, in_maps, core_ids=list(range(NCORES)))
    return np.asarray(res.results[0]["out"], np.float32)


# revision 21
# speedup vs baseline: 1.4007x; 1.0484x over previous
"""Trainium2 Bass kernel for nn_EssentialMatrixEstimator (v2).

Distribution (8 cores):
  - XN: natural row-shard  (384 rows x 3072 cols) -> exact row top-3 thresholds.
  - XC: transposed col-shard (384 cols x 3072 rows as [col, row]) -> exact col
    top-3 thresholds + dense masking + col-sharded gram.
  - coll1: AllGather of per-core row thresholds (384 f32 -> 3072).
  - coll2: AllReduce of the 6x6 gram C' on PRE-CENTERED monomials.

Math: the (N*M,9) epipolar Gram collapses to the 6x6 monomial Gram C'.
Monomials are pre-centered about the host constant c0 (grid centroid), so C'
is well-conditioned; the Hartley normalization is recovered from C' moments
(row/col 5) and applied as a 6x6 L-transform C2 = L1 C' L2^T instead of a
second gram pass.  Mmat (9x9) is an index expansion of C2; min-eigvector via
50-step shifted power iteration (rescaled repeated squaring), projection via
a 32-step 6x6 blockdiag chain (insensitive; validated 2.9e-4).

The big T = M2'^T W^T contraction streams in float32r (1 cy/row); validated
tolerant to tf32/bf16-level rounding (5e-4 / 3.9e-3 final rel err).
"""

import os

os.environ.setdefault("JAX_PLATFORMS", "axon")

import numpy as np

import concourse.bass as bass
import concourse.bass_isa as bass_isa
import concourse.mybir as mybir
import concourse.bacc as bacc
import concourse.tile as tile

NCORES = 8
N = 3072
SH = N // NCORES          # 384 rows/cols per core
RT = SH // 128            # 3 tiles per core shard
CB = N // 128             # 24 tiles across the full dim
F32 = mybir.dt.float32
F32R = mybir.dt.float32r
AF = mybir.ActivationFunctionType
OP = mybir.AluOpType
AX = mybir.AxisListType

EPS = 1e-8
SQRT2 = 1.4142135623730951
INV_SQRT3 = 1.0 / 1.7320508075688772
T0 = float(np.nextafter(np.float32(0.01), np.float32(1)))  # x > 0.01 == x >= T0
H, W = 64, 64

# cpack const layout (tensor [9, C_TOT]): column ranges
C_I9H = 0      # I9 * 0.5            [9, 9]
C_ET69 = 9     # E^T selector        [6, 9]
C_I3 = 18      # I3                  [3, 3]
C_V09 = 21     # full(1/3)           [9, 1]
C_V06 = 22     # full(1/sqrt3)       [6, 1]
C_SEL1 = 23    # [I3 | 0]            [3, 6]
C_SEL2 = 29    # [0 | I3]            [3, 6]
C_SHT = 35     # Sh component mats^T: I6, E1^T..E5^T   [6, 6*6]
C_MSK = 71     # svec masks [c2m c1m c0m]  [6, 3]
C_IDN = 74     # identity 9x9        [9, 9]
C_ONE = 83     # all-ones            [9, 9]
C_TOT = 92

PAIRS = [(0, 0), (0, 1), (0, 2), (1, 1), (1, 2), (2, 2)]


def _pidx():
    d = {}
    for i, (a, b) in enumerate(PAIRS):
        d[(a, b)] = i
        d[(b, a)] = i
    return d


def grid_pts(K):
    idx = np.arange(H * W, dtype=np.float32)
    pix = np.stack([idx % np.float32(W), np.floor(idx / np.float32(W))], -1)
    K_inv = np.linalg.inv(np.asarray(K, np.float32)).astype(np.float32)
    p1h = np.concatenate([pix[:N], np.ones((N, 1), np.float32)], -1)
    pts = (p1h @ K_inv.T)[:, :2].astype(np.float32)
    return pts


def host_constants(K):
    """Pre-centered monomials + packed tail constants (f32)."""
    pts = grid_pts(K)
    x, y = pts[:, 0], pts[:, 1]
    c0x = np.float32(x.mean())
    c0y = np.float32(y.mean())
    xs = (x - c0x).astype(np.float32)
    ys = (y - c0y).astype(np.float32)
    Mp = np.stack([xs * xs, xs * ys, xs, ys * ys, ys, np.ones_like(xs)],
                  -1).astype(np.float32)

    cpack = np.zeros((9, C_TOT), np.float32)
    cpack[:9, C_I9H:C_I9H + 9] = 0.5 * np.eye(9, dtype=np.float32)
    pid = _pidx()
    for a in range(3):
        for b in range(3):
            cpack[pid[(a, b)], C_ET69 + 3 * a + b] = 1.0
    cpack[:3, C_I3:C_I3 + 3] = np.eye(3, dtype=np.float32)
    cpack[:9, C_V09] = 1.0 / 3.0
    cpack[:6, C_V06] = INV_SQRT3
    cpack[:3, C_SEL1:C_SEL1 + 3] = np.eye(3, dtype=np.float32)
    cpack[:3, C_SEL2 + 3:C_SEL2 + 6] = np.eye(3, dtype=np.float32)

    # Sh(dx,dy) = I + dx*E1 + dy*E2 + dx^2*E3 + dx*dy*E4 + dy^2*E5
    # (rows of L before the diag scale; see proto.Lmat)
    E1 = np.zeros((6, 6), np.float32)  # dx terms
    E1[0, 2] = -2.0
    E1[1, 4] = -1.0
    E1[2, 5] = -1.0
    E2 = np.zeros((6, 6), np.float32)  # dy terms
    E2[1, 2] = -1.0
    E2[3, 4] = -2.0
    E2[4, 5] = -1.0
    E3 = np.zeros((6, 6), np.float32)  # dx^2
    E3[0, 5] = 1.0
    E4 = np.zeros((6, 6), np.float32)  # dx*dy
    E4[1, 5] = 1.0
    E5 = np.zeros((6, 6), np.float32)  # dy^2
    E5[3, 5] = 1.0
    mats = [np.eye(6, dtype=np.float32), E1, E2, E3, E4, E5]
    for i, Em in enumerate(mats):
        cpack[:6, C_SHT + 6 * i:C_SHT + 6 * i + 6] = Em.T
    # svec masks: svec = [s2,s2,s,s2,s,1] = c2m*s2 + c1m*s + c0m
    cpack[:6, C_MSK + 0] = [1, 1, 0, 1, 0, 0]
    cpack[:6, C_MSK + 1] = [0, 0, 1, 0, 1, 0]
    cpack[:6, C_MSK + 2] = [0, 0, 0, 0, 0, 1]
    cpack[:9, C_IDN:C_IDN + 9] = np.eye(9, dtype=np.float32)
    cpack[:9, C_ONE:C_ONE + 9] = 1.0
    return Mp, cpack, float(c0x), float(c0y)


def _tile128(a, ntiles):
    """[ntiles*128, F] -> [128, ntiles*F] with [p, t*F+f] = a[t*128+p, f]."""
    F = a.shape[1]
    return np.ascontiguousarray(
        a.reshape(ntiles, 128, F).transpose(1, 0, 2).reshape(128, ntiles * F)
    )


def build_nc(repeats=1, no_coll=False, no_tail=False, use_f32r=True, dbg_c=False):
    nc = bacc.Bacc("TRN2", target_bir_lowering=False, debug=False,
                   num_devices=NCORES)

    xn = nc.dram_tensor("xn", [128, RT * N], F32, kind="ExternalInput")
    xc = nc.dram_tensor("xc", [128, RT * N], F32, kind="ExternalInput")
    m1f = nc.dram_tensor("m1f", [128, CB * 6], F32, kind="ExternalInput")
    m2s = nc.dram_tensor("m2s", [128, RT * 6], F32, kind="ExternalInput")
    cpk = nc.dram_tensor("cpack", [9, C_TOT], F32, kind="ExternalInput")
    c0t = nc.dram_tensor("c0t", [1, 4], F32, kind="ExternalInput")
    out_d = nc.dram_tensor("out", [6, 6] if dbg_c else [3, 3], F32, kind="ExternalOutput")

    tr_in = nc.dram_tensor("tr_in", [1, SH], F32)
    tr_out = nc.dram_tensor("tr_out", [NCORES, SH], F32, addr_space="Shared")
    cr_in = nc.dram_tensor("cr_in", [6, 6], F32)
    cr_out = nc.dram_tensor("cr_out", [6, 6], F32, addr_space="Shared")
    tb = nc.dram_tensor("tb", [6, N], F32)        # T bounce
    stage = nc.dram_tensor("stage", [64], F32)
    mshuf = nc.dram_tensor("mshuf", [81], F32)

    groups = [list(range(NCORES))]

    with tile.TileContext(nc) as tc:
        with (
            tc.tile_pool(name="persist", bufs=1) as pp,
            tc.tile_pool(name="scratch", bufs=2) as sp,
            tc.tile_pool(name="ps_t", bufs=2, space="PSUM") as ps,
            tc.tile_pool(name="ps_T", bufs=2, space="PSUM") as psT,
            tc.tile_pool(name="ps_c", bufs=1, space="PSUM") as psc,
        ):
            for _rep in range(repeats):
                # ---------- P0: loads (XN on qSP, XC on qACT) ----------
                XN = pp.tile([128, RT * N], F32, tag="XN")
                XC = pp.tile([128, RT * N], F32, tag="XC")
                HN = N // 2
                for t in range(RT):
                    a = t * N
                    nc.sync.dma_start(XN[:, a:a + HN], xn[:, a:a + HN])
                    nc.scalar.dma_start(XN[:, a + HN:a + N],
                                        xn[:, a + HN:a + N])
                for t in range(RT):
                    a = t * N
                    nc.sync.dma_start(XC[:, a:a + HN], xc[:, a:a + HN])
                    nc.scalar.dma_start(XC[:, a + HN:a + N],
                                        xc[:, a + HN:a + N])
                m1s_s = pp.tile([128, CB * 6], F32, tag="m1f")
                nc.scalar.dma_start(m1s_s[:], m1f[:])
                m2s_s = pp.tile([128, RT * 6], F32, tag="m2s")
                nc.scalar.dma_start(m2s_s[:], m2s[:])
                cps = pp.tile([9, C_TOT], F32, tag="cpk")
                nc.scalar.dma_start(cps[:], cpk[:])
                c0s = pp.tile([1, 4], F32, tag="c0")
                nc.scalar.dma_start(c0s[:], c0t[:])
                sqwarm = sp.tile([1, 1], F32, tag="sqwarm")
                nc.scalar.activation(sqwarm[:], cps[0:1, 0:1], AF.Sqrt)

                def XNt(t):
                    return XN[:, t * N:(t + 1) * N]

                def XCt(t):
                    return XC[:, t * N:(t + 1) * N]

                # ---------- P1: row thresholds -> coll1 ----------
                r8 = pp.tile([128, RT * 8], F32, tag="r8")
                for t in range(RT):
                    nc.vector.max(out=r8[:, t * 8:t * 8 + 8], in_=XNt(t))
                trT0 = pp.tile([128, RT], F32, tag="trT0")
                nc.vector.tensor_scalar_max(
                    trT0[:],
                    r8[:].rearrange("p (t e) -> p t e", e=8)[:, :, 2], T0)
                for t in range(RT):
                    nc.sync.dma_start(tr_in[0:1, t * 128:(t + 1) * 128],
                                      trT0[:, t:t + 1])

                if no_coll:
                    nc.sync.dma_start(tr_out[0:1, :], tr_in[:])
                else:
                    nc.gpsimd.collective_compute(
                        "AllGather", OP.bypass, replica_groups=groups,
                        ins=[tr_in[:]], outs=[tr_out[:]])

                # ---------- P2: col thresholds (local, exact) ----------
                c8 = pp.tile([128, RT * 8], F32, tag="c8")
                for t in range(RT):
                    nc.vector.max(out=c8[:, t * 8:t * 8 + 8], in_=XCt(t))

                # ---------- P3: broadcast row-threshold table ----------
                trow = pp.tile([1, N], F32, tag="trow")
                nc.sync.dma_start(trow[:], tr_out[:].rearrange("k i -> (k i)"))
                trB = pp.tile([128, N], F32, tag="trB")
                MCH = 1536
                for c0_ in range(0, N, MCH):
                    nc.gpsimd.partition_broadcast(
                        trB[:, c0_:c0_ + MCH], trow[:, c0_:c0_ + MCH],
                        channels=128)

                # ---------- P4: dense mask + fp32r T-gram ----------
                # W (f32r): W = XC * [XC >= max(trB, tc_t)]
                # T[b, r] = sum_c m2'[c, b] * W^T[c, r]   (PSUM chunks [6,512])
                WDT = F32R if use_f32r else F32
                m2r = pp.tile([128, RT * 6], WDT, tag="m2r")
                nc.vector.tensor_copy(m2r[:], m2s_s[:])
                Wr = pp.tile([128, RT * N], WDT, tag="Wr")
                Wf = pp.tile([128, N], F32, tag="Wf")  # t=2 chunk via gpsimd
                Tsb = pp.tile([6, N], F32, tag="Tsb")
                TT = pp.tile([128, CB * 6], F32, tag="TT")
                i6 = cps[0:6, C_IDN:C_IDN + 6]
                for h in range(2):
                    for t in range(RT):
                        tcl = c8[:, t * 8 + 2:t * 8 + 3]
                        sl = slice(t * N + h * MCH, t * N + (h + 1) * MCH)
                        msk = pp.tile([128, MCH], F32, tag=f"msk{h}{t}")
                        nc.vector.scalar_tensor_tensor(
                            msk[:], trB[:, h * MCH:(h + 1) * MCH], tcl,
                            XC[:, sl], OP.max, OP.is_le)
                        if t == RT - 1:
                            nc.gpsimd.tensor_tensor(
                                Wf[:, h * MCH:(h + 1) * MCH], XC[:, sl],
                                msk[:], OP.mult)
                        else:
                            nc.vector.tensor_tensor(Wr[:, sl], XC[:, sl],
                                                    msk[:], OP.mult)
                    for q in range(3):
                        ch = h * 3 + q
                        Tp = psT.tile([6, 512], F32, tag="Tp")
                        for t in range(RT):
                            c0_ = t * N + h * MCH + q * 512
                            if t == RT - 1:
                                nc.tensor.matmul(
                                    Tp[:], m2s_s[:, t * 6:(t + 1) * 6],
                                    Wf[:, h * MCH + q * 512:
                                        h * MCH + q * 512 + 512],
                                    start=False, stop=True)
                            else:
                                nc.tensor.matmul(
                                    Tp[:],
                                    m2r[:, t * 6:(t + 1) * 6],
                                    Wr[:, c0_:c0_ + 512],
                                    start=(t == 0), stop=False)
                        nc.scalar.activation(Tsb[:, ch * 512:(ch + 1) * 512],
                                             Tp[:], AF.Copy)
                        # PE-transpose T chunk into TT[p, (j b)] blocks
                        for jj in range(4):
                            j = ch * 4 + jj
                            pt = ps.tile([128, 6], F32, tag="tps")
                            nc.tensor.transpose(
                                pt[:], Tsb[:, j * 128:(j + 1) * 128], i6)
                            nc.scalar.activation(TT[:, j * 6:(j + 1) * 6],
                                                 pt[:], AF.Copy)

                # C[a, b] = sum_j m1'_j^T TT_j  (two groups for overlap)
                pc0 = psc.tile([6, 6], F32, tag="pc0")
                pc1 = psc.tile([6, 6], F32, tag="pc1")
                for j in range(CB):
                    pc = pc0 if j < 12 else pc1
                    nc.tensor.matmul(pc[:], m1s_s[:, j * 6:(j + 1) * 6],
                                     TT[:, j * 6:(j + 1) * 6],
                                     start=(j % 12 == 0), stop=(j % 12 == 11))
                Cp = sp.tile([6, 6], F32, tag="Cp")
                nc.vector.tensor_copy(Cp[:], pc0[:])
                nc.vector.tensor_tensor(Cp[:], Cp[:], pc1[:], OP.add)
                nc.sync.dma_start(cr_in[:], Cp[:])

                # ---------- coll2: AllReduce 6x6 gram ----------
                if no_coll:
                    nc.sync.dma_start(cr_out[:], cr_in[:])
                else:
                    nc.gpsimd.collective_compute(
                        "AllReduce", OP.add, replica_groups=groups,
                        ins=[cr_in[:]], outs=[cr_out[:]])

                if no_tail:
                    nn = 6 if dbg_c else 3
                    dummy = sp.tile([nn, nn], F32, tag="dummy")
                    nc.sync.dma_start(dummy[:], cr_out[0:nn, 0:nn])
                    nc.sync.dma_start(out_d[:], dummy[:])
                    continue

                # ---------- tail ----------
                _tail(nc, pp, sp, ps, psc, cps, c0s, cr_out, stage, mshuf, out_d)

    nc.compile()
    return nc


def _transpose(nc, ps, sp, in_sb, n, idn, tag):
    pt = ps.tile([n, n], F32, tag="tps")
    nc.tensor.transpose(pt[:], in_sb, idn[:n, :n])
    ot = sp.tile([n, n], F32, tag=f"ot_{tag}")
    nc.vector.tensor_copy(ot[:], pt[:])
    return ot


def _powchain(nc, ps, sp, m_sb, n, tag, n_squarings=5, extra=True):
    """M^50 (extra=True: 5 squarings + M48=M32@M16 + M50=M48@M2) or M^32."""
    powers = {}
    cur = m_sb
    for i in range(1, n_squarings + 1):
        pm = ps.tile([n, n], F32, tag="tps")
        nc.tensor.matmul(pm[:], cur, cur, start=True, stop=True)
        nxt = sp.tile([n, n], F32, tag=f"pw_{tag}_{i}")
        nc.vector.tensor_scalar_mul(nxt[:], pm[:], 2.0)
        powers[2 ** i] = nxt
        cur = nxt[:]
    if not extra:
        return powers[2 ** n_squarings]
    pm = ps.tile([n, n], F32, tag="tps")
    nc.tensor.matmul(pm[:], powers[32][:], powers[16][:], start=True, stop=True)
    m48 = sp.tile([n, n], F32, tag=f"pw_{tag}_48")
    nc.vector.tensor_scalar_mul(m48[:], pm[:], 2.0)
    pm = ps.tile([n, n], F32, tag="tps")
    nc.tensor.matmul(pm[:], m48[:], powers[2][:], start=True, stop=True)
    m50 = sp.tile([n, n], F32, tag=f"pw_{tag}_50")
    nc.vector.tensor_scalar_mul(m50[:], pm[:], 2.0)
    return m50


def _tail(nc, pp, sp, ps, psc, cps, c0s, cr_out, stage, mshuf, out_d):
    """C' -> Hartley -> L-transform -> Mmat -> chains -> projection."""
    idn = cps[0:9, C_IDN:C_IDN + 9]

    Cp = sp.tile([6, 6], F32, tag="Cpr")
    nc.sync.dma_start(Cp[:], cr_out[:])
    CpT = sp.tile([6, 6], F32, tag="CprT")
    nc.scalar.dma_start(CpT[:], cr_out[:].rearrange("a b -> b a"))

    # moments [1,12]: side1 = C'[:,5], side2 = C'[5,:]
    sc = pp.tile([128, 112], F32, tag="tailsc")
    nc.scalar.dma_start(sc[0:1, 0:6],
                        cr_out[:].rearrange("a b -> b a")[5:6, :])
    nc.sync.dma_start(sc[0:1, 6:12], cr_out[5:6, :])

    def scv(a, b):
        return sc[0:1, a:b]

    def pair(k):
        return sc[0:1, 0:12].rearrange("p (g d) -> p d g", g=2)[:, k, :]

    Sxx, Sx, Syy, Sy, Sw = pair(0), pair(2), pair(3), pair(4), pair(5)
    ws = scv(12, 14); nc.vector.tensor_scalar_add(ws, Sw, EPS)
    rws = scv(14, 16); nc.vector.reciprocal(rws, ws)
    cx = scv(16, 18); nc.vector.tensor_tensor(cx, Sx, rws, OP.mult)  # = dx
    cy = scv(18, 20); nc.vector.tensor_tensor(cy, Sy, rws, OP.mult)  # = dy
    t_a = scv(20, 22); nc.vector.tensor_tensor(t_a, cx, Sx, OP.mult)
    t_b = scv(22, 24); nc.vector.tensor_tensor(t_b, cy, Sy, OP.mult)
    cdS = scv(24, 26); nc.vector.tensor_tensor(cdS, t_a, t_b, OP.add)
    u_a = scv(26, 28); nc.vector.tensor_tensor(u_a, cx, cx, OP.mult)
    u_b = scv(28, 30); nc.vector.tensor_tensor(u_b, cy, cy, OP.mult)
    c2_ = scv(30, 32); nc.vector.tensor_tensor(c2_, u_a, u_b, OP.add)
    sq_ = scv(32, 34); nc.vector.tensor_tensor(sq_, Sxx, Syy, OP.add)
    n2c = scv(34, 36); nc.vector.tensor_scalar_mul(n2c, cdS, -2.0)
    c2w = scv(36, 38); nc.vector.tensor_tensor(c2w, c2_, Sw, OP.mult)
    m_ = scv(38, 40); nc.vector.tensor_tensor(m_, sq_, n2c, OP.add)
    m2_ = scv(40, 42); nc.vector.tensor_tensor(m2_, m_, c2w, OP.add)
    md2 = scv(42, 44); nc.vector.tensor_tensor(md2, m2_, rws, OP.mult)
    md2e = scv(44, 46); nc.vector.tensor_scalar_add(md2e, md2, EPS)
    md = scv(46, 48); nc.scalar.activation(md, md2e, AF.Sqrt)
    mde = scv(48, 50); nc.vector.tensor_scalar_add(mde, md, EPS)
    rmd = scv(50, 52); nc.vector.reciprocal(rmd, mde)
    s_ = scv(52, 54); nc.vector.tensor_scalar_mul(s_, rmd, SQRT2)
    # real centroids: cr = dx + c0 ; c0s = [c0x c0x c0y c0y] paired
    cxr = scv(54, 56); nc.vector.tensor_tensor(cxr, cx, c0s[0:1, 0:2], OP.add)
    cyr = scv(56, 58); nc.vector.tensor_tensor(cyr, cy, c0s[0:1, 2:4], OP.add)
    scx = scv(58, 60); nc.vector.tensor_tensor(scx, s_, cxr, OP.mult)
    scy = scv(60, 62); nc.vector.tensor_tensor(scy, s_, cyr, OP.mult)
    nscx = scv(62, 64); nc.vector.tensor_scalar_mul(nscx, scx, -1.0)
    nscy = scv(64, 66); nc.vector.tensor_scalar_mul(nscy, scy, -1.0)
    # L scalars: s2, dx2, dxy, dy2 (paired)
    s2p = scv(66, 68); nc.vector.tensor_tensor(s2p, s_, s_, OP.mult)
    dx2 = scv(68, 70); nc.vector.tensor_tensor(dx2, cx, cx, OP.mult)
    dxy = scv(70, 72); nc.vector.tensor_tensor(dxy, cx, cy, OP.mult)
    dy2 = scv(72, 74); nc.vector.tensor_tensor(dy2, cy, cy, OP.mult)

    # T row-major 9-vectors: t1v at 76:85, t2v at 85:94
    nc.vector.memset(scv(76, 94), 0.0)
    tv = sc[0:1, 76:94]
    tv9 = tv.rearrange("p (v f) -> p v f", v=2)
    nc.vector.tensor_copy(tv9[:, :, 0:1], s_.unsqueeze(2))
    nc.vector.tensor_copy(tv9[:, :, 4:5], s_.unsqueeze(2))
    nc.vector.tensor_copy(
        tv9[:, :, 2:8].rearrange("p v (c d) -> p v c d", c=2)[:, :, :, 0:1],
        sc[0:1, 62:66].rearrange("p (c v) -> p v c", c=2).unsqueeze(3))
    nc.vector.memset(tv9[:, :, 8:9], 1.0)
    nc.sync.dma_start(stage[0:18], tv)
    T12 = sp.tile([3, 6], F32, tag="T12")
    nc.sync.dma_start(
        T12[:].rearrange("i (v j) -> i v j", v=2),
        stage[0:18].rearrange("(v i j) -> i v j", i=3, j=3))

    # broadcast scalar strip to 6 partitions for the L build (PE ones)
    ones16 = cps[0:1, C_ONE:C_ONE + 6]
    scBp = ps.tile([6, 80], F32, tag="tps")
    nc.tensor.matmul(scBp[:], ones16, sc[0:1, 0:80], start=True, stop=True)
    scB = sp.tile([6, 80], F32, tag="scB")
    nc.vector.tensor_copy(scB[:], scBp[:])

    def shT(side, tag):
        """Sh^T for side (0/1): I^T + dx E1^T + dy E2^T + dx2 E3^T + ..."""
        dx = scB[:, 16 + side:17 + side]
        dy = scB[:, 18 + side:19 + side]
        dx2_ = scB[:, 68 + side:69 + side]
        dxy_ = scB[:, 70 + side:71 + side]
        dy2_ = scB[:, 72 + side:73 + side]
        def M(i):
            return cps[0:6, C_SHT + 6 * i:C_SHT + 6 * i + 6]
        acc = sp.tile([6, 6], F32, tag=f"sh_{tag}")
        nc.vector.scalar_tensor_tensor(acc[:], M(1), dx, M(0), OP.mult, OP.add)
        for i, sval in [(2, dy), (3, dx2_), (4, dxy_), (5, dy2_)]:
            nc.vector.scalar_tensor_tensor(acc[:], M(i), sval, acc[:],
                                           OP.mult, OP.add)
        return acc

    Sh1T = shT(0, "1")
    Sh2T = shT(1, "2")
    # svec side1 as a [6,1] column (per-partition): c2m*s2 + c1m*s + c0m
    sv1c = sp.tile([6, 1], F32, tag="sv1c")
    tmp1 = sp.tile([6, 1], F32, tag="svt1")
    nc.vector.scalar_tensor_tensor(
        tmp1[:], cps[0:6, C_MSK:C_MSK + 1], scB[:, 66:67],
        cps[0:6, C_MSK + 2:C_MSK + 3], OP.mult, OP.add)
    nc.vector.scalar_tensor_tensor(
        sv1c[:], cps[0:6, C_MSK + 1:C_MSK + 2], scB[:, 52:53],
        tmp1[:], OP.mult, OP.add)
    # svec side2 as a [1,6] row on partition 0: [s2 s2 s s2 s 1]
    svr2 = sc[0:1, 96:102]
    s2v2 = sc[0:1, 67:68]
    sv2 = sc[0:1, 53:54]
    nc.vector.tensor_copy(
        svr2.rearrange("p (a b) -> p a b", a=3)[:, 0:2, 0:1],
        s2v2.unsqueeze(2).to_broadcast([1, 2, 1]))   # slots 0,2 = s2 (a-major)
    nc.vector.tensor_copy(svr2[:, 1:2], s2v2)        # slot 1 = s2
    nc.vector.tensor_copy(svr2[:, 3:4], s2v2)        # slot 3 = s2
    nc.vector.tensor_copy(svr2[:, 2:3], sv2)         # slot 2 = s
    nc.vector.tensor_copy(svr2[:, 4:5], sv2)         # slot 4 = s
    nc.vector.memset(svr2[:, 5:6], 1.0)
    sv2B = sp.tile([6, 6], F32, tag="sv2B")
    sv2Bp = ps.tile([6, 6], F32, tag="tps")
    nc.tensor.matmul(sv2Bp[:], ones16, svr2, start=True, stop=True)
    nc.vector.tensor_copy(sv2B[:], sv2Bp[:])

    # C2 = D1 Sh1 C' Sh2^T D2
    vps = ps.tile([6, 6], F32, tag="tps")
    nc.tensor.matmul(vps[:], Sh1T[:], Cp[:], start=True, stop=True)  # Sh1 C'
    vS = sp.tile([6, 6], F32, tag="vS")
    nc.vector.tensor_copy(vS[:], vps[:])
    vT = _transpose(nc, ps, sp, vS[:], 6, idn, "vT")
    ups = ps.tile([6, 6], F32, tag="tps")
    nc.tensor.matmul(ups[:], vT[:], Sh2T[:], start=True, stop=True)  # v Sh2^T
    # C2[r, c] = svec1[r] * u[r, c] * svec2[c]
    u1 = sp.tile([6, 6], F32, tag="u1")
    nc.vector.tensor_scalar_mul(u1[:], ups[:], sv1c[:])
    C2 = sp.tile([6, 6], F32, tag="C2")
    nc.vector.tensor_tensor(C2[:], u1[:], sv2B[:], OP.mult)
    C2T = _transpose(nc, ps, sp, C2[:], 6, idn, "c2t")

    _solve(nc, pp, sp, ps, psc, cps, idn, sc, C2[:], C2T[:], stage, mshuf,
           out_d, T12)


def _solve(nc, pp, sp, ps, psc, cps, idn, sc, C2, C2T, stage, mshuf, out_d,
           T12):
    i9h = cps[0:9, C_I9H:C_I9H + 9]
    et69 = cps[0:6, C_ET69:C_ET69 + 9]
    i3c = cps[0:3, C_I3:C_I3 + 3]
    v09 = cps[0:9, C_V09:C_V09 + 1]
    v06 = cps[0:6, C_V06:C_V06 + 1]
    sel1 = cps[0:3, C_SEL1:C_SEL1 + 6]
    sel2 = cps[0:3, C_SEL2:C_SEL2 + 6]

    # G2 = E C2 E^T : G2[3a+b, 3c+d] = C2[pair(a,b), pair(c,d)]
    z_ps = ps.tile([6, 9], F32, tag="tps")
    nc.tensor.matmul(z_ps[:], C2T, et69, start=True, stop=True)  # C2 E^T
    Zs = sp.tile([6, 9], F32, tag="Zs")
    nc.vector.tensor_copy(Zs[:], z_ps[:])
    g_ps = ps.tile([9, 9], F32, tag="tps")
    nc.tensor.matmul(g_ps[:], et69, Zs[:], start=True, stop=True)    # E @ Z
    G2 = sp.tile([9, 9], F32, tag="G2")
    nc.vector.tensor_copy(G2[:], g_ps[:])

    # Mmat[3p+q, 3r+s] = G2[3p+r, 3q+s]: bounce via DRAM
    nc.sync.dma_start(mshuf[:], G2[:])
    Mmat = sp.tile([9, 9], F32, tag="Mmat")
    for p in range(3):
        eng = nc.scalar if p == 1 else nc.sync
        eng.dma_start(
            Mmat[3 * p:3 * p + 3, :].rearrange("q (r s) -> q r s", s=3),
            mshuf[:].rearrange("(p q1 r s) -> p q1 r s", p=3, q1=3, r=3)
            .transpose([0, 2, 1, 3])[p])

    # Msp = Mmat/(2 lam) - I/2
    dg = sp.tile([9, 9], F32, tag="dg")
    nc.vector.tensor_tensor(dg[:], Mmat[:], i9h, OP.mult)
    lam2 = sp.tile([9, 1], F32, tag="lam2")
    nc.vector.tensor_reduce(lam2[:], dg[:], AX.X, OP.add)
    ones99 = cps[0:9, C_ONE:C_ONE + 9]
    lam2r = ps.tile([9, 1], F32, tag="tps")
    nc.tensor.matmul(lam2r[:], ones99, lam2[:], start=True, stop=True)
    lam4 = sp.tile([9, 1], F32, tag="lam4")
    nc.vector.tensor_scalar_mul(lam4[:], lam2r[:], 4.0)
    inv2l = sp.tile([9, 1], F32, tag="inv2l")
    nc.vector.reciprocal(inv2l[:], lam4[:])
    Msp = sp.tile([9, 9], F32, tag="Msp")
    nc.vector.scalar_tensor_tensor(Msp[:], Mmat[:], inv2l[:], i9h,
                                   OP.mult, OP.subtract)
    M50 = _powchain(nc, ps, sp, Msp[:], 9, "m9", 5, extra=True)

    w9ps = ps.tile([1, 9], F32, tag="tps")
    nc.tensor.matmul(w9ps[:], v09, M50[:], start=True, stop=True)
    w9 = sp.tile([1, 9], F32, tag="w9")
    nc.vector.tensor_copy(w9[:], w9ps[:])
    nc.sync.dma_start(stage[24:33], w9[:])  # raw; 1/||w9|| folded at the end
    w9sq = sp.tile([1, 9], F32, tag="w9sq")
    nc.vector.tensor_tensor(w9sq[:], w9[:], w9[:], OP.mult)
    nn9 = sp.tile([1, 1], F32, tag="nn9")
    nc.vector.tensor_reduce(nn9[:], w9sq[:], AX.X, OP.add)
    sr9 = sp.tile([1, 1], F32, tag="sr9")
    nc.scalar.activation(sr9[:], nn9[:], AF.Sqrt)
    rs9 = sp.tile([1, 1], F32, tag="rs9")
    nc.vector.reciprocal(rs9[:], sr9[:])
    rs9c = psc.tile([3, 1], F32, tag="rs9c")
    nc.tensor.matmul(rs9c[:], cps[0:1, C_ONE:C_ONE + 3], rs9[:],
                     start=True, stop=True)

    # E = T2^T E_raw T1 (and E^T);  T1m/T2m preloaded in T12
    T1m = T12[:, 0:3]
    T2m = T12[:, 3:6]
    Eraw = sp.tile([3, 3], F32, tag="Eraw")
    nc.sync.dma_start(Eraw[:], stage[24:33].rearrange("(i j) -> i j", j=3))

    a1ps = ps.tile([3, 3], F32, tag="tps")
    nc.tensor.matmul(a1ps[:], T2m, Eraw[:], start=True, stop=True)
    A1 = sp.tile([3, 3], F32, tag="A1")
    nc.vector.tensor_copy(A1[:], a1ps[:])
    A1T = _transpose(nc, ps, sp, A1[:], 3, idn, "a1t")
    etps = ps.tile([3, 3], F32, tag="tps")
    nc.tensor.matmul(etps[:], T1m, A1T[:], start=True, stop=True)
    ETs = sp.tile([3, 3], F32, tag="ETs")
    nc.vector.tensor_copy(ETs[:], etps[:])
    Es = _transpose(nc, ps, sp, ETs[:], 3, idn, "es")

    # B = E^T E ; blockdiag 6x6 chain (32 iters) for v1 (max) and v3 (min)
    bps = ps.tile([3, 3], F32, tag="tps")
    nc.tensor.matmul(bps[:], Es[:], Es[:], start=True, stop=True)
    Bm = sp.tile([3, 3], F32, tag="Bm")
    nc.vector.tensor_copy(Bm[:], bps[:])
    dg3 = sp.tile([3, 3], F32, tag="dg3")
    nc.vector.tensor_tensor(dg3[:], Bm[:], i3c, OP.mult)
    lb = sp.tile([3, 1], F32, tag="lb")
    nc.vector.tensor_reduce(lb[:], dg3[:], AX.X, OP.add)
    lbr = ps.tile([3, 1], F32, tag="tps")
    nc.tensor.matmul(lbr[:], cps[0:3, C_ONE:C_ONE + 3], lb[:],
                     start=True, stop=True)
    invlb = sp.tile([3, 1], F32, tag="invlb")
    nc.vector.reciprocal(invlb[:], lbr[:])
    Bs3 = sp.tile([3, 3], F32, tag="Bs3")
    nc.vector.tensor_scalar_mul(Bs3[:], Bm[:], invlb[:])
    IB = sp.tile([3, 3], F32, tag="IB")
    nc.vector.tensor_tensor(IB[:], i3c, Bs3[:], OP.subtract)
    bdps = ps.tile([6, 6], F32, tag="tps")
    nc.tensor.matmul(bdps[:, 0:3], sel1, Bs3[:], start=True, stop=True)
    nc.tensor.matmul(bdps[:, 3:6], sel2, IB[:], start=True, stop=True)
    BD = sp.tile([6, 6], F32, tag="BD")
    nc.vector.tensor_copy(BD[:], bdps[:])
    BD32 = _powchain(nc, ps, sp, BD[:], 6, "m6", 5, extra=False)

    w6ps = ps.tile([1, 6], F32, tag="tps")
    nc.tensor.matmul(w6ps[:], v06, BD32[:], start=True, stop=True)
    w6 = sp.tile([1, 6], F32, tag="w6")
    nc.vector.tensor_copy(w6[:], w6ps[:])
    w6sq = sp.tile([1, 6], F32, tag="w6sq")
    nc.vector.tensor_tensor(w6sq[:], w6[:], w6[:], OP.mult)
    nn6 = sp.tile([1, 2], F32, tag="nn6")
    nc.vector.tensor_reduce(nn6[:].unsqueeze(2),
                            w6sq[:].rearrange("p (g d) -> p g d", g=2), AX.X,
                            OP.add)
    sr6 = sp.tile([1, 2], F32, tag="sr6")
    nc.scalar.activation(sr6[:], nn6[:], AF.Sqrt)
    rs6 = sp.tile([1, 2], F32, tag="rs6")
    nc.vector.reciprocal(rs6[:], sr6[:])
    vv = sp.tile([1, 6], F32, tag="vv")
    nc.vector.tensor_tensor(
        vv[:].rearrange("p (g d) -> p g d", g=2),
        w6[:].rearrange("p (g d) -> p g d", g=2),
        rs6[:].unsqueeze(2).to_broadcast([1, 2, 3]), OP.mult)

    # v2 = cross(v3, v1), normalized with EPS
    aa = sp.tile([1, 6], F32, tag="aa")
    nc.vector.tensor_copy(
        aa[:].rearrange("p (r d) -> p r d", r=2),
        vv[:, 3:6].unsqueeze(1).to_broadcast([1, 2, 3]))
    bb = sp.tile([1, 6], F32, tag="bb")
    nc.vector.tensor_copy(
        bb[:].rearrange("p (r d) -> p r d", r=2),
        vv[:, 0:3].unsqueeze(1).to_broadcast([1, 2, 3]))
    cr1 = sp.tile([1, 3], F32, tag="cr1")
    nc.vector.tensor_tensor(cr1[:], aa[:, 1:4], bb[:, 2:5], OP.mult)
    cr2 = sp.tile([1, 3], F32, tag="cr2")
    nc.vector.tensor_tensor(cr2[:], aa[:, 2:5], bb[:, 1:4], OP.mult)
    v2r = sp.tile([1, 3], F32, tag="v2r")
    nc.vector.tensor_tensor(v2r[:], cr1[:], cr2[:], OP.subtract)
    v2sq = sp.tile([1, 3], F32, tag="v2sq")
    nc.vector.tensor_tensor(v2sq[:], v2r[:], v2r[:], OP.mult)
    nn2 = sp.tile([1, 1], F32, tag="nn2")
    nc.vector.tensor_reduce(nn2[:], v2sq[:], AX.X, OP.add)
    sr2 = sp.tile([1, 1], F32, tag="sr2")
    nc.scalar.activation(sr2[:], nn2[:], AF.Sqrt)
    sr2e = sp.tile([1, 1], F32, tag="sr2e")
    nc.vector.tensor_scalar_add(sr2e[:], sr2[:], EPS)
    rs2 = sp.tile([1, 1], F32, tag="rs2")
    nc.vector.reciprocal(rs2[:], sr2e[:])
    v2 = sp.tile([1, 3], F32, tag="v2")
    nc.vector.tensor_tensor(v2[:], v2r[:], rs2[:].to_broadcast([1, 3]), OP.mult)

    vvv = sp.tile([1, 6], F32, tag="vvv")
    nc.vector.tensor_copy(vvv[:, 0:3], vv[:, 0:3])
    nc.vector.tensor_copy(vvv[:, 3:6], v2[:])
    nc.sync.dma_start(stage[33:39], vvv[:])
    Vr = sp.tile([2, 3], F32, tag="Vr")
    nc.sync.dma_start(Vr[:], stage[33:39].rearrange("(i k) -> i k", k=3))
    Vc = sp.tile([3, 2], F32, tag="Vc")
    nc.scalar.dma_start(Vc[:], stage[33:39].rearrange("(i k) -> k i", k=3))
    evps = ps.tile([2, 3], F32, tag="tps")
    nc.tensor.matmul(evps[:], Vc[:], ETs[:], start=True, stop=True)
    Evr = sp.tile([2, 3], F32, tag="Evr")
    nc.vector.tensor_copy(Evr[:], evps[:])
    evsq = sp.tile([2, 3], F32, tag="evsq")
    nc.vector.tensor_tensor(evsq[:], Evr[:], Evr[:], OP.mult)
    ss2 = sp.tile([2, 1], F32, tag="ss2")
    nc.vector.tensor_reduce(ss2[:], evsq[:], AX.X, OP.add)
    sv = sp.tile([2, 1], F32, tag="sv")
    nc.scalar.activation(sv[:], ss2[:], AF.Sqrt)
    ssum = ps.tile([2, 1], F32, tag="tps")
    nc.tensor.matmul(ssum[:], cps[0:2, C_ONE:C_ONE + 2], sv[:],
                     start=True, stop=True)
    savg = sp.tile([2, 1], F32, tag="savg")
    nc.vector.tensor_scalar_mul(savg[:], ssum[:], 0.5)
    sve = sp.tile([2, 1], F32, tag="sve")
    nc.vector.tensor_scalar_add(sve[:], sv[:], EPS)
    rsv = sp.tile([2, 1], F32, tag="rsv")
    nc.vector.reciprocal(rsv[:], sve[:])
    f2 = sp.tile([2, 1], F32, tag="f2")
    nc.vector.tensor_tensor(f2[:], rsv[:], savg[:], OP.mult)
    U2 = sp.tile([2, 3], F32, tag="U2")
    nc.vector.tensor_scalar_mul(U2[:], Evr[:], f2[:])
    ops_ = ps.tile([3, 3], F32, tag="tps")
    nc.tensor.matmul(ops_[:], U2[:], Vr[:], start=True, stop=True)
    outs = sp.tile([3, 3], F32, tag="outs")
    nc.vector.tensor_scalar_mul(outs[:], ops_[:], rs9c[:])
    nc.sync.dma_start(out_d[:], outs[:])


def make_in_maps(P, K):
    P = np.asarray(P, np.float32)
    K = np.asarray(K, np.float32)
    Pc = np.ascontiguousarray(P[:N, :N])
    PcT = np.ascontiguousarray(Pc.T)
    Mp, cpack, c0x, c0y = host_constants(K)
    m1full = _tile128(Mp, CB)
    c0t = np.array([[c0x, c0x, c0y, c0y]], np.float32)
    in_maps = []
    for k in range(NCORES):
        in_maps.append({
            "xn": _tile128(Pc[k * SH:(k + 1) * SH], RT),
            "xc": _tile128(PcT[k * SH:(k + 1) * SH], RT),
            "m1f": m1full,
            "m2s": _tile128(Mp[k * SH:(k + 1) * SH], RT),
            "cpack": cpack,
            "c0t": c0t,
        })
    return in_maps


_NC_CACHE = {}


def kernel(P, K):
    from concourse.bass_utils import run_bass_kernel_spmd
    if "nc" not in _NC_CACHE:
        _NC_CACHE["nc"] = build_nc()
    nc = _NC_CACHE["nc"]
    in_maps = make_in_maps(P, K)
    res = run_bass_kernel_spmd(nc, in_maps, core_ids=list(range(NCORES)))
    return np.asarray(res.results[0]["out"], np.float32)


# revision 22
# speedup vs baseline: 1.4683x; 1.0482x over previous
"""Trainium2 Bass kernel for nn_EssentialMatrixEstimator (v2).

Distribution (8 cores):
  - XN: natural row-shard  (384 rows x 3072 cols) -> exact row top-3 thresholds.
  - XC: transposed col-shard (384 cols x 3072 rows as [col, row]) -> exact col
    top-3 thresholds + dense masking + col-sharded gram.
  - coll1: AllGather of per-core row thresholds (384 f32 -> 3072).
  - coll2: AllReduce of the 6x6 gram C' on PRE-CENTERED monomials.

Math: the (N*M,9) epipolar Gram collapses to the 6x6 monomial Gram C'.
Monomials are pre-centered about the host constant c0 (grid centroid), so C'
is well-conditioned; the Hartley normalization is recovered from C' moments
(row/col 5) and applied as a 6x6 L-transform C2 = L1 C' L2^T instead of a
second gram pass.  Mmat (9x9) is an index expansion of C2; min-eigvector via
50-step shifted power iteration (rescaled repeated squaring), projection via
a 32-step 6x6 blockdiag chain (insensitive; validated 2.9e-4).

The big T = M2'^T W^T contraction streams in float32r (1 cy/row); validated
tolerant to tf32/bf16-level rounding (5e-4 / 3.9e-3 final rel err).
"""

import os

os.environ.setdefault("JAX_PLATFORMS", "axon")

import numpy as np

import concourse.bass as bass
import concourse.bass_isa as bass_isa
import concourse.mybir as mybir
import concourse.bacc as bacc
import concourse.tile as tile

NCORES = 8
N = 3072
SH = N // NCORES          # 384 rows/cols per core
RT = SH // 128            # 3 tiles per core shard
CB = N // 128             # 24 tiles across the full dim
F32 = mybir.dt.float32
F32R = mybir.dt.float32r
AF = mybir.ActivationFunctionType
OP = mybir.AluOpType
AX = mybir.AxisListType

EPS = 1e-8
SQRT2 = 1.4142135623730951
INV_SQRT3 = 1.0 / 1.7320508075688772
T0 = float(np.nextafter(np.float32(0.01), np.float32(1)))  # x > 0.01 == x >= T0
H, W = 64, 64

# cpack const layout (tensor [9, C_TOT]): column ranges
C_I9H = 0      # I9 * 0.5            [9, 9]
C_ET69 = 9     # E^T selector        [6, 9]
C_I3 = 18      # I3                  [3, 3]
C_V09 = 21     # full(1/3)           [9, 1]
C_V06 = 22     # full(1/sqrt3)       [6, 1]
C_SEL1 = 23    # [I3 | 0]            [3, 6]
C_SEL2 = 29    # [0 | I3]            [3, 6]
C_SHT = 35     # Sh component mats^T: I6, E1^T..E5^T   [6, 6*6]
C_MSK = 71     # svec masks [c2m c1m c0m]  [6, 3]
C_IDN = 74     # identity 9x9        [9, 9]
C_ONE = 83     # all-ones            [9, 9]
C_TOT = 92

PAIRS = [(0, 0), (0, 1), (0, 2), (1, 1), (1, 2), (2, 2)]


def _pidx():
    d = {}
    for i, (a, b) in enumerate(PAIRS):
        d[(a, b)] = i
        d[(b, a)] = i
    return d


def grid_pts(K):
    idx = np.arange(H * W, dtype=np.float32)
    pix = np.stack([idx % np.float32(W), np.floor(idx / np.float32(W))], -1)
    K_inv = np.linalg.inv(np.asarray(K, np.float32)).astype(np.float32)
    p1h = np.concatenate([pix[:N], np.ones((N, 1), np.float32)], -1)
    pts = (p1h @ K_inv.T)[:, :2].astype(np.float32)
    return pts


def host_constants(K):
    """Pre-centered monomials + packed tail constants (f32)."""
    pts = grid_pts(K)
    x, y = pts[:, 0], pts[:, 1]
    c0x = np.float32(x.mean())
    c0y = np.float32(y.mean())
    xs = (x - c0x).astype(np.float32)
    ys = (y - c0y).astype(np.float32)
    Mp = np.stack([xs * xs, xs * ys, xs, ys * ys, ys, np.ones_like(xs)],
                  -1).astype(np.float32)

    cpack = np.zeros((9, C_TOT), np.float32)
    cpack[:9, C_I9H:C_I9H + 9] = 0.5 * np.eye(9, dtype=np.float32)
    pid = _pidx()
    for a in range(3):
        for b in range(3):
            cpack[pid[(a, b)], C_ET69 + 3 * a + b] = 1.0
    cpack[:3, C_I3:C_I3 + 3] = np.eye(3, dtype=np.float32)
    cpack[:9, C_V09] = 1.0 / 3.0
    cpack[:6, C_V06] = INV_SQRT3
    cpack[:3, C_SEL1:C_SEL1 + 3] = np.eye(3, dtype=np.float32)
    cpack[:3, C_SEL2 + 3:C_SEL2 + 6] = np.eye(3, dtype=np.float32)

    # Sh(dx,dy) = I + dx*E1 + dy*E2 + dx^2*E3 + dx*dy*E4 + dy^2*E5
    # (rows of L before the diag scale; see proto.Lmat)
    E1 = np.zeros((6, 6), np.float32)  # dx terms
    E1[0, 2] = -2.0
    E1[1, 4] = -1.0
    E1[2, 5] = -1.0
    E2 = np.zeros((6, 6), np.float32)  # dy terms
    E2[1, 2] = -1.0
    E2[3, 4] = -2.0
    E2[4, 5] = -1.0
    E3 = np.zeros((6, 6), np.float32)  # dx^2
    E3[0, 5] = 1.0
    E4 = np.zeros((6, 6), np.float32)  # dx*dy
    E4[1, 5] = 1.0
    E5 = np.zeros((6, 6), np.float32)  # dy^2
    E5[3, 5] = 1.0
    mats = [np.eye(6, dtype=np.float32), E1, E2, E3, E4, E5]
    for i, Em in enumerate(mats):
        cpack[:6, C_SHT + 6 * i:C_SHT + 6 * i + 6] = Em.T
    # svec masks: svec = [s2,s2,s,s2,s,1] = c2m*s2 + c1m*s + c0m
    cpack[:6, C_MSK + 0] = [1, 1, 0, 1, 0, 0]
    cpack[:6, C_MSK + 1] = [0, 0, 1, 0, 1, 0]
    cpack[:6, C_MSK + 2] = [0, 0, 0, 0, 0, 1]
    cpack[:9, C_IDN:C_IDN + 9] = np.eye(9, dtype=np.float32)
    cpack[:9, C_ONE:C_ONE + 9] = 1.0
    return Mp, cpack, float(c0x), float(c0y)


def _tile128(a, ntiles):
    """[ntiles*128, F] -> [128, ntiles*F] with [p, t*F+f] = a[t*128+p, f]."""
    F = a.shape[1]
    return np.ascontiguousarray(
        a.reshape(ntiles, 128, F).transpose(1, 0, 2).reshape(128, ntiles * F)
    )


def build_nc(repeats=1, no_coll=False, no_tail=False, use_f32r=True, dbg_c=False):
    nc = bacc.Bacc("TRN2", target_bir_lowering=False, debug=False,
                   num_devices=NCORES)

    xn = nc.dram_tensor("xn", [128, RT * N], F32, kind="ExternalInput")
    xc = nc.dram_tensor("xc", [128, RT * N], F32, kind="ExternalInput")
    m1f = nc.dram_tensor("m1f", [128, CB * 6], F32, kind="ExternalInput")
    m2s = nc.dram_tensor("m2s", [128, RT * 6], F32, kind="ExternalInput")
    cpk = nc.dram_tensor("cpack", [9, C_TOT], F32, kind="ExternalInput")
    c0t = nc.dram_tensor("c0t", [1, 4], F32, kind="ExternalInput")
    out_d = nc.dram_tensor("out", [6, 6] if dbg_c else [3, 3], F32, kind="ExternalOutput")

    tr_in = nc.dram_tensor("tr_in", [1, SH], F32)
    tr_out = nc.dram_tensor("tr_out", [NCORES, SH], F32, addr_space="Shared")
    cr_in = nc.dram_tensor("cr_in", [6, 6], F32)
    cr_out = nc.dram_tensor("cr_out", [6, 6], F32, addr_space="Shared")
    tb = nc.dram_tensor("tb", [6, N], F32)        # T bounce
    stage = nc.dram_tensor("stage", [64], F32)
    mshuf = nc.dram_tensor("mshuf", [81], F32)

    groups = [list(range(NCORES))]

    with tile.TileContext(nc) as tc:
        with (
            tc.tile_pool(name="persist", bufs=1) as pp,
            tc.tile_pool(name="scratch", bufs=2) as sp,
            tc.tile_pool(name="ps_t", bufs=2, space="PSUM") as ps,
            tc.tile_pool(name="ps_T", bufs=2, space="PSUM") as psT,
            tc.tile_pool(name="ps_c", bufs=1, space="PSUM") as psc,
        ):
            for _rep in range(repeats):
                # ---------- P0: loads (XN on qSP, XC on qACT) ----------
                XN = pp.tile([128, RT * N], F32, tag="XN")
                XC = pp.tile([128, RT * N], F32, tag="XC")
                HN = N // 2
                for t in range(RT):
                    a = t * N
                    nc.sync.dma_start(XN[:, a:a + HN], xn[:, a:a + HN])
                    nc.scalar.dma_start(XN[:, a + HN:a + N],
                                        xn[:, a + HN:a + N])
                for t in range(RT):
                    a = t * N
                    nc.sync.dma_start(XC[:, a:a + HN], xc[:, a:a + HN])
                    nc.scalar.dma_start(XC[:, a + HN:a + N],
                                        xc[:, a + HN:a + N])
                m1s_s = pp.tile([128, CB * 6], F32, tag="m1f")
                nc.scalar.dma_start(m1s_s[:], m1f[:])
                m2s_s = pp.tile([128, RT * 6], F32, tag="m2s")
                nc.scalar.dma_start(m2s_s[:], m2s[:])
                cps = pp.tile([9, C_TOT], F32, tag="cpk")
                nc.scalar.dma_start(cps[:], cpk[:])
                c0s = pp.tile([1, 4], F32, tag="c0")
                nc.scalar.dma_start(c0s[:], c0t[:])
                sqwarm = sp.tile([1, 1], F32, tag="sqwarm")
                nc.scalar.activation(sqwarm[:], cps[0:1, 0:1], AF.Sqrt)

                def XNt(t):
                    return XN[:, t * N:(t + 1) * N]

                def XCt(t):
                    return XC[:, t * N:(t + 1) * N]

                # ---------- P1: row thresholds -> coll1 ----------
                r8 = pp.tile([128, RT * 8], F32, tag="r8")
                for t in range(RT):
                    nc.vector.max(out=r8[:, t * 8:t * 8 + 8], in_=XNt(t))
                trT0 = pp.tile([128, RT], F32, tag="trT0")
                nc.vector.tensor_scalar_max(
                    trT0[:],
                    r8[:].rearrange("p (t e) -> p t e", e=8)[:, :, 2], T0)
                for t in range(RT):
                    nc.sync.dma_start(tr_in[0:1, t * 128:(t + 1) * 128],
                                      trT0[:, t:t + 1])

                if no_coll:
                    nc.sync.dma_start(tr_out[0:1, :], tr_in[:])
                else:
                    nc.gpsimd.collective_compute(
                        "AllGather", OP.bypass, replica_groups=groups,
                        ins=[tr_in[:]], outs=[tr_out[:]])

                # ---------- P2: col thresholds (local, exact) ----------
                c8 = pp.tile([128, RT * 8], F32, tag="c8")
                for t in range(RT):
                    nc.vector.max(out=c8[:, t * 8:t * 8 + 8], in_=XCt(t))

                # ---------- P3: broadcast row-threshold table ----------
                trow = pp.tile([1, N], F32, tag="trow")
                nc.sync.dma_start(trow[:], tr_out[:].rearrange("k i -> (k i)"))
                trB = pp.tile([128, N], F32, tag="trB")
                MCH = 1536
                for c0_ in range(0, N, MCH):
                    nc.gpsimd.partition_broadcast(
                        trB[:, c0_:c0_ + MCH], trow[:, c0_:c0_ + MCH],
                        channels=128)

                # ---------- P4: dense mask + fp32r T-gram ----------
                # W (f32r): W = XC * [XC >= max(trB, tc_t)]
                # T[b, r] = sum_c m2'[c, b] * W^T[c, r]   (PSUM chunks [6,512])
                WDT = F32R if use_f32r else F32
                m2r = pp.tile([128, RT * 6], WDT, tag="m2r")
                nc.vector.tensor_copy(m2r[:], m2s_s[:])
                Wr = pp.tile([128, RT * N], WDT, tag="Wr")
                Wf = pp.tile([128, N], F32, tag="Wf")  # t=2 chunk via gpsimd
                Tsb = pp.tile([6, N], F32, tag="Tsb")
                TT = pp.tile([128, CB * 6], F32, tag="TT")
                i6 = cps[0:6, C_IDN:C_IDN + 6]
                for h in range(2):
                    for t in range(RT):
                        tcl = c8[:, t * 8 + 2:t * 8 + 3]
                        sl = slice(t * N + h * MCH, t * N + (h + 1) * MCH)
                        msk = pp.tile([128, MCH], F32, tag=f"msk{h}{t}")
                        nc.vector.scalar_tensor_tensor(
                            msk[:], trB[:, h * MCH:(h + 1) * MCH], tcl,
                            XC[:, sl], OP.max, OP.is_le)
                        if t == 0:
                            nc.gpsimd.tensor_tensor(
                                Wf[:, h * MCH:(h + 1) * MCH], XC[:, sl],
                                msk[:], OP.mult)
                        else:
                            nc.vector.tensor_tensor(Wr[:, sl], XC[:, sl],
                                                    msk[:], OP.mult)
                    for q in range(3):
                        ch = h * 3 + q
                        Tp = psT.tile([6, 512], F32, tag="Tp")
                        for t in range(RT):
                            c0_ = t * N + h * MCH + q * 512
                            if t == 0:
                                nc.tensor.matmul(
                                    Tp[:], m2s_s[:, t * 6:(t + 1) * 6],
                                    Wf[:, h * MCH + q * 512:
                                        h * MCH + q * 512 + 512],
                                    start=True, stop=False)
                            else:
                                nc.tensor.matmul(
                                    Tp[:],
                                    m2r[:, t * 6:(t + 1) * 6],
                                    Wr[:, c0_:c0_ + 512],
                                    start=False, stop=(t == RT - 1))
                        nc.scalar.activation(Tsb[:, ch * 512:(ch + 1) * 512],
                                             Tp[:], AF.Copy)
                        # PE-transpose T chunk into TT[p, (j b)] blocks
                        for jj in range(4):
                            j = ch * 4 + jj
                            pt = ps.tile([128, 6], F32, tag="tps")
                            nc.tensor.transpose(
                                pt[:], Tsb[:, j * 128:(j + 1) * 128], i6)
                            nc.scalar.activation(TT[:, j * 6:(j + 1) * 6],
                                                 pt[:], AF.Copy)

                # C[a, b] = sum_j m1'_j^T TT_j  (two groups for overlap)
                pc0 = psc.tile([6, 6], F32, tag="pc0")
                pc1 = psc.tile([6, 6], F32, tag="pc1")
                for j in range(CB):
                    pc = pc0 if j < 12 else pc1
                    nc.tensor.matmul(pc[:], m1s_s[:, j * 6:(j + 1) * 6],
                                     TT[:, j * 6:(j + 1) * 6],
                                     start=(j % 12 == 0), stop=(j % 12 == 11))
                Cp = sp.tile([6, 6], F32, tag="Cp")
                nc.vector.tensor_copy(Cp[:], pc0[:])
                nc.vector.tensor_tensor(Cp[:], Cp[:], pc1[:], OP.add)
                nc.sync.dma_start(cr_in[:], Cp[:])

                # ---------- coll2: AllReduce 6x6 gram ----------
                if no_coll:
                    nc.sync.dma_start(cr_out[:], cr_in[:])
                else:
                    nc.gpsimd.collective_compute(
                        "AllReduce", OP.add, replica_groups=groups,
                        ins=[cr_in[:]], outs=[cr_out[:]])

                if no_tail:
                    nn = 6 if dbg_c else 3
                    dummy = sp.tile([nn, nn], F32, tag="dummy")
                    nc.sync.dma_start(dummy[:], cr_out[0:nn, 0:nn])
                    nc.sync.dma_start(out_d[:], dummy[:])
                    continue

                # ---------- tail ----------
                _tail(nc, pp, sp, ps, psc, cps, c0s, cr_out, stage, mshuf, out_d)

    nc.compile()
    return nc


def _transpose(nc, ps, sp, in_sb, n, idn, tag):
    pt = ps.tile([n, n], F32, tag="tps")
    nc.tensor.transpose(pt[:], in_sb, idn[:n, :n])
    ot = sp.tile([n, n], F32, tag=f"ot_{tag}")
    nc.vector.tensor_copy(ot[:], pt[:])
    return ot


def _powchain(nc, ps, sp, m_sb, n, tag, n_squarings=5, extra=True):
    """M^50 (extra=True: 5 squarings + M48=M32@M16 + M50=M48@M2) or M^32."""
    powers = {}
    cur = m_sb
    for i in range(1, n_squarings + 1):
        pm = ps.tile([n, n], F32, tag="tps")
        nc.tensor.matmul(pm[:], cur, cur, start=True, stop=True)
        nxt = sp.tile([n, n], F32, tag=f"pw_{tag}_{i}")
        nc.vector.tensor_scalar_mul(nxt[:], pm[:], 2.0)
        powers[2 ** i] = nxt
        cur = nxt[:]
    if not extra:
        return powers[2 ** n_squarings]
    pm = ps.tile([n, n], F32, tag="tps")
    nc.tensor.matmul(pm[:], powers[32][:], powers[16][:], start=True, stop=True)
    m48 = sp.tile([n, n], F32, tag=f"pw_{tag}_48")
    nc.vector.tensor_scalar_mul(m48[:], pm[:], 2.0)
    pm = ps.tile([n, n], F32, tag="tps")
    nc.tensor.matmul(pm[:], m48[:], powers[2][:], start=True, stop=True)
    m50 = sp.tile([n, n], F32, tag=f"pw_{tag}_50")
    nc.vector.tensor_scalar_mul(m50[:], pm[:], 2.0)
    return m50


def _tail(nc, pp, sp, ps, psc, cps, c0s, cr_out, stage, mshuf, out_d):
    """C' -> Hartley -> L-transform -> Mmat -> chains -> projection."""
    idn = cps[0:9, C_IDN:C_IDN + 9]

    Cp = sp.tile([6, 6], F32, tag="Cpr")
    nc.sync.dma_start(Cp[:], cr_out[:])
    CpT = sp.tile([6, 6], F32, tag="CprT")
    nc.scalar.dma_start(CpT[:], cr_out[:].rearrange("a b -> b a"))

    # moments [1,12]: side1 = C'[:,5], side2 = C'[5,:]
    sc = pp.tile([128, 112], F32, tag="tailsc")
    nc.scalar.dma_start(sc[0:1, 0:6],
                        cr_out[:].rearrange("a b -> b a")[5:6, :])
    nc.sync.dma_start(sc[0:1, 6:12], cr_out[5:6, :])

    def scv(a, b):
        return sc[0:1, a:b]

    def pair(k):
        return sc[0:1, 0:12].rearrange("p (g d) -> p d g", g=2)[:, k, :]

    Sxx, Sx, Syy, Sy, Sw = pair(0), pair(2), pair(3), pair(4), pair(5)
    ws = scv(12, 14); nc.vector.tensor_scalar_add(ws, Sw, EPS)
    rws = scv(14, 16); nc.vector.reciprocal(rws, ws)
    cx = scv(16, 18); nc.vector.tensor_tensor(cx, Sx, rws, OP.mult)  # = dx
    cy = scv(18, 20); nc.vector.tensor_tensor(cy, Sy, rws, OP.mult)  # = dy
    t_a = scv(20, 22); nc.vector.tensor_tensor(t_a, cx, Sx, OP.mult)
    t_b = scv(22, 24); nc.vector.tensor_tensor(t_b, cy, Sy, OP.mult)
    cdS = scv(24, 26); nc.vector.tensor_tensor(cdS, t_a, t_b, OP.add)
    u_a = scv(26, 28); nc.vector.tensor_tensor(u_a, cx, cx, OP.mult)
    u_b = scv(28, 30); nc.vector.tensor_tensor(u_b, cy, cy, OP.mult)
    c2_ = scv(30, 32); nc.vector.tensor_tensor(c2_, u_a, u_b, OP.add)
    sq_ = scv(32, 34); nc.vector.tensor_tensor(sq_, Sxx, Syy, OP.add)
    n2c = scv(34, 36); nc.vector.tensor_scalar_mul(n2c, cdS, -2.0)
    c2w = scv(36, 38); nc.vector.tensor_tensor(c2w, c2_, Sw, OP.mult)
    m_ = scv(38, 40); nc.vector.tensor_tensor(m_, sq_, n2c, OP.add)
    m2_ = scv(40, 42); nc.vector.tensor_tensor(m2_, m_, c2w, OP.add)
    md2 = scv(42, 44); nc.vector.tensor_tensor(md2, m2_, rws, OP.mult)
    md2e = scv(44, 46); nc.vector.tensor_scalar_add(md2e, md2, EPS)
    md = scv(46, 48); nc.scalar.activation(md, md2e, AF.Sqrt)
    mde = scv(48, 50); nc.vector.tensor_scalar_add(mde, md, EPS)
    rmd = scv(50, 52); nc.vector.reciprocal(rmd, mde)
    s_ = scv(52, 54); nc.vector.tensor_scalar_mul(s_, rmd, SQRT2)
    # real centroids: cr = dx + c0 ; c0s = [c0x c0x c0y c0y] paired
    cxr = scv(54, 56); nc.vector.tensor_tensor(cxr, cx, c0s[0:1, 0:2], OP.add)
    cyr = scv(56, 58); nc.vector.tensor_tensor(cyr, cy, c0s[0:1, 2:4], OP.add)
    scx = scv(58, 60); nc.vector.tensor_tensor(scx, s_, cxr, OP.mult)
    scy = scv(60, 62); nc.vector.tensor_tensor(scy, s_, cyr, OP.mult)
    nscx = scv(62, 64); nc.vector.tensor_scalar_mul(nscx, scx, -1.0)
    nscy = scv(64, 66); nc.vector.tensor_scalar_mul(nscy, scy, -1.0)
    # L scalars: s2, dx2, dxy, dy2 (paired)
    s2p = scv(66, 68); nc.vector.tensor_tensor(s2p, s_, s_, OP.mult)
    dx2 = scv(68, 70); nc.vector.tensor_tensor(dx2, cx, cx, OP.mult)
    dxy = scv(70, 72); nc.vector.tensor_tensor(dxy, cx, cy, OP.mult)
    dy2 = scv(72, 74); nc.vector.tensor_tensor(dy2, cy, cy, OP.mult)

    # T row-major 9-vectors: t1v at 76:85, t2v at 85:94
    nc.vector.memset(scv(76, 94), 0.0)
    tv = sc[0:1, 76:94]
    tv9 = tv.rearrange("p (v f) -> p v f", v=2)
    nc.vector.tensor_copy(tv9[:, :, 0:1], s_.unsqueeze(2))
    nc.vector.tensor_copy(tv9[:, :, 4:5], s_.unsqueeze(2))
    nc.vector.tensor_copy(
        tv9[:, :, 2:8].rearrange("p v (c d) -> p v c d", c=2)[:, :, :, 0:1],
        sc[0:1, 62:66].rearrange("p (c v) -> p v c", c=2).unsqueeze(3))
    nc.vector.memset(tv9[:, :, 8:9], 1.0)
    nc.sync.dma_start(stage[0:18], tv)
    T12 = sp.tile([3, 6], F32, tag="T12")
    nc.sync.dma_start(
        T12[:].rearrange("i (v j) -> i v j", v=2),
        stage[0:18].rearrange("(v i j) -> i v j", i=3, j=3))

    # broadcast scalar strip to 6 partitions for the L build (PE ones)
    ones16 = cps[0:1, C_ONE:C_ONE + 6]
    scBp = ps.tile([6, 80], F32, tag="tps")
    nc.tensor.matmul(scBp[:], ones16, sc[0:1, 0:80], start=True, stop=True)
    scB = sp.tile([6, 80], F32, tag="scB")
    nc.vector.tensor_copy(scB[:], scBp[:])

    def shT(side, tag):
        """Sh^T for side (0/1): I^T + dx E1^T + dy E2^T + dx2 E3^T + ..."""
        dx = scB[:, 16 + side:17 + side]
        dy = scB[:, 18 + side:19 + side]
        dx2_ = scB[:, 68 + side:69 + side]
        dxy_ = scB[:, 70 + side:71 + side]
        dy2_ = scB[:, 72 + side:73 + side]
        def M(i):
            return cps[0:6, C_SHT + 6 * i:C_SHT + 6 * i + 6]
        acc = sp.tile([6, 6], F32, tag=f"sh_{tag}")
        nc.vector.scalar_tensor_tensor(acc[:], M(1), dx, M(0), OP.mult, OP.add)
        for i, sval in [(2, dy), (3, dx2_), (4, dxy_), (5, dy2_)]:
            nc.vector.scalar_tensor_tensor(acc[:], M(i), sval, acc[:],
                                           OP.mult, OP.add)
        return acc

    Sh1T = shT(0, "1")
    Sh2T = shT(1, "2")
    # svec side1 as a [6,1] column (per-partition): c2m*s2 + c1m*s + c0m
    sv1c = sp.tile([6, 1], F32, tag="sv1c")
    tmp1 = sp.tile([6, 1], F32, tag="svt1")
    nc.vector.scalar_tensor_tensor(
        tmp1[:], cps[0:6, C_MSK:C_MSK + 1], scB[:, 66:67],
        cps[0:6, C_MSK + 2:C_MSK + 3], OP.mult, OP.add)
    nc.vector.scalar_tensor_tensor(
        sv1c[:], cps[0:6, C_MSK + 1:C_MSK + 2], scB[:, 52:53],
        tmp1[:], OP.mult, OP.add)
    # svec side2 as a [1,6] row on partition 0: [s2 s2 s s2 s 1]
    svr2 = sc[0:1, 96:102]
    s2v2 = sc[0:1, 67:68]
    sv2 = sc[0:1, 53:54]
    nc.vector.tensor_copy(
        svr2.rearrange("p (a b) -> p a b", a=3)[:, 0:2, 0:1],
        s2v2.unsqueeze(2).to_broadcast([1, 2, 1]))   # slots 0,2 = s2 (a-major)
    nc.vector.tensor_copy(svr2[:, 1:2], s2v2)        # slot 1 = s2
    nc.vector.tensor_copy(svr2[:, 3:4], s2v2)        # slot 3 = s2
    nc.vector.tensor_copy(svr2[:, 2:3], sv2)         # slot 2 = s
    nc.vector.tensor_copy(svr2[:, 4:5], sv2)         # slot 4 = s
    nc.vector.memset(svr2[:, 5:6], 1.0)
    sv2B = sp.tile([6, 6], F32, tag="sv2B")
    sv2Bp = ps.tile([6, 6], F32, tag="tps")
    nc.tensor.matmul(sv2Bp[:], ones16, svr2, start=True, stop=True)
    nc.vector.tensor_copy(sv2B[:], sv2Bp[:])

    # C2 = D1 Sh1 C' Sh2^T D2
    vps = ps.tile([6, 6], F32, tag="tps")
    nc.tensor.matmul(vps[:], Sh1T[:], Cp[:], start=True, stop=True)  # Sh1 C'
    vS = sp.tile([6, 6], F32, tag="vS")
    nc.vector.tensor_copy(vS[:], vps[:])
    vT = _transpose(nc, ps, sp, vS[:], 6, idn, "vT")
    ups = ps.tile([6, 6], F32, tag="tps")
    nc.tensor.matmul(ups[:], vT[:], Sh2T[:], start=True, stop=True)  # v Sh2^T
    # C2[r, c] = svec1[r] * u[r, c] * svec2[c]
    u1 = sp.tile([6, 6], F32, tag="u1")
    nc.vector.tensor_scalar_mul(u1[:], ups[:], sv1c[:])
    C2 = sp.tile([6, 6], F32, tag="C2")
    nc.vector.tensor_tensor(C2[:], u1[:], sv2B[:], OP.mult)
    C2T = _transpose(nc, ps, sp, C2[:], 6, idn, "c2t")

    _solve(nc, pp, sp, ps, psc, cps, idn, sc, C2[:], C2T[:], stage, mshuf,
           out_d, T12)


def _solve(nc, pp, sp, ps, psc, cps, idn, sc, C2, C2T, stage, mshuf, out_d,
           T12):
    i9h = cps[0:9, C_I9H:C_I9H + 9]
    et69 = cps[0:6, C_ET69:C_ET69 + 9]
    i3c = cps[0:3, C_I3:C_I3 + 3]
    v09 = cps[0:9, C_V09:C_V09 + 1]
    v06 = cps[0:6, C_V06:C_V06 + 1]
    sel1 = cps[0:3, C_SEL1:C_SEL1 + 6]
    sel2 = cps[0:3, C_SEL2:C_SEL2 + 6]

    # G2 = E C2 E^T : G2[3a+b, 3c+d] = C2[pair(a,b), pair(c,d)]
    z_ps = ps.tile([6, 9], F32, tag="tps")
    nc.tensor.matmul(z_ps[:], C2T, et69, start=True, stop=True)  # C2 E^T
    Zs = sp.tile([6, 9], F32, tag="Zs")
    nc.vector.tensor_copy(Zs[:], z_ps[:])
    g_ps = ps.tile([9, 9], F32, tag="tps")
    nc.tensor.matmul(g_ps[:], et69, Zs[:], start=True, stop=True)    # E @ Z
    G2 = sp.tile([9, 9], F32, tag="G2")
    nc.vector.tensor_copy(G2[:], g_ps[:])

    # Mmat[3p+q, 3r+s] = G2[3p+r, 3q+s]: bounce via DRAM
    nc.sync.dma_start(mshuf[:], G2[:])
    Mmat = sp.tile([9, 9], F32, tag="Mmat")
    for p in range(3):
        eng = nc.scalar if p == 1 else nc.sync
        eng.dma_start(
            Mmat[3 * p:3 * p + 3, :].rearrange("q (r s) -> q r s", s=3),
            mshuf[:].rearrange("(p q1 r s) -> p q1 r s", p=3, q1=3, r=3)
            .transpose([0, 2, 1, 3])[p])

    # Msp = Mmat/(2 lam) - I/2
    dg = sp.tile([9, 9], F32, tag="dg")
    nc.vector.tensor_tensor(dg[:], Mmat[:], i9h, OP.mult)
    lam2 = sp.tile([9, 1], F32, tag="lam2")
    nc.vector.tensor_reduce(lam2[:], dg[:], AX.X, OP.add)
    ones99 = cps[0:9, C_ONE:C_ONE + 9]
    lam2r = ps.tile([9, 1], F32, tag="tps")
    nc.tensor.matmul(lam2r[:], ones99, lam2[:], start=True, stop=True)
    lam4 = sp.tile([9, 1], F32, tag="lam4")
    nc.vector.tensor_scalar_mul(lam4[:], lam2r[:], 4.0)
    inv2l = sp.tile([9, 1], F32, tag="inv2l")
    nc.vector.reciprocal(inv2l[:], lam4[:])
    Msp = sp.tile([9, 9], F32, tag="Msp")
    nc.vector.scalar_tensor_tensor(Msp[:], Mmat[:], inv2l[:], i9h,
                                   OP.mult, OP.subtract)
    M50 = _powchain(nc, ps, sp, Msp[:], 9, "m9", 5, extra=True)

    w9ps = ps.tile([1, 9], F32, tag="tps")
    nc.tensor.matmul(w9ps[:], v09, M50[:], start=True, stop=True)
    w9 = sp.tile([1, 9], F32, tag="w9")
    nc.vector.tensor_copy(w9[:], w9ps[:])
    nc.sync.dma_start(stage[24:33], w9[:])  # raw; 1/||w9|| folded at the end
    w9sq = sp.tile([1, 9], F32, tag="w9sq")
    nc.vector.tensor_tensor(w9sq[:], w9[:], w9[:], OP.mult)
    nn9 = sp.tile([1, 1], F32, tag="nn9")
    nc.vector.tensor_reduce(nn9[:], w9sq[:], AX.X, OP.add)
    sr9 = sp.tile([1, 1], F32, tag="sr9")
    nc.scalar.activation(sr9[:], nn9[:], AF.Sqrt)
    rs9 = sp.tile([1, 1], F32, tag="rs9")
    nc.vector.reciprocal(rs9[:], sr9[:])
    rs9c = psc.tile([3, 1], F32, tag="rs9c")
    nc.tensor.matmul(rs9c[:], cps[0:1, C_ONE:C_ONE + 3], rs9[:],
                     start=True, stop=True)

    # E = T2^T E_raw T1 (and E^T);  T1m/T2m preloaded in T12
    T1m = T12[:, 0:3]
    T2m = T12[:, 3:6]
    Eraw = sp.tile([3, 3], F32, tag="Eraw")
    nc.sync.dma_start(Eraw[:], stage[24:33].rearrange("(i j) -> i j", j=3))

    a1ps = ps.tile([3, 3], F32, tag="tps")
    nc.tensor.matmul(a1ps[:], T2m, Eraw[:], start=True, stop=True)
    A1 = sp.tile([3, 3], F32, tag="A1")
    nc.vector.tensor_copy(A1[:], a1ps[:])
    A1T = _transpose(nc, ps, sp, A1[:], 3, idn, "a1t")
    etps = ps.tile([3, 3], F32, tag="tps")
    nc.tensor.matmul(etps[:], T1m, A1T[:], start=True, stop=True)
    ETs = sp.tile([3, 3], F32, tag="ETs")
    nc.vector.tensor_copy(ETs[:], etps[:])
    Es = _transpose(nc, ps, sp, ETs[:], 3, idn, "es")

    # B = E^T E ; blockdiag 6x6 chain (32 iters) for v1 (max) and v3 (min)
    bps = ps.tile([3, 3], F32, tag="tps")
    nc.tensor.matmul(bps[:], Es[:], Es[:], start=True, stop=True)
    Bm = sp.tile([3, 3], F32, tag="Bm")
    nc.vector.tensor_copy(Bm[:], bps[:])
    dg3 = sp.tile([3, 3], F32, tag="dg3")
    nc.vector.tensor_tensor(dg3[:], Bm[:], i3c, OP.mult)
    lb = sp.tile([3, 1], F32, tag="lb")
    nc.vector.tensor_reduce(lb[:], dg3[:], AX.X, OP.add)
    lbr = ps.tile([3, 1], F32, tag="tps")
    nc.tensor.matmul(lbr[:], cps[0:3, C_ONE:C_ONE + 3], lb[:],
                     start=True, stop=True)
    invlb = sp.tile([3, 1], F32, tag="invlb")
    nc.vector.reciprocal(invlb[:], lbr[:])
    Bs3 = sp.tile([3, 3], F32, tag="Bs3")
    nc.vector.tensor_scalar_mul(Bs3[:], Bm[:], invlb[:])
    IB = sp.tile([3, 3], F32, tag="IB")
    nc.vector.tensor_tensor(IB[:], i3c, Bs3[:], OP.subtract)
    bdps = ps.tile([6, 6], F32, tag="tps")
    nc.tensor.matmul(bdps[:, 0:3], sel1, Bs3[:], start=True, stop=True)
    nc.tensor.matmul(bdps[:, 3:6], sel2, IB[:], start=True, stop=True)
    BD = sp.tile([6, 6], F32, tag="BD")
    nc.vector.tensor_copy(BD[:], bdps[:])
    BD32 = _powchain(nc, ps, sp, BD[:], 6, "m6", 5, extra=False)

    w6ps = ps.tile([1, 6], F32, tag="tps")
    nc.tensor.matmul(w6ps[:], v06, BD32[:], start=True, stop=True)
    w6 = sp.tile([1, 6], F32, tag="w6")
    nc.vector.tensor_copy(w6[:], w6ps[:])
    w6sq = sp.tile([1, 6], F32, tag="w6sq")
    nc.vector.tensor_tensor(w6sq[:], w6[:], w6[:], OP.mult)
    nn6 = sp.tile([1, 2], F32, tag="nn6")
    nc.vector.tensor_reduce(nn6[:].unsqueeze(2),
                            w6sq[:].rearrange("p (g d) -> p g d", g=2), AX.X,
                            OP.add)
    sr6 = sp.tile([1, 2], F32, tag="sr6")
    nc.scalar.activation(sr6[:], nn6[:], AF.Sqrt)
    rs6 = sp.tile([1, 2], F32, tag="rs6")
    nc.vector.reciprocal(rs6[:], sr6[:])
    vv = sp.tile([1, 6], F32, tag="vv")
    nc.vector.tensor_tensor(
        vv[:].rearrange("p (g d) -> p g d", g=2),
        w6[:].rearrange("p (g d) -> p g d", g=2),
        rs6[:].unsqueeze(2).to_broadcast([1, 2, 3]), OP.mult)

    # v2 = cross(v3, v1), normalized with EPS
    aa = sp.tile([1, 6], F32, tag="aa")
    nc.vector.tensor_copy(
        aa[:].rearrange("p (r d) -> p r d", r=2),
        vv[:, 3:6].unsqueeze(1).to_broadcast([1, 2, 3]))
    bb = sp.tile([1, 6], F32, tag="bb")
    nc.vector.tensor_copy(
        bb[:].rearrange("p (r d) -> p r d", r=2),
        vv[:, 0:3].unsqueeze(1).to_broadcast([1, 2, 3]))
    cr1 = sp.tile([1, 3], F32, tag="cr1")
    nc.vector.tensor_tensor(cr1[:], aa[:, 1:4], bb[:, 2:5], OP.mult)
    cr2 = sp.tile([1, 3], F32, tag="cr2")
    nc.vector.tensor_tensor(cr2[:], aa[:, 2:5], bb[:, 1:4], OP.mult)
    v2r = sp.tile([1, 3], F32, tag="v2r")
    nc.vector.tensor_tensor(v2r[:], cr1[:], cr2[:], OP.subtract)
    v2sq = sp.tile([1, 3], F32, tag="v2sq")
    nc.vector.tensor_tensor(v2sq[:], v2r[:], v2r[:], OP.mult)
    nn2 = sp.tile([1, 1], F32, tag="nn2")
    nc.vector.tensor_reduce(nn2[:], v2sq[:], AX.X, OP.add)
    sr2 = sp.tile([1, 1], F32, tag="sr2")
    nc.scalar.activation(sr2[:], nn2[:], AF.Sqrt)
    sr2e = sp.tile([1, 1], F32, tag="sr2e")
    nc.vector.tensor_scalar_add(sr2e[:], sr2[:], EPS)
    rs2 = sp.tile([1, 1], F32, tag="rs2")
    nc.vector.reciprocal(rs2[:], sr2e[:])
    v2 = sp.tile([1, 3], F32, tag="v2")
    nc.vector.tensor_tensor(v2[:], v2r[:], rs2[:].to_broadcast([1, 3]), OP.mult)

    vvv = sp.tile([1, 6], F32, tag="vvv")
    nc.vector.tensor_copy(vvv[:, 0:3], vv[:, 0:3])
    nc.vector.tensor_copy(vvv[:, 3:6], v2[:])
    nc.sync.dma_start(stage[33:39], vvv[:])
    Vr = sp.tile([2, 3], F32, tag="Vr")
    nc.sync.dma_start(Vr[:], stage[33:39].rearrange("(i k) -> i k", k=3))
    Vc = sp.tile([3, 2], F32, tag="Vc")
    nc.scalar.dma_start(Vc[:], stage[33:39].rearrange("(i k) -> k i", k=3))
    evps = ps.tile([2, 3], F32, tag="tps")
    nc.tensor.matmul(evps[:], Vc[:], ETs[:], start=True, stop=True)
    Evr = sp.tile([2, 3], F32, tag="Evr")
    nc.vector.tensor_copy(Evr[:], evps[:])
    evsq = sp.tile([2, 3], F32, tag="evsq")
    nc.vector.tensor_tensor(evsq[:], Evr[:], Evr[:], OP.mult)
    ss2 = sp.tile([2, 1], F32, tag="ss2")
    nc.vector.tensor_reduce(ss2[:], evsq[:], AX.X, OP.add)
    sv = sp.tile([2, 1], F32, tag="sv")
    nc.scalar.activation(sv[:], ss2[:], AF.Sqrt)
    ssum = ps.tile([2, 1], F32, tag="tps")
    nc.tensor.matmul(ssum[:], cps[0:2, C_ONE:C_ONE + 2], sv[:],
                     start=True, stop=True)
    savg = sp.tile([2, 1], F32, tag="savg")
    nc.vector.tensor_scalar_mul(savg[:], ssum[:], 0.5)
    sve = sp.tile([2, 1], F32, tag="sve")
    nc.vector.tensor_scalar_add(sve[:], sv[:], EPS)
    rsv = sp.tile([2, 1], F32, tag="rsv")
    nc.vector.reciprocal(rsv[:], sve[:])
    f2 = sp.tile([2, 1], F32, tag="f2")
    nc.vector.tensor_tensor(f2[:], rsv[:], savg[:], OP.mult)
    U2 = sp.tile([2, 3], F32, tag="U2")
    nc.vector.tensor_scalar_mul(U2[:], Evr[:], f2[:])
    ops_ = ps.tile([3, 3], F32, tag="tps")
    nc.tensor.matmul(ops_[:], U2[:], Vr[:], start=True, stop=True)
    outs = sp.tile([3, 3], F32, tag="outs")
    nc.vector.tensor_scalar_mul(outs[:], ops_[:], rs9c[:])
    nc.sync.dma_start(out_d[:], outs[:])


def make_in_maps(P, K):
    P = np.asarray(P, np.float32)
    K = np.asarray(K, np.float32)
    Pc = np.ascontiguousarray(P[:N, :N])
    PcT = np.ascontiguousarray(Pc.T)
    Mp, cpack, c0x, c0y = host_constants(K)
    m1full = _tile128(Mp, CB)
    c0t = np.array([[c0x, c0x, c0y, c0y]], np.float32)
    in_maps = []
    for k in range(NCORES):
        in_maps.append({
            "xn": _tile128(Pc[k * SH:(k + 1) * SH], RT),
            "xc": _tile128(PcT[k * SH:(k + 1) * SH], RT),
            "m1f": m1full,
            "m2s": _tile128(Mp[k * SH:(k + 1) * SH], RT),
            "cpack": cpack,
            "c0t": c0t,
        })
    return in_maps


_NC_CACHE = {}


def kernel(P, K):
    from concourse.bass_utils import run_bass_kernel_spmd
    if "nc" not in _NC_CACHE:
        _NC_CACHE["nc"] = build_nc()
    nc = _NC_CACHE["nc"]
    in_maps = make_in_maps(P, K)
    res = run_bass_kernel_spmd(nc, in_maps, core_ids=list(range(NCORES)))
    return np.asarray(res.results[0]["out"], np.float32)


# revision 23
# speedup vs baseline: 1.4924x; 1.0164x over previous
"""Trainium2 Bass kernel for nn_EssentialMatrixEstimator (v2).

Distribution (8 cores):
  - XN: natural row-shard  (384 rows x 3072 cols) -> exact row top-3 thresholds.
  - XC: transposed col-shard (384 cols x 3072 rows as [col, row]) -> exact col
    top-3 thresholds + dense masking + col-sharded gram.
  - coll1: AllGather of per-core row thresholds (384 f32 -> 3072).
  - coll2: AllReduce of the 6x6 gram C' on PRE-CENTERED monomials.

Math: the (N*M,9) epipolar Gram collapses to the 6x6 monomial Gram C'.
Monomials are pre-centered about the host constant c0 (grid centroid), so C'
is well-conditioned; the Hartley normalization is recovered from C' moments
(row/col 5) and applied as a 6x6 L-transform C2 = L1 C' L2^T instead of a
second gram pass.  Mmat (9x9) is an index expansion of C2; min-eigvector via
50-step shifted power iteration (rescaled repeated squaring), projection via
a 32-step 6x6 blockdiag chain (insensitive; validated 2.9e-4).

The big T = M2'^T W^T contraction streams in float32r (1 cy/row); validated
tolerant to tf32/bf16-level rounding (5e-4 / 3.9e-3 final rel err).  T chunks
are PE-transposed into TT [128, (j b)] (partition-scattered DMA transposes
measured 15-20us and were replaced).  The tail runs gpsimd-free (PE ones-
matmul broadcasts/reductions) to avoid gpsimd library-swap stalls; w9
normalization is deferred and folded into the final output scale.
"""

import os

os.environ.setdefault("JAX_PLATFORMS", "axon")

import numpy as np

import concourse.bass as bass
import concourse.bass_isa as bass_isa
import concourse.mybir as mybir
import concourse.bacc as bacc
import concourse.tile as tile

NCORES = 8
N = 3072
SH = N // NCORES          # 384 rows/cols per core
RT = SH // 128            # 3 tiles per core shard
CB = N // 128             # 24 tiles across the full dim
F32 = mybir.dt.float32
F32R = mybir.dt.float32r
AF = mybir.ActivationFunctionType
OP = mybir.AluOpType
AX = mybir.AxisListType

EPS = 1e-8
SQRT2 = 1.4142135623730951
INV_SQRT3 = 1.0 / 1.7320508075688772
T0 = float(np.nextafter(np.float32(0.01), np.float32(1)))  # x > 0.01 == x >= T0
H, W = 64, 64

# cpack const layout (tensor [9, C_TOT]): column ranges
C_I9H = 0      # I9 * 0.5            [9, 9]
C_ET69 = 9     # E^T selector        [6, 9]
C_I3 = 18      # I3                  [3, 3]
C_V09 = 21     # full(1/3)           [9, 1]
C_V06 = 22     # full(1/sqrt3)       [6, 1]
C_SEL1 = 23    # [I3 | 0]            [3, 6]
C_SEL2 = 29    # [0 | I3]            [3, 6]
C_SHT = 35     # Sh component mats^T: I6, E1^T..E5^T   [6, 6*6]
C_MSK = 71     # svec masks [c2m c1m c0m]  [6, 3]
C_IDN = 74     # identity 9x9        [9, 9]
C_ONE = 83     # all-ones            [9, 9]
C_TOT = 92

PAIRS = [(0, 0), (0, 1), (0, 2), (1, 1), (1, 2), (2, 2)]


def _pidx():
    d = {}
    for i, (a, b) in enumerate(PAIRS):
        d[(a, b)] = i
        d[(b, a)] = i
    return d


def grid_pts(K):
    idx = np.arange(H * W, dtype=np.float32)
    pix = np.stack([idx % np.float32(W), np.floor(idx / np.float32(W))], -1)
    K_inv = np.linalg.inv(np.asarray(K, np.float32)).astype(np.float32)
    p1h = np.concatenate([pix[:N], np.ones((N, 1), np.float32)], -1)
    pts = (p1h @ K_inv.T)[:, :2].astype(np.float32)
    return pts


def host_constants(K):
    """Pre-centered monomials + packed tail constants (f32)."""
    pts = grid_pts(K)
    x, y = pts[:, 0], pts[:, 1]
    c0x = np.float32(x.mean())
    c0y = np.float32(y.mean())
    xs = (x - c0x).astype(np.float32)
    ys = (y - c0y).astype(np.float32)
    Mp = np.stack([xs * xs, xs * ys, xs, ys * ys, ys, np.ones_like(xs)],
                  -1).astype(np.float32)

    cpack = np.zeros((9, C_TOT), np.float32)
    cpack[:9, C_I9H:C_I9H + 9] = 0.5 * np.eye(9, dtype=np.float32)
    pid = _pidx()
    for a in range(3):
        for b in range(3):
            cpack[pid[(a, b)], C_ET69 + 3 * a + b] = 1.0
    cpack[:3, C_I3:C_I3 + 3] = np.eye(3, dtype=np.float32)
    cpack[:9, C_V09] = 1.0 / 3.0
    cpack[:6, C_V06] = INV_SQRT3
    cpack[:3, C_SEL1:C_SEL1 + 3] = np.eye(3, dtype=np.float32)
    cpack[:3, C_SEL2 + 3:C_SEL2 + 6] = np.eye(3, dtype=np.float32)

    # Sh(dx,dy) = I + dx*E1 + dy*E2 + dx^2*E3 + dx*dy*E4 + dy^2*E5
    # (rows of L before the diag scale; see proto.Lmat)
    E1 = np.zeros((6, 6), np.float32)  # dx terms
    E1[0, 2] = -2.0
    E1[1, 4] = -1.0
    E1[2, 5] = -1.0
    E2 = np.zeros((6, 6), np.float32)  # dy terms
    E2[1, 2] = -1.0
    E2[3, 4] = -2.0
    E2[4, 5] = -1.0
    E3 = np.zeros((6, 6), np.float32)  # dx^2
    E3[0, 5] = 1.0
    E4 = np.zeros((6, 6), np.float32)  # dx*dy
    E4[1, 5] = 1.0
    E5 = np.zeros((6, 6), np.float32)  # dy^2
    E5[3, 5] = 1.0
    mats = [np.eye(6, dtype=np.float32), E1, E2, E3, E4, E5]
    for i, Em in enumerate(mats):
        cpack[:6, C_SHT + 6 * i:C_SHT + 6 * i + 6] = Em.T
    # svec masks: svec = [s2,s2,s,s2,s,1] = c2m*s2 + c1m*s + c0m
    cpack[:6, C_MSK + 0] = [1, 1, 0, 1, 0, 0]
    cpack[:6, C_MSK + 1] = [0, 0, 1, 0, 1, 0]
    cpack[:6, C_MSK + 2] = [0, 0, 0, 0, 0, 1]
    cpack[:9, C_IDN:C_IDN + 9] = np.eye(9, dtype=np.float32)
    cpack[:9, C_ONE:C_ONE + 9] = 1.0
    return Mp, cpack, float(c0x), float(c0y)


def _tile128(a, ntiles):
    """[ntiles*128, F] -> [128, ntiles*F] with [p, t*F+f] = a[t*128+p, f]."""
    F = a.shape[1]
    return np.ascontiguousarray(
        a.reshape(ntiles, 128, F).transpose(1, 0, 2).reshape(128, ntiles * F)
    )


def build_nc(repeats=1, no_coll=False, no_tail=False, use_f32r=True, dbg_c=False):
    nc = bacc.Bacc("TRN2", target_bir_lowering=False, debug=False,
                   num_devices=NCORES)

    xn = nc.dram_tensor("xn", [128, RT * N], F32, kind="ExternalInput")
    xc = nc.dram_tensor("xc", [128, RT * N], F32, kind="ExternalInput")
    m1f = nc.dram_tensor("m1f", [128, CB * 6], F32, kind="ExternalInput")
    m2s = nc.dram_tensor("m2s", [128, RT * 6], F32, kind="ExternalInput")
    cpk = nc.dram_tensor("cpack", [9, C_TOT], F32, kind="ExternalInput")
    c0t = nc.dram_tensor("c0t", [1, 4], F32, kind="ExternalInput")
    out_d = nc.dram_tensor("out", [6, 6] if dbg_c else [3, 3], F32, kind="ExternalOutput")

    tr_in = nc.dram_tensor("tr_in", [1, SH], F32)
    tr_out = nc.dram_tensor("tr_out", [NCORES, SH], F32, addr_space="Shared")
    cr_in = nc.dram_tensor("cr_in", [6, 6], F32)
    cr_out = nc.dram_tensor("cr_out", [6, 6], F32, addr_space="Shared")
    stage = nc.dram_tensor("stage", [64], F32)
    mshuf = nc.dram_tensor("mshuf", [81], F32)

    groups = [list(range(NCORES))]

    with tile.TileContext(nc) as tc:
        with (
            tc.tile_pool(name="persist", bufs=1) as pp,
            tc.tile_pool(name="scratch", bufs=2) as sp,
            tc.tile_pool(name="ps_t", bufs=2, space="PSUM") as ps,
            tc.tile_pool(name="ps_T", bufs=2, space="PSUM") as psT,
            tc.tile_pool(name="ps_c", bufs=1, space="PSUM") as psc,
        ):
            for _rep in range(repeats):
                # ---------- P0: loads (XN on qSP, XC on qACT) ----------
                XN = pp.tile([128, RT * N], F32, tag="XN")
                XC = pp.tile([128, RT * N], F32, tag="XC")
                HN = N // 2
                for t in range(RT):
                    a = t * N
                    nc.sync.dma_start(XN[:, a:a + HN], xn[:, a:a + HN])
                    nc.scalar.dma_start(XN[:, a + HN:a + N],
                                        xn[:, a + HN:a + N])
                for t in range(RT):
                    a = t * N
                    nc.sync.dma_start(XC[:, a:a + HN], xc[:, a:a + HN])
                    nc.scalar.dma_start(XC[:, a + HN:a + N],
                                        xc[:, a + HN:a + N])
                m1s_s = pp.tile([128, CB * 6], F32, tag="m1f")
                nc.scalar.dma_start(m1s_s[:], m1f[:])
                m2s_s = pp.tile([128, RT * 6], F32, tag="m2s")
                nc.scalar.dma_start(m2s_s[:], m2s[:])
                cps = pp.tile([9, C_TOT], F32, tag="cpk")
                nc.scalar.dma_start(cps[:], cpk[:])
                c0s = pp.tile([1, 4], F32, tag="c0")
                nc.scalar.dma_start(c0s[:], c0t[:])
                sqwarm = sp.tile([1, 1], F32, tag="sqwarm")
                nc.scalar.activation(sqwarm[:], cps[0:1, 0:1], AF.Sqrt)

                def XNt(t):
                    return XN[:, t * N:(t + 1) * N]

                def XCt(t):
                    return XC[:, t * N:(t + 1) * N]

                # ---------- P1: row thresholds -> coll1 ----------
                r8 = pp.tile([128, RT * 8], F32, tag="r8")
                for t in range(RT):
                    nc.vector.max(out=r8[:, t * 8:t * 8 + 8], in_=XNt(t))
                trT0 = pp.tile([128, RT], F32, tag="trT0")
                nc.vector.tensor_scalar_max(
                    trT0[:],
                    r8[:].rearrange("p (t e) -> p t e", e=8)[:, :, 2], T0)
                for t in range(RT):
                    nc.sync.dma_start(tr_in[0:1, t * 128:(t + 1) * 128],
                                      trT0[:, t:t + 1])

                if no_coll:
                    nc.sync.dma_start(tr_out[0:1, :], tr_in[:])
                else:
                    nc.gpsimd.collective_compute(
                        "AllGather", OP.bypass, replica_groups=groups,
                        ins=[tr_in[:]], outs=[tr_out[:]])

                # ---------- P2: col thresholds (local, exact) ----------
                c8 = pp.tile([128, RT * 8], F32, tag="c8")
                for t in range(RT):
                    nc.vector.max(out=c8[:, t * 8:t * 8 + 8], in_=XCt(t))

                # ---------- P3: broadcast row-threshold table ----------
                trow = pp.tile([1, N], F32, tag="trow")
                nc.sync.dma_start(trow[:], tr_out[:].rearrange("k i -> (k i)"))
                trB = pp.tile([128, N], F32, tag="trB")
                MCH = 1536
                for c0_ in range(0, N, MCH):
                    nc.gpsimd.partition_broadcast(
                        trB[:, c0_:c0_ + MCH], trow[:, c0_:c0_ + MCH],
                        channels=128)

                # ---------- P4: dense mask + fp32r T-gram ----------
                # W (f32r): W = XC * [XC >= max(trB, tc_t)]
                # T[b, r] = sum_c m2'[c, b] * W^T[c, r]   (PSUM chunks [6,512])
                WDT = F32R if use_f32r else F32
                m2r = pp.tile([128, RT * 6], WDT, tag="m2r")
                nc.vector.tensor_copy(m2r[:], m2s_s[:])
                Wr = pp.tile([128, RT * N], WDT, tag="Wr")
                Wf = pp.tile([128, N], F32, tag="Wf")  # t=2 chunk via gpsimd
                Tsb = pp.tile([6, N], F32, tag="Tsb")
                TT = pp.tile([128, CB * 6], F32, tag="TT")
                i6 = cps[0:6, C_IDN:C_IDN + 6]
                for h in range(2):
                    for t in range(RT):
                        tcl = c8[:, t * 8 + 2:t * 8 + 3]
                        sl = slice(t * N + h * MCH, t * N + (h + 1) * MCH)
                        msk = pp.tile([128, MCH], F32, tag=f"msk{h}{t}")
                        nc.vector.scalar_tensor_tensor(
                            msk[:], trB[:, h * MCH:(h + 1) * MCH], tcl,
                            XC[:, sl], OP.max, OP.is_le)
                        if t == 0:
                            nc.gpsimd.tensor_tensor(
                                Wf[:, h * MCH:(h + 1) * MCH], XC[:, sl],
                                msk[:], OP.mult)
                        else:
                            nc.vector.tensor_tensor(Wr[:, sl], XC[:, sl],
                                                    msk[:], OP.mult)
                    for q in range(3):
                        ch = h * 3 + q
                        Tp = psT.tile([6, 512], F32, tag="Tp")
                        for t in range(RT):
                            c0_ = t * N + h * MCH + q * 512
                            if t == 0:
                                nc.tensor.matmul(
                                    Tp[:], m2s_s[:, t * 6:(t + 1) * 6],
                                    Wf[:, h * MCH + q * 512:
                                        h * MCH + q * 512 + 512],
                                    start=True, stop=False)
                            else:
                                nc.tensor.matmul(
                                    Tp[:],
                                    m2r[:, t * 6:(t + 1) * 6],
                                    Wr[:, c0_:c0_ + 512],
                                    start=False, stop=(t == RT - 1))
                        nc.scalar.activation(Tsb[:, ch * 512:(ch + 1) * 512],
                                             Tp[:], AF.Copy)
                        # PE-transpose T chunk into TT[p, (j b)] blocks
                        for jj in range(4):
                            j = ch * 4 + jj
                            pt = ps.tile([128, 6], F32, tag="tps")
                            nc.tensor.transpose(
                                pt[:], Tsb[:, j * 128:(j + 1) * 128], i6)
                            nc.scalar.activation(TT[:, j * 6:(j + 1) * 6],
                                                 pt[:], AF.Copy)

                # C[a, b] = sum_j m1'_j^T TT_j  (two groups for overlap)
                pc0 = psc.tile([6, 6], F32, tag="pc0")
                pc1 = psc.tile([6, 6], F32, tag="pc1")
                for j in range(CB):
                    pc = pc0 if j < 12 else pc1
                    nc.tensor.matmul(pc[:], m1s_s[:, j * 6:(j + 1) * 6],
                                     TT[:, j * 6:(j + 1) * 6],
                                     start=(j % 12 == 0), stop=(j % 12 == 11))
                Cp = sp.tile([6, 6], F32, tag="Cp")
                nc.vector.tensor_copy(Cp[:], pc0[:])
                nc.vector.tensor_tensor(Cp[:], Cp[:], pc1[:], OP.add)
                nc.sync.dma_start(cr_in[:], Cp[:])

                # ---------- coll2: AllReduce 6x6 gram ----------
                if no_coll:
                    nc.sync.dma_start(cr_out[:], cr_in[:])
                else:
                    nc.gpsimd.collective_compute(
                        "AllReduce", OP.add, replica_groups=groups,
                        ins=[cr_in[:]], outs=[cr_out[:]])

                if no_tail:
                    nn = 6 if dbg_c else 3
                    dummy = sp.tile([nn, nn], F32, tag="dummy")
                    nc.sync.dma_start(dummy[:], cr_out[0:nn, 0:nn])
                    nc.sync.dma_start(out_d[:], dummy[:])
                    continue

                # ---------- tail ----------
                _tail(nc, pp, sp, ps, psc, cps, c0s, cr_out, stage, mshuf, out_d)

    nc.compile()
    return nc


def _transpose(nc, ps, sp, in_sb, n, idn, tag):
    pt = ps.tile([n, n], F32, tag="tps")
    nc.tensor.transpose(pt[:], in_sb, idn[:n, :n])
    ot = sp.tile([n, n], F32, tag=f"ot_{tag}")
    nc.vector.tensor_copy(ot[:], pt[:])
    return ot


def _powchain(nc, ps, sp, m_sb, n, tag, n_squarings=5, extra=True):
    """M^50 (extra=True: 5 squarings + M48=M32@M16 + M50=M48@M2) or M^32."""
    powers = {}
    cur = m_sb
    for i in range(1, n_squarings + 1):
        pm = ps.tile([n, n], F32, tag="tps")
        nc.tensor.matmul(pm[:], cur, cur, start=True, stop=True)
        nxt = sp.tile([n, n], F32, tag=f"pw_{tag}_{i}")
        nc.vector.tensor_scalar_mul(nxt[:], pm[:], 2.0)
        powers[2 ** i] = nxt
        cur = nxt[:]
    if not extra:
        return powers[2 ** n_squarings]
    pm = ps.tile([n, n], F32, tag="tps")
    nc.tensor.matmul(pm[:], powers[32][:], powers[16][:], start=True, stop=True)
    m48 = sp.tile([n, n], F32, tag=f"pw_{tag}_48")
    nc.vector.tensor_scalar_mul(m48[:], pm[:], 2.0)
    pm = ps.tile([n, n], F32, tag="tps")
    nc.tensor.matmul(pm[:], m48[:], powers[2][:], start=True, stop=True)
    m50 = sp.tile([n, n], F32, tag=f"pw_{tag}_50")
    nc.vector.tensor_scalar_mul(m50[:], pm[:], 2.0)
    return m50


def _tail(nc, pp, sp, ps, psc, cps, c0s, cr_out, stage, mshuf, out_d):
    """C' -> Hartley -> L-transform -> Mmat -> chains -> projection."""
    idn = cps[0:9, C_IDN:C_IDN + 9]

    Cp = sp.tile([6, 6], F32, tag="Cpr")
    nc.sync.dma_start(Cp[:], cr_out[:])
    CpT = sp.tile([6, 6], F32, tag="CprT")
    nc.scalar.dma_start(CpT[:], cr_out[:].rearrange("a b -> b a"))

    # moments [1,12]: side1 = C'[:,5], side2 = C'[5,:]
    sc = pp.tile([128, 112], F32, tag="tailsc")
    nc.scalar.dma_start(sc[0:1, 0:6],
                        cr_out[:].rearrange("a b -> b a")[5:6, :])
    nc.sync.dma_start(sc[0:1, 6:12], cr_out[5:6, :])

    def scv(a, b):
        return sc[0:1, a:b]

    def pair(k):
        return sc[0:1, 0:12].rearrange("p (g d) -> p d g", g=2)[:, k, :]

    Sxx, Sx, Syy, Sy, Sw = pair(0), pair(2), pair(3), pair(4), pair(5)
    ws = scv(12, 14); nc.vector.tensor_scalar_add(ws, Sw, EPS)
    rws = scv(14, 16); nc.vector.reciprocal(rws, ws)
    cx = scv(16, 18); nc.vector.tensor_tensor(cx, Sx, rws, OP.mult)  # = dx
    cy = scv(18, 20); nc.vector.tensor_tensor(cy, Sy, rws, OP.mult)  # = dy
    t_a = scv(20, 22); nc.vector.tensor_tensor(t_a, cx, Sx, OP.mult)
    t_b = scv(22, 24); nc.vector.tensor_tensor(t_b, cy, Sy, OP.mult)
    cdS = scv(24, 26); nc.vector.tensor_tensor(cdS, t_a, t_b, OP.add)
    u_a = scv(26, 28); nc.vector.tensor_tensor(u_a, cx, cx, OP.mult)
    u_b = scv(28, 30); nc.vector.tensor_tensor(u_b, cy, cy, OP.mult)
    c2_ = scv(30, 32); nc.vector.tensor_tensor(c2_, u_a, u_b, OP.add)
    sq_ = scv(32, 34); nc.vector.tensor_tensor(sq_, Sxx, Syy, OP.add)
    n2c = scv(34, 36); nc.vector.tensor_scalar_mul(n2c, cdS, -2.0)
    c2w = scv(36, 38); nc.vector.tensor_tensor(c2w, c2_, Sw, OP.mult)
    m_ = scv(38, 40); nc.vector.tensor_tensor(m_, sq_, n2c, OP.add)
    m2_ = scv(40, 42); nc.vector.tensor_tensor(m2_, m_, c2w, OP.add)
    md2 = scv(42, 44); nc.vector.tensor_tensor(md2, m2_, rws, OP.mult)
    md2e = scv(44, 46); nc.vector.tensor_scalar_add(md2e, md2, EPS)
    md = scv(46, 48); nc.scalar.activation(md, md2e, AF.Sqrt)
    mde = scv(48, 50); nc.vector.tensor_scalar_add(mde, md, EPS)
    rmd = scv(50, 52); nc.vector.reciprocal(rmd, mde)
    s_ = scv(52, 54); nc.vector.tensor_scalar_mul(s_, rmd, SQRT2)
    # real centroids: cr = dx + c0 ; c0s = [c0x c0x c0y c0y] paired
    cxr = scv(54, 56); nc.vector.tensor_tensor(cxr, cx, c0s[0:1, 0:2], OP.add)
    cyr = scv(56, 58); nc.vector.tensor_tensor(cyr, cy, c0s[0:1, 2:4], OP.add)
    scx = scv(58, 60); nc.vector.tensor_tensor(scx, s_, cxr, OP.mult)
    scy = scv(60, 62); nc.vector.tensor_tensor(scy, s_, cyr, OP.mult)
    nscx = scv(62, 64); nc.vector.tensor_scalar_mul(nscx, scx, -1.0)
    nscy = scv(64, 66); nc.vector.tensor_scalar_mul(nscy, scy, -1.0)
    # L scalars: s2, dx2, dxy, dy2 (paired)
    s2p = scv(66, 68); nc.vector.tensor_tensor(s2p, s_, s_, OP.mult)
    dx2 = scv(68, 70); nc.vector.tensor_tensor(dx2, cx, cx, OP.mult)
    dxy = scv(70, 72); nc.vector.tensor_tensor(dxy, cx, cy, OP.mult)
    dy2 = scv(72, 74); nc.vector.tensor_tensor(dy2, cy, cy, OP.mult)

    # T row-major 9-vectors: t1v at 76:85, t2v at 85:94
    nc.vector.memset(scv(76, 94), 0.0)
    tv = sc[0:1, 76:94]
    tv9 = tv.rearrange("p (v f) -> p v f", v=2)
    nc.vector.tensor_copy(tv9[:, :, 0:1], s_.unsqueeze(2))
    nc.vector.tensor_copy(tv9[:, :, 4:5], s_.unsqueeze(2))
    nc.vector.tensor_copy(
        tv9[:, :, 2:8].rearrange("p v (c d) -> p v c d", c=2)[:, :, :, 0:1],
        sc[0:1, 62:66].rearrange("p (c v) -> p v c", c=2).unsqueeze(3))
    nc.vector.memset(tv9[:, :, 8:9], 1.0)
    nc.sync.dma_start(stage[0:18], tv)
    T12 = sp.tile([3, 6], F32, tag="T12")
    nc.sync.dma_start(
        T12[:].rearrange("i (v j) -> i v j", v=2),
        stage[0:18].rearrange("(v i j) -> i v j", i=3, j=3))

    # broadcast scalar strip to 6 partitions for the L build (PE ones)
    ones16 = cps[0:1, C_ONE:C_ONE + 6]
    scBp = ps.tile([6, 80], F32, tag="tps")
    nc.tensor.matmul(scBp[:], ones16, sc[0:1, 0:80], start=True, stop=True)
    scB = sp.tile([6, 80], F32, tag="scB")
    nc.vector.tensor_copy(scB[:], scBp[:])

    def shT(side, tag):
        """Sh^T for side (0/1): I^T + dx E1^T + dy E2^T + dx2 E3^T + ..."""
        dx = scB[:, 16 + side:17 + side]
        dy = scB[:, 18 + side:19 + side]
        dx2_ = scB[:, 68 + side:69 + side]
        dxy_ = scB[:, 70 + side:71 + side]
        dy2_ = scB[:, 72 + side:73 + side]
        def M(i):
            return cps[0:6, C_SHT + 6 * i:C_SHT + 6 * i + 6]
        acc = sp.tile([6, 6], F32, tag=f"sh_{tag}")
        nc.vector.scalar_tensor_tensor(acc[:], M(1), dx, M(0), OP.mult, OP.add)
        for i, sval in [(2, dy), (3, dx2_), (4, dxy_), (5, dy2_)]:
            nc.vector.scalar_tensor_tensor(acc[:], M(i), sval, acc[:],
                                           OP.mult, OP.add)
        return acc

    Sh1T = shT(0, "1")
    Sh2T = shT(1, "2")
    # svec side1 as a [6,1] column (per-partition): c2m*s2 + c1m*s + c0m
    sv1c = sp.tile([6, 1], F32, tag="sv1c")
    tmp1 = sp.tile([6, 1], F32, tag="svt1")
    nc.vector.scalar_tensor_tensor(
        tmp1[:], cps[0:6, C_MSK:C_MSK + 1], scB[:, 66:67],
        cps[0:6, C_MSK + 2:C_MSK + 3], OP.mult, OP.add)
    nc.vector.scalar_tensor_tensor(
        sv1c[:], cps[0:6, C_MSK + 1:C_MSK + 2], scB[:, 52:53],
        tmp1[:], OP.mult, OP.add)
    # svec side2 as a [1,6] row on partition 0: [s2 s2 s s2 s 1]
    svr2 = sc[0:1, 96:102]
    s2v2 = sc[0:1, 67:68]
    sv2 = sc[0:1, 53:54]
    nc.vector.tensor_copy(
        svr2.rearrange("p (a b) -> p a b", a=3)[:, 0:2, 0:1],
        s2v2.unsqueeze(2).to_broadcast([1, 2, 1]))   # slots 0,2 = s2 (a-major)
    nc.vector.tensor_copy(svr2[:, 1:2], s2v2)        # slot 1 = s2
    nc.vector.tensor_copy(svr2[:, 3:4], s2v2)        # slot 3 = s2
    nc.vector.tensor_copy(svr2[:, 2:3], sv2)         # slot 2 = s
    nc.vector.tensor_copy(svr2[:, 4:5], sv2)         # slot 4 = s
    nc.vector.memset(svr2[:, 5:6], 1.0)
    sv2B = sp.tile([6, 6], F32, tag="sv2B")
    sv2Bp = ps.tile([6, 6], F32, tag="tps")
    nc.tensor.matmul(sv2Bp[:], ones16, svr2, start=True, stop=True)
    nc.vector.tensor_copy(sv2B[:], sv2Bp[:])

    # C2 = D1 Sh1 C' Sh2^T D2
    vps = ps.tile([6, 6], F32, tag="tps")
    nc.tensor.matmul(vps[:], Sh1T[:], Cp[:], start=True, stop=True)  # Sh1 C'
    vS = sp.tile([6, 6], F32, tag="vS")
    nc.vector.tensor_copy(vS[:], vps[:])
    vT = _transpose(nc, ps, sp, vS[:], 6, idn, "vT")
    ups = ps.tile([6, 6], F32, tag="tps")
    nc.tensor.matmul(ups[:], vT[:], Sh2T[:], start=True, stop=True)  # v Sh2^T
    # C2[r, c] = svec1[r] * u[r, c] * svec2[c]
    u1 = sp.tile([6, 6], F32, tag="u1")
    nc.vector.tensor_scalar_mul(u1[:], ups[:], sv1c[:])
    C2 = sp.tile([6, 6], F32, tag="C2")
    nc.vector.tensor_tensor(C2[:], u1[:], sv2B[:], OP.mult)
    C2T = _transpose(nc, ps, sp, C2[:], 6, idn, "c2t")

    _solve(nc, pp, sp, ps, psc, cps, idn, sc, C2[:], C2T[:], stage, mshuf,
           out_d, T12)


def _solve(nc, pp, sp, ps, psc, cps, idn, sc, C2, C2T, stage, mshuf, out_d,
           T12):
    i9h = cps[0:9, C_I9H:C_I9H + 9]
    et69 = cps[0:6, C_ET69:C_ET69 + 9]
    i3c = cps[0:3, C_I3:C_I3 + 3]
    v09 = cps[0:9, C_V09:C_V09 + 1]
    v06 = cps[0:6, C_V06:C_V06 + 1]
    sel1 = cps[0:3, C_SEL1:C_SEL1 + 6]
    sel2 = cps[0:3, C_SEL2:C_SEL2 + 6]

    # G2 = E C2 E^T : G2[3a+b, 3c+d] = C2[pair(a,b), pair(c,d)]
    z_ps = ps.tile([6, 9], F32, tag="tps")
    nc.tensor.matmul(z_ps[:], C2T, et69, start=True, stop=True)  # C2 E^T
    Zs = sp.tile([6, 9], F32, tag="Zs")
    nc.vector.tensor_copy(Zs[:], z_ps[:])
    g_ps = ps.tile([9, 9], F32, tag="tps")
    nc.tensor.matmul(g_ps[:], et69, Zs[:], start=True, stop=True)    # E @ Z
    G2 = sp.tile([9, 9], F32, tag="G2")
    nc.vector.tensor_copy(G2[:], g_ps[:])

    # Mmat[3p+q, 3r+s] = G2[3p+r, 3q+s]: bounce via DRAM
    nc.sync.dma_start(mshuf[:], G2[:])
    Mmat = sp.tile([9, 9], F32, tag="Mmat")
    for p in range(3):
        eng = nc.scalar if p == 1 else nc.sync
        eng.dma_start(
            Mmat[3 * p:3 * p + 3, :].rearrange("q (r s) -> q r s", s=3),
            mshuf[:].rearrange("(p q1 r s) -> p q1 r s", p=3, q1=3, r=3)
            .transpose([0, 2, 1, 3])[p])

    # Msp = Mmat/(2 lam) - I/2
    dg = sp.tile([9, 9], F32, tag="dg")
    nc.vector.tensor_tensor(dg[:], Mmat[:], i9h, OP.mult)
    lam2 = sp.tile([9, 1], F32, tag="lam2")
    nc.vector.tensor_reduce(lam2[:], dg[:], AX.X, OP.add)
    ones99 = cps[0:9, C_ONE:C_ONE + 9]
    lam2r = ps.tile([9, 1], F32, tag="tps")
    nc.tensor.matmul(lam2r[:], ones99, lam2[:], start=True, stop=True)
    lam4 = sp.tile([9, 1], F32, tag="lam4")
    nc.vector.tensor_scalar_mul(lam4[:], lam2r[:], 4.0)
    inv2l = sp.tile([9, 1], F32, tag="inv2l")
    nc.vector.reciprocal(inv2l[:], lam4[:])
    Msp = sp.tile([9, 9], F32, tag="Msp")
    nc.vector.scalar_tensor_tensor(Msp[:], Mmat[:], inv2l[:], i9h,
                                   OP.mult, OP.subtract)
    M50 = _powchain(nc, ps, sp, Msp[:], 9, "m9", 5, extra=True)

    w9ps = ps.tile([1, 9], F32, tag="tps")
    nc.tensor.matmul(w9ps[:], v09, M50[:], start=True, stop=True)
    w9 = sp.tile([1, 9], F32, tag="w9")
    nc.vector.tensor_copy(w9[:], w9ps[:])
    nc.sync.dma_start(stage[24:33], w9[:])  # raw; 1/||w9|| folded at the end
    w9sq = sp.tile([1, 9], F32, tag="w9sq")
    nc.vector.tensor_tensor(w9sq[:], w9[:], w9[:], OP.mult)
    nn9 = sp.tile([1, 1], F32, tag="nn9")
    nc.vector.tensor_reduce(nn9[:], w9sq[:], AX.X, OP.add)
    sr9 = sp.tile([1, 1], F32, tag="sr9")
    nc.scalar.activation(sr9[:], nn9[:], AF.Sqrt)
    rs9 = sp.tile([1, 1], F32, tag="rs9")
    nc.vector.reciprocal(rs9[:], sr9[:])
    rs9c = psc.tile([3, 1], F32, tag="rs9c")
    nc.tensor.matmul(rs9c[:], cps[0:1, C_ONE:C_ONE + 3], rs9[:],
                     start=True, stop=True)

    # E = T2^T E_raw T1 (and E^T);  T1m/T2m preloaded in T12
    T1m = T12[:, 0:3]
    T2m = T12[:, 3:6]
    Eraw = sp.tile([3, 3], F32, tag="Eraw")
    nc.sync.dma_start(Eraw[:], stage[24:33].rearrange("(i j) -> i j", j=3))

    a1ps = ps.tile([3, 3], F32, tag="tps")
    nc.tensor.matmul(a1ps[:], T2m, Eraw[:], start=True, stop=True)
    A1 = sp.tile([3, 3], F32, tag="A1")
    nc.vector.tensor_copy(A1[:], a1ps[:])
    A1T = _transpose(nc, ps, sp, A1[:], 3, idn, "a1t")
    etps = ps.tile([3, 3], F32, tag="tps")
    nc.tensor.matmul(etps[:], T1m, A1T[:], start=True, stop=True)
    ETs = sp.tile([3, 3], F32, tag="ETs")
    nc.vector.tensor_copy(ETs[:], etps[:])
    Es = _transpose(nc, ps, sp, ETs[:], 3, idn, "es")

    # B = E^T E ; blockdiag 6x6 chain (32 iters) for v1 (max) and v3 (min)
    bps = ps.tile([3, 3], F32, tag="tps")
    nc.tensor.matmul(bps[:], Es[:], Es[:], start=True, stop=True)
    Bm = sp.tile([3, 3], F32, tag="Bm")
    nc.vector.tensor_copy(Bm[:], bps[:])
    dg3 = sp.tile([3, 3], F32, tag="dg3")
    nc.vector.tensor_tensor(dg3[:], Bm[:], i3c, OP.mult)
    lb = sp.tile([3, 1], F32, tag="lb")
    nc.vector.tensor_reduce(lb[:], dg3[:], AX.X, OP.add)
    lbr = ps.tile([3, 1], F32, tag="tps")
    nc.tensor.matmul(lbr[:], cps[0:3, C_ONE:C_ONE + 3], lb[:],
                     start=True, stop=True)
    invlb = sp.tile([3, 1], F32, tag="invlb")
    nc.vector.reciprocal(invlb[:], lbr[:])
    Bs3 = sp.tile([3, 3], F32, tag="Bs3")
    nc.vector.tensor_scalar_mul(Bs3[:], Bm[:], invlb[:])
    IB = sp.tile([3, 3], F32, tag="IB")
    nc.vector.tensor_tensor(IB[:], i3c, Bs3[:], OP.subtract)
    bdps = ps.tile([6, 6], F32, tag="tps")
    nc.tensor.matmul(bdps[:, 0:3], sel1, Bs3[:], start=True, stop=True)
    nc.tensor.matmul(bdps[:, 3:6], sel2, IB[:], start=True, stop=True)
    BD = sp.tile([6, 6], F32, tag="BD")
    nc.vector.tensor_copy(BD[:], bdps[:])
    BD32 = _powchain(nc, ps, sp, BD[:], 6, "m6", 5, extra=False)

    w6ps = ps.tile([1, 6], F32, tag="tps")
    nc.tensor.matmul(w6ps[:], v06, BD32[:], start=True, stop=True)
    w6 = sp.tile([1, 6], F32, tag="w6")
    nc.vector.tensor_copy(w6[:], w6ps[:])
    w6sq = sp.tile([1, 6], F32, tag="w6sq")
    nc.vector.tensor_tensor(w6sq[:], w6[:], w6[:], OP.mult)
    nn6 = sp.tile([1, 2], F32, tag="nn6")
    nc.vector.tensor_reduce(nn6[:].unsqueeze(2),
                            w6sq[:].rearrange("p (g d) -> p g d", g=2), AX.X,
                            OP.add)
    sr6 = sp.tile([1, 2], F32, tag="sr6")
    nc.scalar.activation(sr6[:], nn6[:], AF.Sqrt)
    rs6 = sp.tile([1, 2], F32, tag="rs6")
    nc.vector.reciprocal(rs6[:], sr6[:])
    vv = sp.tile([1, 6], F32, tag="vv")
    nc.vector.tensor_tensor(
        vv[:].rearrange("p (g d) -> p g d", g=2),
        w6[:].rearrange("p (g d) -> p g d", g=2),
        rs6[:].unsqueeze(2).to_broadcast([1, 2, 3]), OP.mult)

    # v2 = cross(v3, v1), normalized with EPS
    aa = sp.tile([1, 6], F32, tag="aa")
    nc.vector.tensor_copy(
        aa[:].rearrange("p (r d) -> p r d", r=2),
        vv[:, 3:6].unsqueeze(1).to_broadcast([1, 2, 3]))
    bb = sp.tile([1, 6], F32, tag="bb")
    nc.vector.tensor_copy(
        bb[:].rearrange("p (r d) -> p r d", r=2),
        vv[:, 0:3].unsqueeze(1).to_broadcast([1, 2, 3]))
    cr1 = sp.tile([1, 3], F32, tag="cr1")
    nc.vector.tensor_tensor(cr1[:], aa[:, 1:4], bb[:, 2:5], OP.mult)
    cr2 = sp.tile([1, 3], F32, tag="cr2")
    nc.vector.tensor_tensor(cr2[:], aa[:, 2:5], bb[:, 1:4], OP.mult)
    v2r = sp.tile([1, 3], F32, tag="v2r")
    nc.vector.tensor_tensor(v2r[:], cr1[:], cr2[:], OP.subtract)
    v2sq = sp.tile([1, 3], F32, tag="v2sq")
    nc.vector.tensor_tensor(v2sq[:], v2r[:], v2r[:], OP.mult)
    nn2 = sp.tile([1, 1], F32, tag="nn2")
    nc.vector.tensor_reduce(nn2[:], v2sq[:], AX.X, OP.add)
    sr2 = sp.tile([1, 1], F32, tag="sr2")
    nc.scalar.activation(sr2[:], nn2[:], AF.Sqrt)
    sr2e = sp.tile([1, 1], F32, tag="sr2e")
    nc.vector.tensor_scalar_add(sr2e[:], sr2[:], EPS)
    rs2 = sp.tile([1, 1], F32, tag="rs2")
    nc.vector.reciprocal(rs2[:], sr2e[:])
    v2 = sp.tile([1, 3], F32, tag="v2")
    nc.vector.tensor_tensor(v2[:], v2r[:], rs2[:].to_broadcast([1, 3]), OP.mult)

    vvv = sp.tile([1, 6], F32, tag="vvv")
    nc.vector.tensor_copy(vvv[:, 0:3], vv[:, 0:3])
    nc.vector.tensor_copy(vvv[:, 3:6], v2[:])
    nc.sync.dma_start(stage[33:39], vvv[:])
    Vr = sp.tile([2, 3], F32, tag="Vr")
    nc.sync.dma_start(Vr[:], stage[33:39].rearrange("(i k) -> i k", k=3))
    Vc = sp.tile([3, 2], F32, tag="Vc")
    nc.scalar.dma_start(Vc[:], stage[33:39].rearrange("(i k) -> k i", k=3))
    evps = ps.tile([2, 3], F32, tag="tps")
    nc.tensor.matmul(evps[:], Vc[:], ETs[:], start=True, stop=True)
    Evr = sp.tile([2, 3], F32, tag="Evr")
    nc.vector.tensor_copy(Evr[:], evps[:])
    evsq = sp.tile([2, 3], F32, tag="evsq")
    nc.vector.tensor_tensor(evsq[:], Evr[:], Evr[:], OP.mult)
    ss2 = sp.tile([2, 1], F32, tag="ss2")
    nc.vector.tensor_reduce(ss2[:], evsq[:], AX.X, OP.add)
    sv = sp.tile([2, 1], F32, tag="sv")
    nc.scalar.activation(sv[:], ss2[:], AF.Sqrt)
    ssum = ps.tile([2, 1], F32, tag="tps")
    nc.tensor.matmul(ssum[:], cps[0:2, C_ONE:C_ONE + 2], sv[:],
                     start=True, stop=True)
    savg = sp.tile([2, 1], F32, tag="savg")
    nc.vector.tensor_scalar_mul(savg[:], ssum[:], 0.5)
    sve = sp.tile([2, 1], F32, tag="sve")
    nc.vector.tensor_scalar_add(sve[:], sv[:], EPS)
    rsv = sp.tile([2, 1], F32, tag="rsv")
    nc.vector.reciprocal(rsv[:], sve[:])
    f2 = sp.tile([2, 1], F32, tag="f2")
    nc.vector.tensor_tensor(f2[:], rsv[:], savg[:], OP.mult)
    U2 = sp.tile([2, 3], F32, tag="U2")
    nc.vector.tensor_scalar_mul(U2[:], Evr[:], f2[:])
    ops_ = ps.tile([3, 3], F32, tag="tps")
    nc.tensor.matmul(ops_[:], U2[:], Vr[:], start=True, stop=True)
    outs = sp.tile([3, 3], F32, tag="outs")
    nc.vector.tensor_scalar_mul(outs[:], ops_[:], rs9c[:])
    nc.sync.dma_start(out_d[:], outs[:])


def make_in_maps(P, K):
    P = np.asarray(P, np.float32)
    K = np.asarray(K, np.float32)
    Pc = np.ascontiguousarray(P[:N, :N])
    PcT = np.ascontiguousarray(Pc.T)
    Mp, cpack, c0x, c0y = host_constants(K)
    m1full = _tile128(Mp, CB)
    c0t = np.array([[c0x, c0x, c0y, c0y]], np.float32)
    in_maps = []
    for k in range(NCORES):
        in_maps.append({
            "xn": _tile128(Pc[k * SH:(k + 1) * SH], RT),
            "xc": _tile128(PcT[k * SH:(k + 1) * SH], RT),
            "m1f": m1full,
            "m2s": _tile128(Mp[k * SH:(k + 1) * SH], RT),
            "cpack": cpack,
            "c0t": c0t,
        })
    return in_maps


_NC_CACHE = {}


def kernel(P, K):
    from concourse.bass_utils import run_bass_kernel_spmd
    if "nc" not in _NC_CACHE:
        _NC_CACHE["nc"] = build_nc()
    nc = _NC_CACHE["nc"]
    in_maps = make_in_maps(P, K)
    res = run_bass_kernel_spmd(nc, in_maps, core_ids=list(range(NCORES)))
    return np.asarray(res.results[0]["out"], np.float32)
